# revision 1
# baseline (speedup 1.0000x reference)
"""GATv2 (2-layer, GAT_56727928046275) on 8 TRN2 NeuronCores.

Strategy:
  - Nodes sharded by destination across 8 cores (6250 dst nodes each).
  - Edges (incl. self-loops) partitioned by dst, grouped into 128-dst
    "windows"; per-node softmax + scatter-add stay core-local via
    per-window PSUM accumulation with indicator matmuls.
  - xl/xr node tables stored in DRAM (bf16, |att|-prescaled and
    sign-permuted per head so per-edge logits become plain sign-split
    reductions); per-edge rows fetched with gpsimd dma_gather.
  - int16 gather indices -> tables split at row 25088, two gathers.
  - BatchNorm stats via masked ones-matmul column sums + AllReduce;
    layer-1 xl table assembled with AllGather.
"""

import numpy as np
import ml_dtypes

import concourse.bass as bass
import concourse.mybir as mybir
import concourse.tile as tile
from concourse.bacc import Bacc
from concourse.bass_utils import run_bass_kernel_spmd

# The ucode's 256B elem_size restriction applies only to transpose-mode
# gathers; relax the bass-level assert so non-transpose gathers can move
# sub-row payloads (row stride stays 256B-aligned via elem_step).
import inspect as _insp
import textwrap as _tw
_src = _tw.dedent(_insp.getsource(bass.BassGpSimd.dma_gather))
_src = _src.replace("elem_size_bytes > 0 and elem_size_bytes % 256 == 0",
                    "elem_size_bytes > 0")
_ns = dict(bass.__dict__)
exec(_src, _ns)
bass.BassGpSimd.dma_gather = _ns["dma_gather"]

BF = mybir.dt.bfloat16
F32 = mybir.dt.float32
I16 = mybir.dt.int16
OP = mybir.AluOpType
AF = mybir.ActivationFunctionType
bf16 = ml_dtypes.bfloat16

NEG_SLOPE = 0.2
BN_EPS = 1e-5


# ---------------------------------------------------------------- host prep

def _wrap16(a):
    """idx array [n] (n % 16 == 0) -> [128, n//16] int16 (16-wrap, x8 rep)."""
    a = np.asarray(a, np.int16).reshape(-1, 16).T  # [16, n/16]
    return np.tile(a, (8, 1))


def _rep(v, dt=np.float32):
    v = np.asarray(v, dt).reshape(1, -1)
    return np.ascontiguousarray(np.broadcast_to(v, (128, v.shape[1])))


def _sign_perm(att):
    """Per-head permutation putting att>=0 channels first.
    att: [H, C]. Returns perm [H*C] (flat), npos [H]."""
    H, C = att.shape
    perm = np.zeros(H * C, np.int64)
    npos = np.zeros(H, np.int64)
    for h in range(H):
        a = att[h]
        pos = np.nonzero(a >= 0)[0]
        neg = np.nonzero(a < 0)[0]
        perm[h * C:(h + 1) * C] = h * C + np.concatenate([pos, neg])
        npos[h] = len(pos)
    return perm, npos


def preprocess(x, edge_index, params, n_cores=8):
    """Build per-core input maps + program metadata."""
    N, IN = x.shape
    SLICE = N // n_cores
    SP = ((SLICE + 127) // 128) * 128          # padded slice rows (6272)
    W = SP // 128                              # windows per core (49)
    BLK = (n_cores // 2) * SP                  # int16 table split row (25088)

    src = np.concatenate([edge_index[0], np.arange(N, dtype=np.int64)])
    dst = np.concatenate([edge_index[1], np.arange(N, dtype=np.int64)])
    core = dst // SLICE
    # table row for node n in the SP-padded concat layout
    row = (src // SLICE) * SP + (src % SLICE)

    # per-core, per-window, per-block edge lists
    ecnt0 = np.zeros((n_cores, W), np.int64)
    ecnt1 = np.zeros((n_cores, W), np.int64)
    lists = [[None] * W for _ in range(n_cores)]
    for k in range(n_cores):
        m = core == k
        r, d = row[m], dst[m] - k * SLICE
        win = d // 128
        blk = (r >= BLK).astype(np.int64)
        order = np.lexsort((blk, win))
        r, d, win, blk = r[order], d[order], win[order], blk[order]
        for w in range(W):
            wm = win == w
            rw, dw, bw = r[wm], d[wm], blk[wm]
            b0 = bw == 0
            lists[k][w] = (rw[b0], dw[b0], rw[~b0], dw[~b0])
            ecnt0[k][w] = int(b0.sum())
            ecnt1[k][w] = int((~b0).sum())

    G0 = np.maximum(1, (ecnt0.max(0) + 127) // 128)   # [W]
    G1 = np.maximum(1, (ecnt1.max(0) + 127) // 128)
    GW = G0 + G1
    NG = int(GW.sum())

    # padded per-core arrays
    src_cols = int((GW * 8).sum())
    srcw = np.zeros((n_cores, 128, src_cols), np.int16)
    drel = np.full((n_cores, 128, NG), -1.0, np.float32)
    swin = np.zeros((n_cores, 128, NG * 128), bf16)
    salh = np.zeros((n_cores, 128, NG * 128), bf16)
    for k in range(n_cores):
        sc = 0
        gc = 0
        for w in range(W):
            r0, d0, r1, d1 = lists[k][w]
            n0, n1 = len(r0), len(r1)
            s0 = np.zeros(int(G0[w]) * 128, np.int64)
            s0[:n0] = r0
            s1 = np.zeros(int(G1[w]) * 128, np.int64)
            s1[:n1] = r1 - BLK
            dd = np.zeros(int(GW[w]) * 128, np.int64)
            dd[:n0] = d0
            dd[int(G0[w]) * 128:int(G0[w]) * 128 + n1] = d1
            rl = np.full(int(GW[w]) * 128, -1.0, np.float32)
            rl[:n0] = (d0 - w * 128).astype(np.float32)
            rl[int(G0[w]) * 128:int(G0[w]) * 128 + n1] = (
                d1 - w * 128).astype(np.float32)
            srcw[k][:, sc:sc + int(G0[w]) * 8] = _wrap16(s0)
            srcw[k][:, sc + int(G0[w]) * 8:sc + int(GW[w]) * 8] = _wrap16(s1)
            drel[k][:, gc:gc + int(GW[w])] = rl.reshape(int(GW[w]), 128).T
            # node-major indicator for xr expansion: [n, g*128+e]
            sw = (np.arange(128)[:, None] == rl[None, :]).astype(bf16)
            swin[k][:, gc * 128:(gc + int(GW[w])) * 128] = sw
            # edge-major indicator for the scatter matmul lhsT: [e%128, g, n]
            se = (rl.reshape(int(GW[w]), 128).T[:, :, None]
                  == np.arange(128)[None, None, :]).astype(bf16)
            salh[k][:, gc * 128:(gc + int(GW[w])) * 128] = se.reshape(
                128, -1)
            sc += int(GW[w]) * 8
            gc += int(GW[w])

    p = params
    H, C = p["att0"].shape
    OUTC = p["att1"].shape[1]
    perm0, npos0 = _sign_perm(np.asarray(p["att0"]))
    perm1, npos1 = _sign_perm(np.asarray(p["att1"]))
    aab0 = np.abs(np.asarray(p["att0"]).reshape(-1)[perm0])   # [192]
    aab1 = np.abs(np.asarray(p["att1"]).reshape(-1)[perm1])   # [64]

    def scale_cols(Wm, b, perm, aab):
        Wp = np.asarray(Wm)[:, perm] * aab[None, :]
        bp = np.asarray(b)[perm] * aab
        return Wp, bp

    Wl0p, bl0p = scale_cols(p["Wl0"], p["bl0"], perm0, aab0)
    Wr0p, br0p = scale_cols(p["Wr0"], p["br0"], perm0, aab0)
    # layer 1 weights: rows permuted by perm0 (h channel order), cols by perm1
    Wl1p, bl1p = scale_cols(np.asarray(p["Wl1"])[perm0, :], p["bl1"],
                            perm1, aab1)
    Wr1p, br1p = scale_cols(np.asarray(p["Wr1"])[perm0, :], p["br1"],
                            perm1, aab1)

    FEAT0 = H * C            # 192
    TBL0 = 256               # bf16 cols -> 512B rows
    TBL1 = 128               # bf16 cols -> 256B rows

    shared = {
        "Wl0p": Wl0p.astype(bf16), "Wr0p": Wr0p.astype(bf16),
        "BL0": _rep(bl0p, bf16), "BR0": _rep(br0p, bf16),
        "Wl1a": Wl1p[:128].astype(bf16), "Wl1b": Wl1p[128:].astype(bf16),
        "Wr1a": Wr1p[:128].astype(bf16), "Wr1b": Wr1p[128:].astype(bf16),
        "BL1": _rep(bl1p), "BR1": _rep(br1p),
        "RA0": _rep(1.0 / aab0), "RA1": _rep(1.0 / aab1),
        "BIAS0": _rep(np.asarray(p["bias0"])[perm0]),
        "BIAS1": _rep(np.asarray(p["bias1"])[perm1]),
        "gamma0": np.asarray(p["gamma0"])[perm0].reshape(1, -1).astype(
            np.float32),
        "beta0": np.asarray(p["beta0"])[perm0].reshape(1, -1).astype(
            np.float32),
        "gamma1": np.asarray(p["gamma1"])[perm1].reshape(1, -1).astype(
            np.float32),
        "beta1": np.asarray(p["beta1"])[perm1].reshape(1, -1).astype(
            np.float32),
        "IDENT": np.eye(128, dtype=bf16),
        "ONE1": np.ones((1, 128), np.float32),
    }
    shared["IOTA"] = np.tile(np.arange(128, dtype=bf16).reshape(1, -1),
                             (128, 1))
    Gmax = int(GW.max())
    shared["IOTAT"] = np.tile(np.arange(128, dtype=bf16).reshape(1, -1),
                              (128, Gmax))
    meta_gmax = Gmax
    # stats masks: [128, 2] col0 for w<W-1 (all valid), col1 for last window
    mlast = np.zeros(128, np.float32)
    mlast[:SLICE - (W - 1) * 128] = 1.0
    shared["MASKS"] = np.stack(
        [np.ones(128, np.float32), mlast], 1).astype(bf16)

    in_maps = []
    xt = np.asarray(x).T  # [IN, N]
    for k in range(n_cores):
        xk = np.zeros((IN, SP), np.float32)
        xk[:, :SLICE] = xt[:, k * SLICE:(k + 1) * SLICE]
        m = {"xT": xk.astype(bf16),
             "srcw": srcw[k], "swin": swin[k], "salh": salh[k],
             "drel": drel[k].astype(bf16)}
        for kk, v in shared.items():
            m[kk] = np.ascontiguousarray(v)
        in_maps.append(m)

    meta = dict(N=N, IN=IN, SLICE=SLICE, SP=SP, W=W, BLK=BLK,
                G0=[int(v) for v in G0], G1=[int(v) for v in G1],
                GW=[int(v) for v in GW], NG=NG, src_cols=src_cols,
                H=H, C=C, FEAT0=FEAT0, OUTC=OUTC, TBL0=TBL0, TBL1=TBL1,
                Gmax=meta_gmax,
                npos0=[int(v) for v in npos0], npos1=[int(v) for v in npos1],
                n_cores=n_cores)
    return in_maps, meta, perm1


# ---------------------------------------------------------------- program

def build_program(meta):
    n_cores = meta["n_cores"]
    IN, SP, W = meta["IN"], meta["SP"], meta["W"]
    G0, G1, GW, NG = meta["G0"], meta["G1"], meta["GW"], meta["NG"]
    H, C, FEAT0 = meta["H"], meta["C"], meta["FEAT0"]
    OUTC, TBL0, TBL1 = meta["OUTC"], meta["TBL0"], meta["TBL1"]
    SLICE = meta["SLICE"]
    BLKROW = (n_cores // 2) * SP
    Gmax = max(GW)
    cores = list(range(n_cores))

    nc = Bacc("TRN2", target_bir_lowering=False, debug=False,
              num_devices=n_cores, num_swdge_queues=4)

    def dp(name, shape, dt):
        return nc.declare_dram_parameter(name, list(shape), dt,
                                         isOutput=False)

    xT = dp("xT", [IN, SP], BF)
    srcw = dp("srcw", [128, meta["src_cols"]], I16)
    swin = dp("swin", [128, NG * 128], BF)
    salh = dp("salh", [128, NG * 128], BF)
    drel = dp("drel", [128, NG], BF)
    Wl0p = dp("Wl0p", [IN, FEAT0], BF)
    Wr0p = dp("Wr0p", [IN, FEAT0], BF)
    BL0 = dp("BL0", [128, FEAT0], BF)
    BR0 = dp("BR0", [128, FEAT0], BF)
    Wl1a = dp("Wl1a", [128, OUTC], BF)
    Wl1b = dp("Wl1b", [FEAT0 - 128, OUTC], BF)
    Wr1a = dp("Wr1a", [128, OUTC], BF)
    Wr1b = dp("Wr1b", [FEAT0 - 128, OUTC], BF)
    BL1 = dp("BL1", [128, OUTC], F32)
    BR1 = dp("BR1", [128, OUTC], F32)
    RA0 = dp("RA0", [128, FEAT0], F32)
    RA1 = dp("RA1", [128, OUTC], F32)
    BIAS0 = dp("BIAS0", [128, FEAT0], F32)
    BIAS1 = dp("BIAS1", [128, OUTC], F32)
    gamma0 = dp("gamma0", [1, FEAT0], F32)
    beta0 = dp("beta0", [1, FEAT0], F32)
    gamma1 = dp("gamma1", [1, OUTC], F32)
    beta1 = dp("beta1", [1, OUTC], F32)
    IOTA = dp("IOTA", [128, 128], BF)
    IOTAT = dp("IOTAT", [128, meta["Gmax"] * 128], BF)
    IDENT = dp("IDENT", [128, 128], BF)
    ONE1 = dp("ONE1", [1, 128], F32)
    MASKS = dp("MASKS", [128, 2], BF)

    out = nc.declare_dram_parameter("out", [SLICE, OUTC], F32, isOutput=True)

    # internal DRAM
    XL0s = nc.dram_tensor("XL0s", [SP, TBL0], BF)
    XR0 = nc.dram_tensor("XR0", [SP, TBL0], BF)
    XL0 = nc.dram_tensor("XL0", [n_cores * SP, TBL0], BF, addr_space="Shared")
    XL1s = nc.dram_tensor("XL1s", [SP, TBL1], BF)
    XR1 = nc.dram_tensor("XR1", [SP, TBL1], BF)
    XL1 = nc.dram_tensor("XL1", [n_cores * SP, TBL1], BF, addr_space="Shared")
    HP0 = nc.dram_tensor("HP0", [SP, FEAT0], BF)
    HP1 = nc.dram_tensor("HP1", [SP, OUTC], BF)
    ST0i = nc.dram_tensor("ST0i", [1, 2 * FEAT0], F32)
    ST0o = nc.dram_tensor("ST0o", [1, 2 * FEAT0], F32, addr_space="Shared")
    ST1i = nc.dram_tensor("ST1i", [1, 2 * OUTC], F32)
    ST1o = nc.dram_tensor("ST1o", [1, 2 * OUTC], F32, addr_space="Shared")

    with tile.TileContext(nc) as tc:
        with tc.tile_pool(name="const", bufs=1) as cp:
            def ld(par, shape, dt):
                t = cp.tile(list(shape), dt, tag=f"c_{par.name}")
                nc.sync.dma_start(out=t[:], in_=par[:])
                return t

            c_iota = ld(IOTA, [128, 128], BF)
            c_iotat = ld(IOTAT, [128, meta["Gmax"] * 128], BF)
            c_ident = ld(IDENT, [128, 128], BF)
            c_one1 = ld(ONE1, [1, 128], F32)
            c_masks = ld(MASKS, [128, 2], BF)
            c_wl0 = ld(Wl0p, [IN, FEAT0], BF)
            c_wr0 = ld(Wr0p, [IN, FEAT0], BF)
            c_bl0 = ld(BL0, [128, FEAT0], BF)
            c_br0 = ld(BR0, [128, FEAT0], BF)
            c_wl1a = ld(Wl1a, [128, OUTC], BF)
            c_wl1b = ld(Wl1b, [FEAT0 - 128, OUTC], BF)
            c_wr1a = ld(Wr1a, [128, OUTC], BF)
            c_wr1b = ld(Wr1b, [FEAT0 - 128, OUTC], BF)
            c_bl1 = ld(BL1, [128, OUTC], F32)
            c_br1 = ld(BR1, [128, OUTC], F32)
            c_ra0 = ld(RA0, [128, FEAT0], F32)
            c_ra1 = ld(RA1, [128, OUTC], F32)
            c_bias0 = ld(BIAS0, [128, FEAT0], F32)
            c_bias1 = ld(BIAS1, [128, OUTC], F32)
            c_g0 = ld(gamma0, [1, FEAT0], F32)
            c_b0 = ld(beta0, [1, FEAT0], F32)
            c_g1 = ld(gamma1, [1, OUTC], F32)
            c_b1 = ld(beta1, [1, OUTC], F32)
            c_src = ld(srcw, [128, meta["src_cols"]], I16)
            c_drel = ld(drel, [128, NG], BF)

            # ---------------- stage A: xl0 / xr0 slice tables ----------
            # XL0s pass first, then its AllGather is issued while the
            # XR0 pass (no dependency on it) computes — overlapping the
            # collective instead of idling every engine behind it.
            with tc.tile_pool(name="sta", bufs=3) as sa, \
                 tc.tile_pool(name="sta_ps", bufs=4, space="PSUM") as sap:
                for (wmat, btile, dram) in ((c_wl0, c_bl0, XL0s),
                                            (c_wr0, c_br0, XR0)):
                    for t in range(W):
                        xt_t = sa.tile([IN, 128], BF, tag="xt")
                        nc.sync.dma_start(out=xt_t[:],
                                          in_=xT[:, t * 128:(t + 1) * 128])
                        ps = sap.tile([128, FEAT0], F32, tag="ps")
                        nc.tensor.matmul(ps[:], xt_t[:], wmat[:],
                                         start=True, stop=True)
                        sb = sa.tile([128, FEAT0], BF, tag="sb")
                        nc.vector.tensor_add(sb[:], ps[:], btile[:])
                        nc.sync.dma_start(
                            out=dram[t * 128:(t + 1) * 128, 0:FEAT0],
                            in_=sb[:])
                    if dram is XL0s:
                        nc.gpsimd.collective_compute(
                            "AllGather", OP.bypass, replica_groups=[cores],
                            ins=[XL0s[:, :]], outs=[XL0[:, :]])

            # ---------------- edge pass helper -------------------------
            def edge_pass(XLt, XRt, tblc, feat, nh, npos, ra, biast,
                          hp_dram, st_in, blkrow):
                """One GATv2 edge pass; writes hpre slice + stats.

                Deferred software pipeline: gathers 2 windows ahead,
                streams 1 ahead, scatter 1 behind the z-chain, finalize
                2 behind — no engine waits on the current window's
                producers. Gathers fetch full table rows (512B descs,
                avoiding the <512B DMA latency penalty).
                """
                ssum = cp.tile([1, 2 * feat], F32, tag=f"ssum{feat}")
                nc.vector.memset(ssum[:], 0.0)
                # window -> start cols in srcw / group space
                scs = [0] * (W + 1)
                gcs = [0] * (W + 1)
                for w in range(W):
                    scs[w + 1] = scs[w] + GW[w] * 8
                    gcs[w + 1] = gcs[w] + GW[w]
                with tc.tile_pool(name="eg", bufs=3) as eg, \
                     tc.tile_pool(name="est", bufs=3) as est, \
                     tc.tile_pool(name="es", bufs=2) as es, \
                     tc.tile_pool(name="eps", bufs=2, space="PSUM") as eps, \
                     tc.tile_pool(name="xrp", bufs=2, space="PSUM") as xrp, \
                     tc.tile_pool(name="sps", bufs=1, space="PSUM") as sps:
                    sp_a = sps.tile([1, feat], F32, tag="spa")
                    sp_b = sps.tile([1, feat], F32, tag="spb")
                    qrr = [0]

                    import os as _os
                    GCAP = int(_os.environ.get("K_GCAP", "8"))
                    LRELU_ACT = bool(int(_os.environ.get("K_LRELU", "0")))

                    def gathers(dst_tile, col0, tbl_ap, idx_col0, n_groups):
                        # SWDGE ring = 1024 desc slots per queue; an op
                        # must fit (8 groups = 1024 idxs max). Spread ops
                        # across the 4 SWDGE queues.
                        done = 0
                        while done < n_groups:
                            cnt = min(GCAP, n_groups - done)
                            nc.gpsimd.dma_gather(
                                dst_tile[:, col0 + done:col0 + done + cnt, :],
                                tbl_ap,
                                c_src[:, idx_col0 + done * 8:
                                      idx_col0 + (done + cnt) * 8],
                                num_idxs=cnt * 128, num_idxs_reg=cnt * 128,
                                elem_size=tblc, elem_step=tblc,
                                queue_num=qrr[0])
                            qrr[0] = (qrr[0] + 1) % 4
                            done += cnt

                    def emit_gather(w):
                        gl = eg.tile([128, GW[w], tblc], BF, tag="gl")
                        gathers(gl, 0, XLt[:, 0:tblc], scs[w], G0[w])
                        gathers(gl, G0[w], XLt[blkrow:, 0:tblc],
                                scs[w] + G0[w] * 8, G1[w])
                        return gl

                    def emit_streams(w):
                        gw, gc = GW[w], gcs[w]
                        xw = est.tile([128, feat], BF, tag="xw")
                        nc.sync.dma_start(
                            out=xw[:],
                            in_=XRt[w * 128:(w + 1) * 128, 0:feat])
                        sw = est.tile([128, gw, 128], BF, tag="sw")
                        nc.sync.dma_start(
                            out=sw[:],
                            in_=swin[:, gc * 128:(gc + gw) * 128])
                        sal = est.tile([128, gw, 128], BF, tag="sal")
                        nc.sync.dma_start(
                            out=sal[:],
                            in_=salh[:, gc * 128:(gc + gw) * 128])
                        return xw, sw, sal

                    def emit_chain(w, gl, xw, sw):
                        """z -> logits -> p -> rhs for window w (PE+DVE+Act).
                        Returns (gl, rhs) for the deferred scatter."""
                        gw = GW[w]
                        z = es.tile([128, gw, feat], BF, tag="z")
                        for b0 in range(0, gw, 2):
                            nb = min(2, gw - b0)
                            xre = xrp.tile([128, 2, 512], F32, tag="xre")
                            for j in range(nb):
                                nc.tensor.matmul(
                                    xre[:, j, 0:feat], sw[:, b0 + j, :],
                                    xw[:], start=True, stop=True)
                            nc.vector.tensor_add(
                                z[:, b0:b0 + nb, :],
                                gl[:, b0:b0 + nb, 0:feat],
                                xre[:, 0:nb, 0:feat])
                            # leaky-relu on the Scalar engine (Prelu takes
                            # alpha as a runtime param), chunk-wise so it
                            # trails the adds instead of serializing
                            # before the reduces.
                            nc.scalar.activation(
                                z[:, b0:b0 + nb, :], z[:, b0:b0 + nb, :],
                                AF.Prelu, alpha=NEG_SLOPE)
                        lp = es.tile([128, gw, nh], BF, tag="lp")
                        ln = es.tile([128, gw, nh], BF, tag="ln")
                        with nc.allow_low_precision(
                                reason="bf16 logit partial sums (<=64 "
                                       "terms, |logit|<~2)"):
                            for h in range(nh):
                                k = npos[h]
                                if k == 0:
                                    nc.vector.memset(lp[:, :, h], 0.0)
                                else:
                                    nc.vector.tensor_reduce(
                                        lp[:, :, h],
                                        z[:, :, h * C:h * C + k],
                                        axis=mybir.AxisListType.X, op=OP.add)
                                if k == C:
                                    nc.vector.memset(ln[:, :, h], 0.0)
                                else:
                                    nc.vector.tensor_reduce(
                                        ln[:, :, h],
                                        z[:, :, h * C + k:(h + 1) * C],
                                        axis=mybir.AxisListType.X, op=OP.add)
                        rhs = es.tile([128, gw, feat + 4], BF, tag="rhs")
                        lg = es.tile([128, gw, nh], BF, tag="lg")
                        nc.vector.tensor_sub(lg[:], lp[:], ln[:])
                        nc.scalar.activation(
                            rhs[:, :, feat:feat + nh], lg[:], AF.Exp)
                        # exp expanded across channels on the Scalar
                        # engine so the big mul below is a plain packed
                        # bf16 TensorTensor (DVE fast-path) instead of a
                        # stride-0-broadcast op pinned at 1x.
                        pe = es.tile([128, gw, nh * C], BF, tag="pe")
                        nc.scalar.activation(
                            pe[:].rearrange("p g (h c) -> p g h c", c=C),
                            lg[:].unsqueeze(3).broadcast_to(
                                [128, gw, nh, C]),
                            AF.Exp)
                        nc.vector.tensor_mul(
                            rhs[:, :, 0:feat], gl[:, :, 0:feat], pe[:])
                        return rhs

                    def emit_scatter(w, sal, rhs):
                        gw = GW[w]
                        ps = eps.tile([128, feat + nh], F32, tag="acc")
                        for g in range(gw):
                            nc.tensor.matmul(
                                ps[:], sal[:, g, :], rhs[:, g, 0:feat + nh],
                                start=(g == 0), stop=(g == gw - 1))
                        return ps

                    def emit_finalize(w, ps):
                        dn = es.tile([128, nh], F32, tag="dn")
                        nc.vector.tensor_scalar_max(
                            dn[:], ps[:, feat:feat + nh], 1e-30)
                        rc = es.tile([128, nh], F32, tag="rc")
                        nc.vector.reciprocal(rc[:], dn[:])
                        hp = es.tile([128, feat], BF, tag="hp")
                        tmp = es.tile([128, feat], F32, tag="tmp")
                        for h in range(nh):
                            nc.vector.scalar_tensor_tensor(
                                tmp[:, h * C:(h + 1) * C],
                                ps[:, h * C:(h + 1) * C],
                                rc[:, h:h + 1],
                                ra[:, h * C:(h + 1) * C],
                                op0=OP.mult, op1=OP.mult)
                        nc.vector.tensor_add(tmp[:], tmp[:], biast[:])
                        # relu/square on DVE: keeps the Scalar queue free
                        # of cross-window deps (its queue head otherwise
                        # stalls next-window prelus on this relu).
                        nc.vector.tensor_scalar_max(hp[:], tmp[:], 0.0)
                        nc.sync.dma_start(
                            out=hp_dram[w * 128:(w + 1) * 128, :],
                            in_=hp[:])
                        # stats
                        sq = es.tile([128, feat], BF, tag="sq")
                        nc.vector.tensor_mul(sq[:], hp[:], hp[:])
                        mc = 0 if w < W - 1 else 1
                        nc.tensor.matmul(sp_a[:], c_masks[:, mc:mc + 1],
                                         hp[:], start=(w == 0),
                                         stop=(w == W - 1))
                        nc.tensor.matmul(sp_b[:], c_masks[:, mc:mc + 1],
                                         sq[:], start=(w == 0),
                                         stop=(w == W - 1))

                    # Deferred pipeline: at iteration i —
                    #   gathers(i) | streams(i-1) | chain(i-2) |
                    #   scatter(i-3) | finalize(i-4)
                    # so every op is data-ready when its engine reaches
                    # it (PE's scatter(w) runs a full window after
                    # chain(w) produced rhs(w); finalize reads a psum
                    # scattered one iteration earlier).
                    pend_g = {}
                    pend_s = {}
                    pend_r = {}
                    pend_p = {}
                    for i in range(W + 4):
                        if i < W:
                            pend_g[i] = emit_gather(i)
                        if 1 <= i <= W:
                            pend_s[i - 1] = emit_streams(i - 1)
                        if 2 <= i <= W + 1:
                            w = i - 2
                            gl = pend_g.pop(w)
                            xw, sw, sal = pend_s[w]
                            pend_r[w] = emit_chain(w, gl, xw, sw)
                        if 3 <= i <= W + 2:
                            w = i - 3
                            _, _, sal = pend_s.pop(w)
                            pend_p[w] = emit_scatter(w, sal, pend_r.pop(w))
                        if 4 <= i:
                            w = i - 4
                            emit_finalize(w, pend_p.pop(w))
                    nc.vector.tensor_copy(ssum[:, 0:feat], sp_a[:])
                    nc.vector.tensor_copy(ssum[:, feat:2 * feat], sp_b[:])
                nc.sync.dma_start(out=st_in[:, :], in_=ssum[:])

            # ---------------- BN coeff helper --------------------------
            def bn_coeffs(st_out, feat, g_row, b_row, nodes):
                """AllReduced stats -> A_rep/B_rep [128, feat] bf16."""
                st = cp.tile([1, 2 * feat], F32, tag=f"st{feat}")
                nc.sync.dma_start(out=st[:], in_=st_out[:, :])
                mu = cp.tile([1, feat], F32, tag=f"mu{feat}")
                var = cp.tile([1, feat], F32, tag=f"va{feat}")
                nc.vector.tensor_scalar_mul(mu[:], st[:, 0:feat], 1.0 / nodes)
                nc.vector.tensor_scalar_mul(var[:], st[:, feat:2 * feat],
                                            1.0 / nodes)
                t = cp.tile([1, feat], F32, tag=f"t{feat}")
                nc.vector.tensor_mul(t[:], mu[:], mu[:])
                nc.vector.tensor_sub(var[:], var[:], t[:])
                nc.vector.tensor_scalar_add(var[:], var[:], BN_EPS)
                nc.scalar.activation(t[:], var[:], AF.Sqrt)
                nc.vector.reciprocal(t[:], t[:])          # 1/sqrt(var+eps)
                arow = cp.tile([1, feat], F32, tag=f"ar{feat}")
                nc.vector.tensor_mul(arow[:], g_row[:], t[:])
                brow = cp.tile([1, feat], F32, tag=f"br{feat}")
                nc.vector.tensor_mul(t[:], mu[:], arow[:])
                nc.vector.tensor_sub(brow[:], b_row[:], t[:])
                with tc.tile_pool(name="bnp", bufs=2, space="PSUM") as bp:
                    pa = bp.tile([128, feat], F32, tag="pa")
                    nc.tensor.matmul(pa[:], c_one1[:], arow[:],
                                     start=True, stop=True)
                    Ar = cp.tile([128, feat], BF, tag=f"A{feat}")
                    nc.scalar.copy(Ar[:], pa[:])
                    pb2 = bp.tile([128, feat], F32, tag="pb")
                    nc.tensor.matmul(pb2[:], c_one1[:], brow[:],
                                     start=True, stop=True)
                    Br = cp.tile([128, feat], BF, tag=f"B{feat}")
                    nc.scalar.copy(Br[:], pb2[:])
                return Ar, Br

            import os
            PHASES = int(os.environ.get("K_PHASES", "9"))

            # ================= layer 0 =================================
            if PHASES >= 1:
                edge_pass(XL0, XR0, TBL0, FEAT0, H, meta["npos0"], c_ra0,
                          c_bias0, HP0, ST0i, BLKROW)
            if PHASES >= 2:
                nc.gpsimd.collective_compute(
                    "AllReduce", OP.add, replica_groups=[cores],
                    ins=[ST0i[:, :]], outs=[ST0o[:, :]])
                A0, B0 = bn_coeffs(ST0o, FEAT0, c_g0, c_b0, meta["N"])

            # pass 2: BN apply + layer-1 tables
            if PHASES >= 3:
              with tc.tile_pool(name="p2", bufs=3) as p2, \
                 tc.tile_pool(name="p2ps", bufs=2, space="PSUM") as pp:
                for w in range(W):
                    hb = p2.tile([128, FEAT0], BF, tag="hb")
                    nc.sync.dma_start(out=hb[:],
                                      in_=HP0[w * 128:(w + 1) * 128, :])
                    nc.vector.tensor_mul(hb[:], hb[:], A0[:])
                    nc.vector.tensor_add(hb[:], hb[:], B0[:])
                    pt0 = pp.tile([128, 128], BF, tag="pt0")
                    nc.tensor.transpose(pt0[:], hb[:, 0:128], c_ident[:])
                    pt1 = pp.tile([64, 128], BF, tag="pt1")
                    nc.tensor.transpose(pt1[:], hb[:, 128:192],
                                        c_ident[:])
                    ht0 = p2.tile([128, 128], BF, tag="ht0")
                    nc.scalar.copy(ht0[:], pt0[:])
                    ht1 = p2.tile([64, 128], BF, tag="ht1")
                    nc.scalar.copy(ht1[:], pt1[:])
                    for (wa, wb, btile, dram) in (
                            (c_wl1a, c_wl1b, c_bl1, XL1s),
                            (c_wr1a, c_wr1b, c_br1, XR1)):
                        px = pp.tile([128, OUTC], F32, tag="px")
                        nc.tensor.matmul(px[:], ht0[:], wa[:],
                                         start=True, stop=False)
                        nc.tensor.matmul(px[:], ht1[:], wb[:],
                                         start=False, stop=True)
                        xs = p2.tile([128, OUTC], BF, tag="xs")
                        nc.vector.tensor_add(xs[:], px[:], btile[:])
                        nc.sync.dma_start(
                            out=dram[w * 128:(w + 1) * 128, 0:OUTC],
                            in_=xs[:])

            if PHASES >= 4:
                nc.gpsimd.collective_compute(
                    "AllGather", OP.bypass, replica_groups=[cores],
                    ins=[XL1s[:, :]], outs=[XL1[:, :]])

            # ================= layer 1 =================================
            if PHASES >= 5:
                edge_pass(XL1, XR1, TBL1, OUTC, 1, meta["npos1"], c_ra1,
                          c_bias1, HP1, ST1i, BLKROW)
            if PHASES >= 6:
                nc.gpsimd.collective_compute(
                    "AllReduce", OP.add, replica_groups=[cores],
                    ins=[ST1i[:, :]], outs=[ST1o[:, :]])
                A1, B1 = bn_coeffs(ST1o, OUTC, c_g1, c_b1, meta["N"])

            if PHASES >= 6:
              with tc.tile_pool(name="p3", bufs=3) as p3:
                for w in range(W):
                    rows = min(128, SLICE - w * 128)
                    hb = p3.tile([128, OUTC], BF, tag="hb")
                    nc.sync.dma_start(out=hb[:],
                                      in_=HP1[w * 128:(w + 1) * 128, :])
                    ob = p3.tile([128, OUTC], F32, tag="ob")
                    nc.vector.tensor_mul(ob[:], hb[:], A1[:])
                    nc.vector.tensor_add(ob[:], ob[:], B1[:])
                    nc.sync.dma_start(
                        out=out[w * 128:w * 128 + rows, :],
                        in_=ob[0:rows, :])
            if PHASES < 6:
                with tc.tile_pool(name="p4", bufs=2) as p4:
                    dbg = p4.tile([128, OUTC], F32, tag="dbg")
                    nc.vector.memset(dbg[:], 1.0)
                    for w in range(W):
                        rows = min(128, SLICE - w * 128)
                        nc.sync.dma_start(
                            out=out[w * 128:w * 128 + rows, :],
                            in_=dbg[0:rows, :])

    nc.compile()
    return nc


# ---------------------------------------------------------------- entry

def kernel(**inputs):
    x = np.asarray(inputs["x"])
    edge_index = np.asarray(inputs["edge_index"])
    params = {k: np.asarray(v) for k, v in inputs.items()
              if k not in ("x", "edge_index")}
    n_cores = 8
    in_maps, meta, perm1 = preprocess(x, edge_index, params, n_cores)
    nc = build_program(meta)
    import os
    trace = bool(int(os.environ.get("K_TRACE", "0")))
    res = run_bass_kernel_spmd(nc, in_maps, list(range(n_cores)),
                               trace=trace)
    global LAST_RES
    LAST_RES = res
    if trace:
        print(f"HW exec time: {res.exec_time_ns} ns", flush=True)
    outs = [res.results[k]["out"] for k in range(n_cores)]
    full = np.concatenate(outs, 0)
    inv = np.argsort(perm1)
    return np.ascontiguousarray(full[:, inv]).astype(np.float32)



# revision 31
# speedup vs baseline: 1.0141x; 1.0141x over previous
"""GATv2 (2-layer, GAT_56727928046275) on 8 TRN2 NeuronCores — v3.

Strategy:
  - Nodes sharded by destination across 8 cores (6250 dst nodes each).
  - Edges (incl. self-loops) partitioned by dst, grouped into 128-dst
    "windows"; per-node softmax + scatter-add stay core-local via
    per-window PSUM accumulation with indicator matmuls.
  - Layer-0 xl table computed LOCALLY IN FULL on every core from a
    replicated x (no AllGather on the critical path); layer-1 table
    still AllGathered (activations are distributed).
  - xl tables carry NO bias: both linear biases are folded into the
    xr side (logits) and the finalize bias (scatter output).
  - Per-edge xl rows fetched with gpsimd dma_gather, full-row payloads
    (512B / 256B).  SWDGE desc-gen (~4.5ns/desc, Pool engine) is the
    floor; ops are 1024-desc, 4-queue round-robin, window-batched on
    layer 1.
  - z = xl[src]+xr[dst] built on the PE (indicator matmul + identity
    accumulate into PSUM); leaky-relu on Scalar (PSUM -> packed bf16);
    DVE does reduces + softmax muls; scatter via indicator matmuls.
  - xr/h tables SBUF-resident between phases; BN stats via masked
    ones-matmul column sums + AllReduce.
"""

import numpy as np
import ml_dtypes

import concourse.bass as bass
import concourse.mybir as mybir
import concourse.tile as tile
from concourse.bacc import Bacc
from concourse.bass_utils import run_bass_kernel_spmd

BF = mybir.dt.bfloat16
F32 = mybir.dt.float32
I16 = mybir.dt.int16
OP = mybir.AluOpType
AF = mybir.ActivationFunctionType
bf16 = ml_dtypes.bfloat16

NEG_SLOPE = 0.2
BN_EPS = 1e-5

import os as _os
B0 = 1   # window batch for layer-0 gathers
B1 = 4   # window batch for layer-1 gathers
# groups per gather op; 8 = 1024 idxs = full SWDGE ring. Smaller ops let
# descriptor GENERATION of op N+1 overlap ring DRAIN of op N.
GCAP = int(_os.environ.get("K_GCAP", "8"))


# ---------------------------------------------------------------- host prep

def _wrap16(a):
    a = np.asarray(a, np.int16).reshape(-1, 16).T  # [16, n/16]
    return np.tile(a, (8, 1))


def _rep(v, dt=np.float32):
    v = np.asarray(v, dt).reshape(1, -1)
    return np.ascontiguousarray(np.broadcast_to(v, (128, v.shape[1])))


def _sign_perm(att):
    H, C = att.shape
    perm = np.zeros(H * C, np.int64)
    npos = np.zeros(H, np.int64)
    for h in range(H):
        a = att[h]
        pos = np.nonzero(a >= 0)[0]
        neg = np.nonzero(a < 0)[0]
        perm[h * C:(h + 1) * C] = h * C + np.concatenate([pos, neg])
        npos[h] = len(pos)
    return perm, npos


def _mkbatches(W, B):
    return [list(range(s, min(s + B, W))) for s in range(0, W, B)]


def preprocess(x, edge_index, params, n_cores=8):
    N, IN = x.shape
    SLICE = N // n_cores
    SP = ((SLICE + 127) // 128) * 128          # padded slice rows (6272)
    W = SP // 128                              # windows per core (49)
    NP = n_cores * SP
    BLK = (n_cores // 2) * SP                  # int16 table split row (25088)

    # PyG's added self-loops are handled by a dense per-window path (no
    # gather descriptors); pre-existing (n,n) edges stay in the lists.
    src = np.asarray(edge_index[0], np.int64)
    dst = np.asarray(edge_index[1], np.int64)
    core = dst // SLICE
    row = (src // SLICE) * SP + (src % SLICE)

    ecnt0 = np.zeros((n_cores, W), np.int64)
    ecnt1 = np.zeros((n_cores, W), np.int64)
    lists = [[None] * W for _ in range(n_cores)]
    for k in range(n_cores):
        m = core == k
        r, d = row[m], dst[m] - k * SLICE
        win = d // 128
        blk = (r >= BLK).astype(np.int64)
        order = np.lexsort((blk, win))
        r, d, win, blk = r[order], d[order], win[order], blk[order]
        for w in range(W):
            wm = win == w
            rw, dw, bw = r[wm], d[wm], blk[wm]
            b0 = bw == 0
            lists[k][w] = (rw[b0], dw[b0], rw[~b0], dw[~b0])
            ecnt0[k][w] = int(b0.sum())
            ecnt1[k][w] = int((~b0).sum())

    G0 = np.maximum(1, (ecnt0.max(0) + 127) // 128)   # [W]
    G1 = np.maximum(1, (ecnt1.max(0) + 127) // 128)
    GW = G0 + G1
    NG = int(GW.sum())

    def build_srcw(batches):
        arr = np.zeros((n_cores, 128, NG * 8), np.int16)
        scs = {}
        sc = 0
        for bi, ws in enumerate(batches):
            scs[bi] = sc
            for k in range(n_cores):
                c = sc
                for w in ws:  # A blocks
                    r0, _, _, _ = lists[k][w]
                    s0 = np.zeros(int(G0[w]) * 128, np.int64)
                    s0[:len(r0)] = r0
                    arr[k][:, c:c + int(G0[w]) * 8] = _wrap16(s0)
                    c += int(G0[w]) * 8
                for w in ws:  # B blocks
                    _, _, r1, _ = lists[k][w]
                    s1 = np.zeros(int(G1[w]) * 128, np.int64)
                    s1[:len(r1)] = r1 - BLK
                    arr[k][:, c:c + int(G1[w]) * 8] = _wrap16(s1)
                    c += int(G1[w]) * 8
            sc += sum(int(GW[w]) for w in ws) * 8
        return arr, scs

    bat0 = _mkbatches(W, B0)
    bat1 = _mkbatches(W, B1)
    srcw0, scs0 = build_srcw(bat0)
    srcw1, scs1 = build_srcw(bat1)

    gcs = [0] * (W + 1)
    for w in range(W):
        gcs[w + 1] = gcs[w] + int(GW[w])
    swin = np.zeros((n_cores, 128, NG * 128), bf16)
    salh = np.zeros((n_cores, 128, NG * 128), bf16)
    for k in range(n_cores):
        for w in range(W):
            r0, d0, r1, d1 = lists[k][w]
            n0, n1 = len(r0), len(r1)
            gw = int(GW[w])
            rl = np.full(gw * 128, -1.0, np.float32)
            rl[:n0] = (d0 - w * 128).astype(np.float32)
            rl[int(G0[w]) * 128:int(G0[w]) * 128 + n1] = (
                d1 - w * 128).astype(np.float32)
            gc = gcs[w]
            sw = (np.arange(128)[:, None] == rl[None, :]).astype(bf16)
            swin[k][:, gc * 128:(gc + gw) * 128] = sw
            se = (rl.reshape(gw, 128).T[:, :, None]
                  == np.arange(128)[None, None, :]).astype(bf16)
            salh[k][:, gc * 128:(gc + gw) * 128] = se.reshape(128, -1)

    p = params
    H, C = p["att0"].shape
    OUTC = p["att1"].shape[1]
    perm0, npos0 = _sign_perm(np.asarray(p["att0"]))
    perm1, npos1 = _sign_perm(np.asarray(p["att1"]))
    aab0 = np.abs(np.asarray(p["att0"]).reshape(-1)[perm0])
    aab1 = np.abs(np.asarray(p["att1"]).reshape(-1)[perm1])

    def scale_cols(Wm, b, perm, aab):
        Wp = np.asarray(Wm)[:, perm] * aab[None, :]
        bp = np.asarray(b)[perm] * aab
        return Wp, bp

    Wl0p, bl0p = scale_cols(p["Wl0"], p["bl0"], perm0, aab0)
    Wr0p, br0p = scale_cols(p["Wr0"], p["br0"], perm0, aab0)
    Wl1p, bl1p = scale_cols(np.asarray(p["Wl1"])[perm0, :], p["bl1"],
                            perm1, aab1)
    Wr1p, br1p = scale_cols(np.asarray(p["Wr1"])[perm0, :], p["br1"],
                            perm1, aab1)

    FEAT0 = H * C            # 192
    TBL0 = 256               # bf16 cols -> 512B row stride
    TBL1 = 128               # bf16 cols -> 256B row stride

    shared = {
        "Wl0p": Wl0p.astype(bf16), "Wr0p": Wr0p.astype(bf16),
        "BSUM0": _rep(bl0p + br0p, bf16),
        "Wl1a": Wl1p[:128].astype(bf16), "Wl1b": Wl1p[128:].astype(bf16),
        "Wr1a": Wr1p[:128].astype(bf16), "Wr1b": Wr1p[128:].astype(bf16),
        "BSUM1": _rep(bl1p + br1p),
        "RA0": _rep(1.0 / aab0), "RA1": _rep(1.0 / aab1),
        "BIAS0": _rep(np.asarray(p["bias0"])[perm0]
                      + np.asarray(p["bl0"])[perm0]),
        "BIAS1": _rep(np.asarray(p["bias1"])[perm1]
                      + np.asarray(p["bl1"])[perm1]),
        "gamma0": np.asarray(p["gamma0"])[perm0].reshape(1, -1).astype(
            np.float32),
        "beta0": np.asarray(p["beta0"])[perm0].reshape(1, -1).astype(
            np.float32),
        "gamma1": np.asarray(p["gamma1"])[perm1].reshape(1, -1).astype(
            np.float32),
        "beta1": np.asarray(p["beta1"])[perm1].reshape(1, -1).astype(
            np.float32),
        "IDENT": np.eye(128, dtype=bf16),
        "ONE1": np.ones((1, 128), np.float32),
    }
    mlast = np.zeros(128, np.float32)
    mlast[:SLICE - (W - 1) * 128] = 1.0
    shared["MASKS"] = np.stack(
        [np.ones(128, np.float32), mlast], 1).astype(bf16)

    # full padded transposed x (same on every core) + local slice
    xt = np.asarray(x).T  # [IN, N]
    xtf = np.zeros((IN, NP), np.float32)
    for k in range(n_cores):
        xtf[:, k * SP:k * SP + SLICE] = xt[:, k * SLICE:(k + 1) * SLICE]
    xtf = xtf.astype(bf16)

    in_maps = []
    for k in range(n_cores):
        m = {"xTf": xtf,
             "xT": np.ascontiguousarray(xtf[:, k * SP:(k + 1) * SP]),
             "srcw0": srcw0[k], "srcw1": srcw1[k],
             "swin": swin[k], "salh": salh[k]}
        for kk, v in shared.items():
            m[kk] = np.ascontiguousarray(v)
        in_maps.append(m)

    meta = dict(N=N, IN=IN, SLICE=SLICE, SP=SP, NP=NP, W=W, BLK=BLK,
                G0=[int(v) for v in G0], G1=[int(v) for v in G1],
                GW=[int(v) for v in GW], NG=NG,
                H=H, C=C, FEAT0=FEAT0, OUTC=OUTC, TBL0=TBL0, TBL1=TBL1,
                npos0=[int(v) for v in npos0], npos1=[int(v) for v in npos1],
                scs0=scs0, scs1=scs1,
                n_cores=n_cores)
    return in_maps, meta, perm1


# ---------------------------------------------------------------- program

def build_program(meta):
    n_cores = meta["n_cores"]
    IN, SP, NP, W = meta["IN"], meta["SP"], meta["NP"], meta["W"]
    G0, G1, GW, NG = meta["G0"], meta["G1"], meta["GW"], meta["NG"]
    H, C, FEAT0 = meta["H"], meta["C"], meta["FEAT0"]
    OUTC, TBL0, TBL1 = meta["OUTC"], meta["TBL0"], meta["TBL1"]
    SLICE = meta["SLICE"]
    BLKROW = (n_cores // 2) * SP
    WF = NP // 128                 # full-table windows (392)
    cores = list(range(n_cores))
    bat0 = _mkbatches(W, B0)
    bat1 = _mkbatches(W, B1)
    gcs = [0] * (W + 1)
    for w in range(W):
        gcs[w + 1] = gcs[w] + GW[w]

    nc = Bacc("TRN2", target_bir_lowering=False, debug=False,
              num_devices=n_cores, num_swdge_queues=4)

    def dp(name, shape, dt):
        return nc.declare_dram_parameter(name, list(shape), dt,
                                         isOutput=False)

    xTf = dp("xTf", [IN, NP], BF)
    xT = dp("xT", [IN, SP], BF)
    srcw0 = dp("srcw0", [128, NG * 8], I16)
    srcw1 = dp("srcw1", [128, NG * 8], I16)
    swin = dp("swin", [128, NG * 128], BF)
    salh = dp("salh", [128, NG * 128], BF)
    Wl0p = dp("Wl0p", [IN, FEAT0], BF)
    Wr0p = dp("Wr0p", [IN, FEAT0], BF)
    BSUM0 = dp("BSUM0", [128, FEAT0], BF)
    Wl1a = dp("Wl1a", [128, OUTC], BF)
    Wl1b = dp("Wl1b", [FEAT0 - 128, OUTC], BF)
    Wr1a = dp("Wr1a", [128, OUTC], BF)
    Wr1b = dp("Wr1b", [FEAT0 - 128, OUTC], BF)
    BSUM1 = dp("BSUM1", [128, OUTC], F32)
    RA0 = dp("RA0", [128, FEAT0], F32)
    RA1 = dp("RA1", [128, OUTC], F32)
    BIAS0 = dp("BIAS0", [128, FEAT0], F32)
    BIAS1 = dp("BIAS1", [128, OUTC], F32)
    gamma0 = dp("gamma0", [1, FEAT0], F32)
    beta0 = dp("beta0", [1, FEAT0], F32)
    gamma1 = dp("gamma1", [1, OUTC], F32)
    beta1 = dp("beta1", [1, OUTC], F32)
    IDENT = dp("IDENT", [128, 128], BF)
    ONE1 = dp("ONE1", [1, 128], F32)
    MASKS = dp("MASKS", [128, 2], BF)

    out = nc.declare_dram_parameter("out", [SLICE, OUTC], F32, isOutput=True)

    # internal DRAM
    XL0 = nc.dram_tensor("XL0", [NP, TBL0], BF)
    XL0loc = nc.dram_tensor("XL0loc", [SP, FEAT0], BF)
    XL1s = nc.dram_tensor("XL1s", [SP, TBL1], BF)
    XL1 = nc.dram_tensor("XL1", [NP, TBL1], BF, addr_space="Shared")
    ST0i = nc.dram_tensor("ST0i", [1, 2 * FEAT0], F32)
    ST0o = nc.dram_tensor("ST0o", [1, 2 * FEAT0], F32, addr_space="Shared")
    ST1i = nc.dram_tensor("ST1i", [1, 2 * OUTC], F32)
    ST1o = nc.dram_tensor("ST1o", [1, 2 * OUTC], F32, addr_space="Shared")

    with tile.TileContext(nc) as tc:
        with tc.tile_pool(name="const", bufs=1) as cp:
            def ld(par, shape, dt):
                t = cp.tile(list(shape), dt, tag=f"c_{par.name}")
                nc.sync.dma_start(out=t[:], in_=par[:])
                return t

            c_ident = ld(IDENT, [128, 128], BF)
            c_one1 = ld(ONE1, [1, 128], F32)
            c_masks = ld(MASKS, [128, 2], BF)
            c_wl0 = ld(Wl0p, [IN, FEAT0], BF)
            c_wr0 = ld(Wr0p, [IN, FEAT0], BF)
            c_bs0 = ld(BSUM0, [128, FEAT0], BF)
            c_wl1a = ld(Wl1a, [128, OUTC], BF)
            c_wl1b = ld(Wl1b, [FEAT0 - 128, OUTC], BF)
            c_wr1a = ld(Wr1a, [128, OUTC], BF)
            c_wr1b = ld(Wr1b, [FEAT0 - 128, OUTC], BF)
            c_bs1 = ld(BSUM1, [128, OUTC], F32)
            c_ra0 = ld(RA0, [128, FEAT0], F32)
            c_ra1 = ld(RA1, [128, OUTC], F32)
            c_bias0 = ld(BIAS0, [128, FEAT0], F32)
            c_bias1 = ld(BIAS1, [128, OUTC], F32)
            c_g0 = ld(gamma0, [1, FEAT0], F32)
            c_b0 = ld(beta0, [1, FEAT0], F32)
            c_g1 = ld(gamma1, [1, OUTC], F32)
            c_b1 = ld(beta1, [1, OUTC], F32)

            # ---------------- BN coeff helper --------------------------
            def bn_coeffs(st_out, feat, g_row, b_row, nodes, reps=True):
                st = cp.tile([1, 2 * feat], F32, tag=f"st{feat}")
                nc.sync.dma_start(out=st[:], in_=st_out[:, :])
                mu = cp.tile([1, feat], F32, tag=f"mu{feat}")
                var = cp.tile([1, feat], F32, tag=f"va{feat}")
                nc.vector.tensor_scalar_mul(mu[:], st[:, 0:feat],
                                            1.0 / nodes)
                nc.vector.tensor_scalar_mul(var[:], st[:, feat:2 * feat],
                                            1.0 / nodes)
                t = cp.tile([1, feat], F32, tag=f"t{feat}")
                nc.vector.tensor_mul(t[:], mu[:], mu[:])
                nc.vector.tensor_sub(var[:], var[:], t[:])
                nc.vector.tensor_scalar_add(var[:], var[:], BN_EPS)
                nc.scalar.activation(t[:], var[:], AF.Sqrt)
                nc.vector.reciprocal(t[:], t[:])
                arow = cp.tile([1, feat], F32, tag=f"ar{feat}")
                nc.vector.tensor_mul(arow[:], g_row[:], t[:])
                brow = cp.tile([1, feat], F32, tag=f"br{feat}")
                nc.vector.tensor_mul(t[:], mu[:], arow[:])
                nc.vector.tensor_sub(brow[:], b_row[:], t[:])
                if not reps:
                    return arow, brow
                with tc.tile_pool(name="bnp", bufs=2, space="PSUM") as bp:
                    pa = bp.tile([128, feat], F32, tag="pa")
                    nc.tensor.matmul(pa[:], c_one1[:], arow[:],
                                     start=True, stop=True)
                    Ar = cp.tile([128, feat], BF, tag=f"A{feat}")
                    nc.scalar.copy(Ar[:], pa[:])
                    pb2 = bp.tile([128, feat], F32, tag="pb")
                    nc.tensor.matmul(pb2[:], c_one1[:], brow[:],
                                     start=True, stop=True)
                    Br = cp.tile([128, feat], BF, tag=f"B{feat}")
                    nc.scalar.copy(Br[:], pb2[:])
                return Ar, Br

            # ---------------- edge pass helper -------------------------
            def edge_pass(XLt, tblc, feat, nh, npos, ra, biast,
                          hp_tile, xr_tile, st_in, srcw_par, batches, scs,
                          xloc):
                ssum = cp.tile([1, 2 * feat], F32, tag=f"ssum{feat}")
                nc.vector.memset(ssum[:], 0.0)
                ssum3 = ssum[:].rearrange("p (a f) -> p a f", a=2)
                nb = len(batches)
                aoff = {}
                boff = {}
                bcols = []
                for bi, ws in enumerate(batches):
                    ga = sum(G0[w] for w in ws)
                    o = 0
                    for w in ws:
                        aoff[w] = o
                        o += G0[w]
                    o = ga
                    for w in ws:
                        boff[w] = o
                        o += G1[w]
                    bcols.append(o)
                gmax = max(bcols)

                with tc.tile_pool(name="eg", bufs=3) as eg, \
                     tc.tile_pool(name="esrc", bufs=1) as esrc, \
                     tc.tile_pool(name="esw", bufs=2) as esw, \
                     tc.tile_pool(name="est", bufs=3) as est, \
                     tc.tile_pool(name="exl", bufs=4) as exl, \
                     tc.tile_pool(name="es", bufs=2) as es, \
                     tc.tile_pool(name="zps", bufs=4, space="PSUM") as zps, \
                     tc.tile_pool(name="eps", bufs=2, space="PSUM") as eps, \
                     tc.tile_pool(name="sps", bufs=1, space="PSUM") as sps:
                    c_src = esrc.tile([128, NG * 8], I16, tag="src")
                    nc.sync.dma_start(out=c_src[:], in_=srcw_par[:])
                    qrr = [0]

                    def gathers(dst_tile, col0, tbl_ap, idx_col0, n_groups):
                        done = 0
                        while done < n_groups:
                            cnt = min(GCAP, n_groups - done)
                            nc.gpsimd.dma_gather(
                                dst_tile[:, col0 + done:col0 + done + cnt,
                                         :],
                                tbl_ap,
                                c_src[:, idx_col0 + done * 8:
                                      idx_col0 + (done + cnt) * 8],
                                num_idxs=cnt * 128, num_idxs_reg=cnt * 128,
                                elem_size=tblc, elem_step=tblc,
                                queue_num=qrr[0])
                            qrr[0] = (qrr[0] + 1) % 4
                            done += cnt

                    def emit_gather(bi):
                        ws = batches[bi]
                        gl = eg.tile([128, gmax, tblc], BF, tag="gl")
                        ga = sum(G0[w] for w in ws)
                        gb = sum(G1[w] for w in ws)
                        sc = scs[bi]
                        gathers(gl, 0, XLt[:, :], sc, ga)
                        gathers(gl, ga, XLt[BLKROW:, :], sc + ga * 8, gb)
                        return gl

                    def emit_streams(w):
                        gw, gc = GW[w], gcs[w]
                        sw = esw.tile([128, gw, 128], BF, tag="sw")
                        nc.sync.dma_start(
                            out=sw[:],
                            in_=swin[:, gc * 128:(gc + gw) * 128])
                        sal = est.tile([128, gw, 128], BF, tag="sal")
                        nc.sync.dma_start(
                            out=sal[:],
                            in_=salh[:, gc * 128:(gc + gw) * 128])
                        xls = exl.tile([128, feat], BF, tag="xls")
                        nc.sync.dma_start(out=xls[:], in_=xloc(w))
                        return sw, sal, xls

                    def emit_chain(w, gl, sw, xls):
                        gw = GW[w]
                        xw = xr_tile[:, w, :]
                        z = es.tile([128, gw, feat], BF, tag="z")
                        ck = 2 if feat > 96 else 4
                        segs = []
                        for j0 in range(0, G0[w], ck):
                            segs.append((j0, aoff[w] + j0,
                                         min(ck, G0[w] - j0)))
                        for j0 in range(0, G1[w], ck):
                            segs.append((G0[w] + j0, boff[w] + j0,
                                         min(ck, G1[w] - j0)))
                        for (zo, go, cnt) in segs:
                            ps = zps.tile([128, ck, feat], F32, tag="zp")
                            for j in range(cnt):
                                nc.tensor.matmul(
                                    ps[:, j, 0:feat], sw[:, zo + j, :],
                                    xw, start=True, stop=False)
                                nc.tensor.matmul(
                                    ps[:, j, 0:feat], c_ident[:],
                                    gl[:, go + j, 0:feat],
                                    start=False, stop=True)
                            nc.scalar.activation(
                                z[:, zo:zo + cnt, :], ps[:, 0:cnt, :],
                                AF.Prelu, alpha=NEG_SLOPE)
                        lp = es.tile([128, gw, nh], BF, tag="lp")
                        ln = es.tile([128, gw, nh], BF, tag="ln")
                        with nc.allow_low_precision(
                                reason="bf16 logit partial sums"):
                            for h in range(nh):
                                k = npos[h]
                                if k == 0:
                                    nc.vector.memset(lp[:, :, h], 0.0)
                                else:
                                    nc.vector.tensor_reduce(
                                        lp[:, :, h],
                                        z[:, :, h * C:h * C + k],
                                        axis=mybir.AxisListType.X,
                                        op=OP.add)
                                if k == C:
                                    nc.vector.memset(ln[:, :, h], 0.0)
                                else:
                                    nc.vector.tensor_reduce(
                                        ln[:, :, h],
                                        z[:, :, h * C + k:(h + 1) * C],
                                        axis=mybir.AxisListType.X,
                                        op=OP.add)
                        rhs = es.tile([128, gw, feat + 4], BF, tag="rhs")
                        lg = es.tile([128, gw, nh], BF, tag="lg")
                        nc.vector.tensor_sub(lg[:], lp[:], ln[:])
                        nc.scalar.activation(
                            rhs[:, :, feat:feat + nh], lg[:], AF.Exp)
                        pe = es.tile([128, gw, nh * C], BF, tag="pe")
                        nc.scalar.activation(
                            pe[:].rearrange("p g (h c) -> p g h c", c=C),
                            lg[:].unsqueeze(3).broadcast_to(
                                [128, gw, nh, C]),
                            AF.Exp)
                        nc.vector.tensor_mul(
                            rhs[:, 0:G0[w], 0:feat],
                            gl[:, aoff[w]:aoff[w] + G0[w], 0:feat],
                            pe[:, 0:G0[w], :])
                        nc.vector.tensor_mul(
                            rhs[:, G0[w]:gw, 0:feat],
                            gl[:, boff[w]:boff[w] + G1[w], 0:feat],
                            pe[:, G0[w]:gw, :])
                        # dense self-loop path (PyG add_self_loops)
                        zs = es.tile([128, feat], BF, tag="zs")
                        nc.vector.tensor_add(zs[:], xls[:], xw)
                        nc.scalar.activation(zs[:], zs[:], AF.Prelu,
                                             alpha=NEG_SLOPE)
                        lgs = es.tile([128, 2 * nh], BF, tag="lgs")
                        with nc.allow_low_precision(
                                reason="bf16 logit partial sums"):
                            for h in range(nh):
                                k = npos[h]
                                if k == 0:
                                    nc.vector.memset(lgs[:, h:h + 1], 0.0)
                                else:
                                    nc.vector.tensor_reduce(
                                        lgs[:, h:h + 1],
                                        zs[:, h * C:h * C + k],
                                        axis=mybir.AxisListType.X,
                                        op=OP.add)
                                if k == C:
                                    nc.vector.memset(
                                        lgs[:, nh + h:nh + h + 1], 0.0)
                                else:
                                    nc.vector.tensor_reduce(
                                        lgs[:, nh + h:nh + h + 1],
                                        zs[:, h * C + k:(h + 1) * C],
                                        axis=mybir.AxisListType.X,
                                        op=OP.add)
                        lgd = es.tile([128, nh], BF, tag="lgd")
                        nc.vector.tensor_sub(lgd[:], lgs[:, 0:nh],
                                             lgs[:, nh:2 * nh])
                        pself = exl.tile([128, nh], F32, tag="pself")
                        nc.scalar.activation(pself[:], lgd[:], AF.Exp)
                        return rhs, pself

                    def emit_scatter(w, sal, rhs):
                        gw = GW[w]
                        ps = eps.tile([128, feat + nh], F32, tag="acc")
                        for g in range(gw):
                            nc.tensor.matmul(
                                ps[:], sal[:, g, :], rhs[:, g, 0:feat + nh],
                                start=(g == 0), stop=(g == gw - 1))
                        return ps

                    def emit_finalize(w, ps, xls, pself):
                        dn = es.tile([128, nh], F32, tag="dn")
                        nc.vector.tensor_add(dn[:], ps[:, feat:feat + nh],
                                             pself[:])
                        nc.vector.tensor_scalar_max(dn[:], dn[:], 1e-30)
                        rc = es.tile([128, nh], F32, tag="rc")
                        nc.vector.reciprocal(rc[:], dn[:])
                        tmp = es.tile([128, feat], F32, tag="tmp")
                        for h in range(nh):
                            nc.vector.scalar_tensor_tensor(
                                tmp[:, h * C:(h + 1) * C],
                                xls[:, h * C:(h + 1) * C],
                                pself[:, h:h + 1],
                                ps[:, h * C:(h + 1) * C],
                                op0=OP.mult, op1=OP.add)
                            nc.vector.scalar_tensor_tensor(
                                tmp[:, h * C:(h + 1) * C],
                                tmp[:, h * C:(h + 1) * C],
                                rc[:, h:h + 1],
                                ra[:, h * C:(h + 1) * C],
                                op0=OP.mult, op1=OP.mult)
                        nc.vector.tensor_add(tmp[:], tmp[:], biast[:])
                        nc.vector.tensor_scalar_max(hp_tile[:, w, :],
                                                    tmp[:], 0.0)
                        sq = es.tile([128, feat], BF, tag="sq")
                        nc.vector.tensor_mul(sq[:], hp_tile[:, w, :],
                                             hp_tile[:, w, :])
                        mc = 0 if w < W - 1 else 1
                        sp = sps.tile([1, 2, feat], F32, tag="sp")
                        nc.tensor.matmul(sp[:, 0, :], c_masks[:, mc:mc + 1],
                                         hp_tile[:, w, :],
                                         start=True, stop=True)
                        nc.tensor.matmul(sp[:, 1, :], c_masks[:, mc:mc + 1],
                                         sq[:], start=True, stop=True)
                        nc.vector.tensor_add(ssum3, ssum3, sp[:])

                    WB = batches[1][0] - batches[0][0] if nb > 1 else W
                    pend_g = {}
                    pend_s = {}
                    pend_x = {}
                    pend_r = {}
                    pend_e = {}
                    pend_p = {}
                    emitted = 0

                    def need_batches(upto_w):
                        nonlocal emitted
                        while (emitted < nb
                               and batches[emitted][0] <= upto_w):
                            pend_g[emitted] = emit_gather(emitted)
                            emitted += 1

                    for i in range(W + 3):
                        if i < W:
                            need_batches(i + WB)
                            sw, sal, xls = emit_streams(i)
                            pend_s[i] = (sw, sal)
                            pend_x[i] = xls
                        if 1 <= i <= W:
                            w = i - 1
                            gl = pend_g[w // WB]
                            sw, sal = pend_s[w]
                            pend_r[w], pend_e[w] = emit_chain(
                                w, gl, sw, pend_x[w])
                        if 2 <= i <= W + 1:
                            w = i - 2
                            _, sal = pend_s.pop(w)
                            pend_p[w] = emit_scatter(w, sal,
                                                     pend_r.pop(w))
                            if (w + 1) % WB == 0 or w == W - 1:
                                pend_g.pop(w // WB, None)
                        if 3 <= i:
                            w = i - 3
                            emit_finalize(w, pend_p.pop(w),
                                          pend_x.pop(w), pend_e.pop(w))
                nc.sync.dma_start(out=st_in[:, :], in_=ssum[:])

            # ======== persistent activation tiles (nested scopes) ======
            with tc.tile_pool(name="actA", bufs=1) as pA:
                XR0s = pA.tile([128, W, FEAT0], BF, tag="XR0s")
                HP0s = pA.tile([128, W, FEAT0], BF, tag="HP0s")

                # ---------------- stage A -----------------------------
                # Full XL0 table computed locally (no AllGather).
                with nc.named_scope("stageA"):
                    with tc.tile_pool(name="sta", bufs=4) as sa, \
                         tc.tile_pool(name="sta_ps", bufs=4,
                                      space="PSUM") as sap:
                        XB = 4
                        for t0 in range(0, WF, XB):
                            xt_t = sa.tile([IN, XB * 128], BF, tag="xt")
                            nc.sync.dma_start(
                                out=xt_t[:],
                                in_=xTf[:, t0 * 128:(t0 + XB) * 128])
                            sb4 = sa.tile([128, XB, FEAT0], BF, tag="sb")
                            for j in range(XB):
                                ps = sap.tile([128, FEAT0], F32, tag="ps")
                                nc.tensor.matmul(
                                    ps[:],
                                    xt_t[:, j * 128:(j + 1) * 128],
                                    c_wl0[:], start=True, stop=True)
                                if j % 2 == 0:
                                    nc.scalar.copy(sb4[:, j, :], ps[:])
                                else:
                                    nc.vector.tensor_copy(sb4[:, j, :],
                                                          ps[:])
                            dview = XL0[t0 * 128:(t0 + XB) * 128,
                                        0:FEAT0].rearrange(
                                "(i p) c -> p i c", p=128)
                            nc.sync.dma_start(out=dview, in_=sb4[:])
                        # local xr0 slice (SBUF resident) + local xl0
                        # rows (DRAM, for the dense self-loop path)
                        xt_l = sa.tile([IN, SP], BF, tag="xtl")
                        nc.sync.dma_start(out=xt_l[:], in_=xT[:, :])
                        for t in range(W):
                            ps = sap.tile([128, FEAT0], F32, tag="ps")
                            nc.tensor.matmul(
                                ps[:], xt_l[:, t * 128:(t + 1) * 128],
                                c_wr0[:], start=True, stop=True)
                            nc.vector.tensor_add(XR0s[:, t, :], ps[:],
                                                 c_bs0[:])
                            psl = sap.tile([128, FEAT0], F32, tag="ps")
                            nc.tensor.matmul(
                                psl[:], xt_l[:, t * 128:(t + 1) * 128],
                                c_wl0[:], start=True, stop=True)
                            sbl = sa.tile([128, FEAT0], BF, tag="sbl")
                            nc.scalar.copy(sbl[:], psl[:])
                            nc.sync.dma_start(
                                out=XL0loc[t * 128:(t + 1) * 128, :],
                                in_=sbl[:])

                # ================= layer 0 =============================
                with nc.named_scope("l0edge"):
                    edge_pass(XL0, TBL0, FEAT0, H, meta["npos0"], c_ra0,
                              c_bias0, HP0s, XR0s, ST0i, srcw0, bat0,
                              meta["scs0"],
                              lambda w: XL0loc[w * 128:(w + 1) * 128, :])

                with tc.tile_pool(name="actB", bufs=1) as pB:
                    XR1s = pB.tile([128, W, OUTC], BF, tag="XR1s")
                    HP1s = pB.tile([128, W, OUTC], BF, tag="HP1s")

                    with tc.tile_pool(name="pht", bufs=1) as pht:
                        HT0 = pht.tile([128, W, 128], BF, tag="HT0")
                        HT1 = pht.tile([64, W, 128], BF, tag="HT1")
                        with nc.named_scope("bn0"):
                            nc.gpsimd.collective_compute(
                                "AllReduce", OP.add, replica_groups=[cores],
                                ins=[ST0i[:, :]], outs=[ST0o[:, :]])
                            # transpose hoist: h^T tiles built while the
                            # AllReduce is in flight (no dependency)
                            with tc.tile_pool(name="p2t", bufs=2,
                                              space="PSUM") as ptp:
                                for w in range(W):
                                    pt0 = ptp.tile([128, 128], BF,
                                                   tag="pt0")
                                    nc.tensor.transpose(
                                        pt0[:], HP0s[:, w, 0:128],
                                        c_ident[:])
                                    nc.scalar.copy(HT0[:, w, :], pt0[:])
                                    pt1 = ptp.tile([64, 128], BF,
                                                   tag="pt1")
                                    nc.tensor.transpose(
                                        pt1[:], HP0s[:, w, 128:192],
                                        c_ident[:])
                                    nc.scalar.copy(HT1[:, w, :], pt1[:])
                            arow0, brow0 = bn_coeffs(ST0o, FEAT0, c_g0,
                                                     c_b0, meta["N"],
                                                     reps=False)

                        with nc.named_scope("pass2"):
                            with tc.tile_pool(name="p2", bufs=3) as p2, \
                                 tc.tile_pool(name="p2c", bufs=1) as p2c, \
                                 tc.tile_pool(name="p2cp", bufs=1,
                                              space="PSUM") as ppc, \
                                 tc.tile_pool(name="p2ps", bufs=2,
                                              space="PSUM") as pp:
                                # BN folded into transposed-side weights:
                                # xl1 = h^T-matmul with A-scaled Wl1 rows
                                # + (brow@Wl1) row; same for Wr1 (+BSUM1).
                                ab = p2c.tile([1, FEAT0], BF, tag="ab")
                                nc.vector.tensor_copy(ab[:], arow0[:])
                                bb = p2c.tile([1, FEAT0], BF, tag="bb")
                                nc.vector.tensor_copy(bb[:], brow0[:])
                                pac = ppc.tile([128, 4], BF, tag="pac")
                                nc.tensor.transpose(pac[:, 0:1],
                                                    ab[:, 0:128],
                                                    c_ident[0:1, 0:1])
                                nc.tensor.transpose(pac[:, 2:3],
                                                    bb[:, 0:128],
                                                    c_ident[0:1, 0:1])
                                pac2 = ppc.tile([64, 4], BF, tag="pac2")
                                nc.tensor.transpose(pac2[:, 0:1],
                                                    ab[:, 128:192],
                                                    c_ident[0:1, 0:1])
                                nc.tensor.transpose(pac2[:, 2:3],
                                                    bb[:, 128:192],
                                                    c_ident[0:1, 0:1])
                                ac = p2c.tile([128, 2], BF, tag="ac")
                                nc.scalar.copy(ac[:, 0:1], pac[:, 0:1])
                                nc.scalar.copy(ac[:, 1:2], pac[:, 2:3])
                                ac2 = p2c.tile([64, 2], BF, tag="ac2")
                                nc.scalar.copy(ac2[:, 0:1], pac2[:, 0:1])
                                nc.scalar.copy(ac2[:, 1:2], pac2[:, 2:3])
                                wl1as = p2c.tile([128, OUTC], BF,
                                                 tag="wl1as")
                                nc.vector.tensor_mul(
                                    wl1as[:], c_wl1a[:],
                                    ac[:, 0:1].broadcast_to([128, OUTC]))
                                wl1bs = p2c.tile([64, OUTC], BF,
                                                 tag="wl1bs")
                                nc.vector.tensor_mul(
                                    wl1bs[:], c_wl1b[:],
                                    ac2[:, 0:1].broadcast_to([64, OUTC]))
                                wr1as = p2c.tile([128, OUTC], BF,
                                                 tag="wr1as")
                                nc.vector.tensor_mul(
                                    wr1as[:], c_wr1a[:],
                                    ac[:, 0:1].broadcast_to([128, OUTC]))
                                wr1bs = p2c.tile([64, OUTC], BF,
                                                 tag="wr1bs")
                                nc.vector.tensor_mul(
                                    wr1bs[:], c_wr1b[:],
                                    ac2[:, 0:1].broadcast_to([64, OUTC]))
                                # bias rows: brow @ Wl1 / Wr1
                                pbc = ppc.tile([1, 2 * OUTC], F32,
                                               tag="pbc")
                                nc.tensor.matmul(pbc[:, 0:OUTC],
                                                 ac[:, 1:2], c_wl1a[:],
                                                 start=True, stop=False)
                                nc.tensor.matmul(pbc[:, 0:OUTC],
                                                 ac2[:, 1:2], c_wl1b[:],
                                                 start=False, stop=True)
                                nc.tensor.matmul(pbc[:, OUTC:2 * OUTC],
                                                 ac[:, 1:2], c_wr1a[:],
                                                 start=True, stop=False)
                                nc.tensor.matmul(pbc[:, OUTC:2 * OUTC],
                                                 ac2[:, 1:2], c_wr1b[:],
                                                 start=False, stop=True)
                                bcrow = p2c.tile([1, 2 * OUTC], F32,
                                                 tag="bcrow")
                                nc.vector.tensor_copy(bcrow[:], pbc[:])
                                pbr = ppc.tile([128, 2 * OUTC], F32,
                                               tag="pbr")
                                nc.tensor.matmul(pbr[:], c_one1[:],
                                                 bcrow[:],
                                                 start=True, stop=True)
                                BCL = p2c.tile([128, OUTC], BF, tag="BCL")
                                nc.scalar.copy(BCL[:], pbr[:, 0:OUTC])
                                BCR = p2c.tile([128, OUTC], F32,
                                               tag="BCR")
                                nc.vector.tensor_add(
                                    BCR[:], pbr[:, OUTC:2 * OUTC],
                                    c_bs1[:])
                                for w in range(W):
                                    px = pp.tile([128, OUTC], F32,
                                                 tag="px")
                                    nc.tensor.matmul(px[:], HT0[:, w, :],
                                                     wl1as[:],
                                                     start=True,
                                                     stop=False)
                                    nc.tensor.matmul(px[:], HT1[:, w, :],
                                                     wl1bs[:],
                                                     start=False,
                                                     stop=True)
                                    xs = p2.tile([128, OUTC], BF,
                                                 tag="xs")
                                    nc.vector.tensor_add(xs[:], px[:],
                                                         BCL[:])
                                    nc.sync.dma_start(
                                        out=XL1s[w * 128:(w + 1) * 128,
                                                 0:OUTC],
                                        in_=xs[:])
                                    px2 = pp.tile([128, OUTC], F32,
                                                  tag="px")
                                    nc.tensor.matmul(px2[:],
                                                     HT0[:, w, :],
                                                     wr1as[:],
                                                     start=True,
                                                     stop=False)
                                    nc.tensor.matmul(px2[:],
                                                     HT1[:, w, :],
                                                     wr1bs[:],
                                                     start=False,
                                                     stop=True)
                                    nc.vector.tensor_add(XR1s[:, w, :],
                                                         px2[:], BCR[:])

                    with nc.named_scope("ag1"):
                        nc.gpsimd.collective_compute(
                            "AllGather", OP.bypass, replica_groups=[cores],
                            ins=[XL1s[:, :]], outs=[XL1[:, :]])

                    # ================= layer 1 =========================
                    with nc.named_scope("l1edge"):
                        edge_pass(XL1, TBL1, OUTC, 1, meta["npos1"],
                                  c_ra1, c_bias1, HP1s, XR1s, ST1i,
                                  srcw1, bat1, meta["scs1"],
                                  lambda w: XL1s[w * 128:(w + 1) * 128,
                                                 0:OUTC])
                    with nc.named_scope("fin"):
                        nc.gpsimd.collective_compute(
                            "AllReduce", OP.add, replica_groups=[cores],
                            ins=[ST1i[:, :]], outs=[ST1o[:, :]])
                        A1, B1t = bn_coeffs(ST1o, OUTC, c_g1, c_b1,
                                            meta["N"])
                        with tc.tile_pool(name="p3", bufs=3) as p3:
                            for w in range(W):
                                rows = min(128, SLICE - w * 128)
                                ob = p3.tile([128, OUTC], F32, tag="ob")
                                nc.vector.tensor_mul(ob[:], HP1s[:, w, :],
                                                     A1[:])
                                nc.vector.tensor_add(ob[:], ob[:], B1t[:])
                                nc.sync.dma_start(
                                    out=out[w * 128:w * 128 + rows, :],
                                    in_=ob[0:rows, :])

    nc.compile()
    return nc


# ---------------------------------------------------------------- entry

def kernel(**inputs):
    x = np.asarray(inputs["x"])
    edge_index = np.asarray(inputs["edge_index"])
    params = {k: np.asarray(v) for k, v in inputs.items()
              if k not in ("x", "edge_index")}
    n_cores = 8
    in_maps, meta, perm1 = preprocess(x, edge_index, params, n_cores)
    nc = build_program(meta)
    import os
    trace = bool(int(os.environ.get("K_TRACE", "0")))
    res = run_bass_kernel_spmd(nc, in_maps, list(range(n_cores)),
                               trace=trace)
    global LAST_RES
    LAST_RES = res
    if trace:
        print(f"HW exec time: {res.exec_time_ns} ns", flush=True)
    outs = [res.results[k]["out"] for k in range(n_cores)]
    full = np.concatenate(outs, 0)
    inv = np.argsort(perm1)
    return np.ascontiguousarray(full[:, inv]).astype(np.float32)


# revision 32
# speedup vs baseline: 1.1692x; 1.1529x over previous
"""GATv2 (2-layer, GAT_56727928046275) on 8 TRN2 NeuronCores — v3.

Strategy:
  - Nodes sharded by destination across 8 cores (6250 dst nodes each).
  - Edges (incl. self-loops) partitioned by dst, grouped into 128-dst
    "windows"; per-node softmax + scatter-add stay core-local via
    per-window PSUM accumulation with indicator matmuls.
  - Layer-0 xl table computed LOCALLY IN FULL on every core from a
    replicated x (no AllGather on the critical path); layer-1 table
    still AllGathered (activations are distributed).
  - xl tables carry NO bias: both linear biases are folded into the
    xr side (logits) and the finalize bias (scatter output).
  - Per-edge xl rows fetched with gpsimd dma_gather, full-row payloads
    (512B / 256B).  SWDGE desc-gen (~4.5ns/desc, Pool engine) is the
    floor; ops are 1024-desc, 4-queue round-robin, window-batched on
    layer 1.
  - z = xl[src]+xr[dst] built on the PE (indicator matmul + identity
    accumulate into PSUM); leaky-relu on Scalar (PSUM -> packed bf16);
    DVE does reduces + softmax muls; scatter via indicator matmuls.
  - xr/h tables SBUF-resident between phases; BN stats via masked
    ones-matmul column sums + AllReduce.
"""

import numpy as np
import ml_dtypes

import concourse.bass as bass
import concourse.mybir as mybir
import concourse.tile as tile
from concourse.bacc import Bacc
from concourse.bass_utils import run_bass_kernel_spmd

BF = mybir.dt.bfloat16
F32 = mybir.dt.float32
I16 = mybir.dt.int16
OP = mybir.AluOpType
AF = mybir.ActivationFunctionType
bf16 = ml_dtypes.bfloat16

NEG_SLOPE = 0.2
BN_EPS = 1e-5

import os as _os
B0 = 1   # window batch for layer-0 gathers
B1 = 4   # window batch for layer-1 gathers
# groups per gather op; 8 = 1024 idxs = full SWDGE ring. Smaller ops let
# descriptor GENERATION of op N+1 overlap ring DRAIN of op N.
GCAP = int(_os.environ.get("K_GCAP", "8"))


# ---------------------------------------------------------------- host prep

def _wrap16(a):
    a = np.asarray(a, np.int16).reshape(-1, 16).T  # [16, n/16]
    return np.tile(a, (8, 1))


def _rep(v, dt=np.float32):
    v = np.asarray(v, dt).reshape(1, -1)
    return np.ascontiguousarray(np.broadcast_to(v, (128, v.shape[1])))


def _sign_perm(att):
    H, C = att.shape
    perm = np.zeros(H * C, np.int64)
    npos = np.zeros(H, np.int64)
    for h in range(H):
        a = att[h]
        pos = np.nonzero(a >= 0)[0]
        neg = np.nonzero(a < 0)[0]
        perm[h * C:(h + 1) * C] = h * C + np.concatenate([pos, neg])
        npos[h] = len(pos)
    return perm, npos


def _mkbatches(W, B):
    return [list(range(s, min(s + B, W))) for s in range(0, W, B)]


def preprocess(x, edge_index, params, n_cores=8):
    N, IN = x.shape
    SLICE = N // n_cores
    SP = ((SLICE + 127) // 128) * 128          # padded slice rows (6272)
    W = SP // 128                              # windows per core (49)
    NP = n_cores * SP
    BLK = (n_cores // 2) * SP                  # int16 table split row (25088)

    # PyG's added self-loops are handled by a dense per-window path (no
    # gather descriptors); pre-existing (n,n) edges stay in the lists.
    src = np.asarray(edge_index[0], np.int64)
    dst = np.asarray(edge_index[1], np.int64)
    core = dst // SLICE
    row = (src // SLICE) * SP + (src % SLICE)

    ecnt0 = np.zeros((n_cores, W), np.int64)
    ecnt1 = np.zeros((n_cores, W), np.int64)
    lists = [[None] * W for _ in range(n_cores)]
    for k in range(n_cores):
        m = core == k
        r, d = row[m], dst[m] - k * SLICE
        win = d // 128
        blk = (r >= BLK).astype(np.int64)
        order = np.lexsort((blk, win))
        r, d, win, blk = r[order], d[order], win[order], blk[order]
        for w in range(W):
            wm = win == w
            rw, dw, bw = r[wm], d[wm], blk[wm]
            b0 = bw == 0
            lists[k][w] = (rw[b0], dw[b0], rw[~b0], dw[~b0])
            ecnt0[k][w] = int(b0.sum())
            ecnt1[k][w] = int((~b0).sum())

    G0 = np.maximum(1, (ecnt0.max(0) + 127) // 128)   # [W]
    G1 = np.maximum(1, (ecnt1.max(0) + 127) // 128)
    GW = G0 + G1
    NG = int(GW.sum())

    def build_srcw(batches):
        arr = np.zeros((n_cores, 128, NG * 8), np.int16)
        scs = {}
        sc = 0
        for bi, ws in enumerate(batches):
            scs[bi] = sc
            for k in range(n_cores):
                c = sc
                for w in ws:  # A blocks
                    r0, _, _, _ = lists[k][w]
                    s0 = np.zeros(int(G0[w]) * 128, np.int64)
                    s0[:len(r0)] = r0
                    arr[k][:, c:c + int(G0[w]) * 8] = _wrap16(s0)
                    c += int(G0[w]) * 8
                for w in ws:  # B blocks
                    _, _, r1, _ = lists[k][w]
                    s1 = np.zeros(int(G1[w]) * 128, np.int64)
                    s1[:len(r1)] = r1 - BLK
                    arr[k][:, c:c + int(G1[w]) * 8] = _wrap16(s1)
                    c += int(G1[w]) * 8
            sc += sum(int(GW[w]) for w in ws) * 8
        return arr, scs

    bat0 = _mkbatches(W, B0)
    bat1 = _mkbatches(W, B1)
    srcw0, scs0 = build_srcw(bat0)
    srcw1, scs1 = build_srcw(bat1)

    gcs = [0] * (W + 1)
    for w in range(W):
        gcs[w + 1] = gcs[w] + int(GW[w])
    swin = np.zeros((n_cores, 128, NG * 128), bf16)
    salh = np.zeros((n_cores, 128, NG * 128), bf16)
    for k in range(n_cores):
        for w in range(W):
            r0, d0, r1, d1 = lists[k][w]
            n0, n1 = len(r0), len(r1)
            gw = int(GW[w])
            rl = np.full(gw * 128, -1.0, np.float32)
            rl[:n0] = (d0 - w * 128).astype(np.float32)
            rl[int(G0[w]) * 128:int(G0[w]) * 128 + n1] = (
                d1 - w * 128).astype(np.float32)
            gc = gcs[w]
            sw = (np.arange(128)[:, None] == rl[None, :]).astype(bf16)
            swin[k][:, gc * 128:(gc + gw) * 128] = sw
            se = (rl.reshape(gw, 128).T[:, :, None]
                  == np.arange(128)[None, None, :]).astype(bf16)
            salh[k][:, gc * 128:(gc + gw) * 128] = se.reshape(128, -1)

    p = params
    H, C = p["att0"].shape
    OUTC = p["att1"].shape[1]
    perm0, npos0 = _sign_perm(np.asarray(p["att0"]))
    perm1, npos1 = _sign_perm(np.asarray(p["att1"]))
    aab0 = np.abs(np.asarray(p["att0"]).reshape(-1)[perm0])
    aab1 = np.abs(np.asarray(p["att1"]).reshape(-1)[perm1])

    def scale_cols(Wm, b, perm, aab):
        Wp = np.asarray(Wm)[:, perm] * aab[None, :]
        bp = np.asarray(b)[perm] * aab
        return Wp, bp

    Wl0p, bl0p = scale_cols(p["Wl0"], p["bl0"], perm0, aab0)
    Wr0p, br0p = scale_cols(p["Wr0"], p["br0"], perm0, aab0)
    Wl1p, bl1p = scale_cols(np.asarray(p["Wl1"])[perm0, :], p["bl1"],
                            perm1, aab1)
    Wr1p, br1p = scale_cols(np.asarray(p["Wr1"])[perm0, :], p["br1"],
                            perm1, aab1)

    FEAT0 = H * C            # 192
    TBL0 = 256               # bf16 cols -> 512B row stride
    TBL1 = 128               # bf16 cols -> 256B row stride

    shared = {
        "Wl0p": Wl0p.astype(bf16), "Wr0p": Wr0p.astype(bf16),
        "BSUM0": _rep(bl0p + br0p, bf16),
        "Wl1a": Wl1p[:128].astype(bf16), "Wl1b": Wl1p[128:].astype(bf16),
        "Wr1a": Wr1p[:128].astype(bf16), "Wr1b": Wr1p[128:].astype(bf16),
        "BSUM1": _rep(bl1p + br1p),
        "RA0": _rep(1.0 / aab0), "RA1": _rep(1.0 / aab1),
        "BIAS0": _rep(np.asarray(p["bias0"])[perm0]
                      + np.asarray(p["bl0"])[perm0]),
        "BIAS1": _rep(np.asarray(p["bias1"])[perm1]
                      + np.asarray(p["bl1"])[perm1]),
        "gamma0": np.asarray(p["gamma0"])[perm0].reshape(1, -1).astype(
            np.float32),
        "beta0": np.asarray(p["beta0"])[perm0].reshape(1, -1).astype(
            np.float32),
        "gamma1": np.asarray(p["gamma1"])[perm1].reshape(1, -1).astype(
            np.float32),
        "beta1": np.asarray(p["beta1"])[perm1].reshape(1, -1).astype(
            np.float32),
        "IDENT": np.eye(128, dtype=bf16),
        "ONE1": np.ones((1, 128), np.float32),
    }
    mlast = np.zeros(128, np.float32)
    mlast[:SLICE - (W - 1) * 128] = 1.0
    shared["MASKS"] = np.stack(
        [np.ones(128, np.float32), mlast], 1).astype(bf16)

    # full padded transposed x (same on every core) + local slice
    xt = np.asarray(x).T  # [IN, N]
    xtf = np.zeros((IN, NP), np.float32)
    for k in range(n_cores):
        xtf[:, k * SP:k * SP + SLICE] = xt[:, k * SLICE:(k + 1) * SLICE]
    xtf = xtf.astype(bf16)

    in_maps = []
    for k in range(n_cores):
        m = {"xTf": xtf,
             "xT": np.ascontiguousarray(xtf[:, k * SP:(k + 1) * SP]),
             "srcw0": srcw0[k], "srcw1": srcw1[k],
             "swin": swin[k], "salh": salh[k]}
        for kk, v in shared.items():
            m[kk] = np.ascontiguousarray(v)
        in_maps.append(m)

    meta = dict(N=N, IN=IN, SLICE=SLICE, SP=SP, NP=NP, W=W, BLK=BLK,
                G0=[int(v) for v in G0], G1=[int(v) for v in G1],
                GW=[int(v) for v in GW], NG=NG,
                H=H, C=C, FEAT0=FEAT0, OUTC=OUTC, TBL0=TBL0, TBL1=TBL1,
                npos0=[int(v) for v in npos0], npos1=[int(v) for v in npos1],
                scs0=scs0, scs1=scs1,
                n_cores=n_cores)
    return in_maps, meta, perm1


# ---------------------------------------------------------------- program

def build_program(meta):
    n_cores = meta["n_cores"]
    IN, SP, NP, W = meta["IN"], meta["SP"], meta["NP"], meta["W"]
    G0, G1, GW, NG = meta["G0"], meta["G1"], meta["GW"], meta["NG"]
    H, C, FEAT0 = meta["H"], meta["C"], meta["FEAT0"]
    OUTC, TBL0, TBL1 = meta["OUTC"], meta["TBL0"], meta["TBL1"]
    SLICE = meta["SLICE"]
    BLKROW = (n_cores // 2) * SP
    WF = NP // 128                 # full-table windows (392)
    cores = list(range(n_cores))
    bat0 = _mkbatches(W, B0)
    bat1 = _mkbatches(W, B1)
    gcs = [0] * (W + 1)
    for w in range(W):
        gcs[w + 1] = gcs[w] + GW[w]

    nc = Bacc("TRN2", target_bir_lowering=False, debug=False,
              num_devices=n_cores, num_swdge_queues=4)

    def dp(name, shape, dt):
        return nc.declare_dram_parameter(name, list(shape), dt,
                                         isOutput=False)

    xTf = dp("xTf", [IN, NP], BF)
    xT = dp("xT", [IN, SP], BF)
    srcw0 = dp("srcw0", [128, NG * 8], I16)
    srcw1 = dp("srcw1", [128, NG * 8], I16)
    swin = dp("swin", [128, NG * 128], BF)
    salh = dp("salh", [128, NG * 128], BF)
    Wl0p = dp("Wl0p", [IN, FEAT0], BF)
    Wr0p = dp("Wr0p", [IN, FEAT0], BF)
    BSUM0 = dp("BSUM0", [128, FEAT0], BF)
    Wl1a = dp("Wl1a", [128, OUTC], BF)
    Wl1b = dp("Wl1b", [FEAT0 - 128, OUTC], BF)
    Wr1a = dp("Wr1a", [128, OUTC], BF)
    Wr1b = dp("Wr1b", [FEAT0 - 128, OUTC], BF)
    BSUM1 = dp("BSUM1", [128, OUTC], F32)
    RA0 = dp("RA0", [128, FEAT0], F32)
    RA1 = dp("RA1", [128, OUTC], F32)
    BIAS0 = dp("BIAS0", [128, FEAT0], F32)
    BIAS1 = dp("BIAS1", [128, OUTC], F32)
    gamma0 = dp("gamma0", [1, FEAT0], F32)
    beta0 = dp("beta0", [1, FEAT0], F32)
    gamma1 = dp("gamma1", [1, OUTC], F32)
    beta1 = dp("beta1", [1, OUTC], F32)
    IDENT = dp("IDENT", [128, 128], BF)
    ONE1 = dp("ONE1", [1, 128], F32)
    MASKS = dp("MASKS", [128, 2], BF)

    out = nc.declare_dram_parameter("out", [SLICE, OUTC], F32, isOutput=True)

    # internal DRAM
    XL0 = nc.dram_tensor("XL0", [NP, TBL0], BF)
    XL0loc = nc.dram_tensor("XL0loc", [SP, FEAT0], BF)
    XL1s = nc.dram_tensor("XL1s", [SP, TBL1], BF)
    XL1 = nc.dram_tensor("XL1", [NP, TBL1], BF, addr_space="Shared")
    ST0i = nc.dram_tensor("ST0i", [1, 2 * FEAT0], F32)
    ST0o = nc.dram_tensor("ST0o", [1, 2 * FEAT0], F32, addr_space="Shared")
    ST1i = nc.dram_tensor("ST1i", [1, 2 * OUTC], F32)
    ST1o = nc.dram_tensor("ST1o", [1, 2 * OUTC], F32, addr_space="Shared")

    with tile.TileContext(nc) as tc:
        with tc.tile_pool(name="const", bufs=1) as cp:
            def ld(par, shape, dt):
                t = cp.tile(list(shape), dt, tag=f"c_{par.name}")
                nc.sync.dma_start(out=t[:], in_=par[:])
                return t

            c_ident = ld(IDENT, [128, 128], BF)
            c_one1 = ld(ONE1, [1, 128], F32)
            c_masks = ld(MASKS, [128, 2], BF)
            c_wl0 = ld(Wl0p, [IN, FEAT0], BF)
            c_wr0 = ld(Wr0p, [IN, FEAT0], BF)
            c_bs0 = ld(BSUM0, [128, FEAT0], BF)
            c_wl1a = ld(Wl1a, [128, OUTC], BF)
            c_wl1b = ld(Wl1b, [FEAT0 - 128, OUTC], BF)
            c_wr1a = ld(Wr1a, [128, OUTC], BF)
            c_wr1b = ld(Wr1b, [FEAT0 - 128, OUTC], BF)
            c_bs1 = ld(BSUM1, [128, OUTC], F32)
            c_ra0 = ld(RA0, [128, FEAT0], F32)
            c_ra1 = ld(RA1, [128, OUTC], F32)
            c_bias0 = ld(BIAS0, [128, FEAT0], F32)
            c_bias1 = ld(BIAS1, [128, OUTC], F32)
            c_g0 = ld(gamma0, [1, FEAT0], F32)
            c_b0 = ld(beta0, [1, FEAT0], F32)
            c_g1 = ld(gamma1, [1, OUTC], F32)
            c_b1 = ld(beta1, [1, OUTC], F32)

            # ---------------- BN coeff helper --------------------------
            def bn_coeffs(st_out, feat, g_row, b_row, nodes, reps=True):
                st = cp.tile([1, 2 * feat], F32, tag=f"st{feat}")
                nc.sync.dma_start(out=st[:], in_=st_out[:, :])
                mu = cp.tile([1, feat], F32, tag=f"mu{feat}")
                var = cp.tile([1, feat], F32, tag=f"va{feat}")
                nc.vector.tensor_scalar_mul(mu[:], st[:, 0:feat],
                                            1.0 / nodes)
                nc.vector.tensor_scalar_mul(var[:], st[:, feat:2 * feat],
                                            1.0 / nodes)
                t = cp.tile([1, feat], F32, tag=f"t{feat}")
                nc.vector.tensor_mul(t[:], mu[:], mu[:])
                nc.vector.tensor_sub(var[:], var[:], t[:])
                nc.vector.tensor_scalar_add(var[:], var[:], BN_EPS)
                nc.scalar.activation(t[:], var[:], AF.Sqrt)
                nc.vector.reciprocal(t[:], t[:])
                arow = cp.tile([1, feat], F32, tag=f"ar{feat}")
                nc.vector.tensor_mul(arow[:], g_row[:], t[:])
                brow = cp.tile([1, feat], F32, tag=f"br{feat}")
                nc.vector.tensor_mul(t[:], mu[:], arow[:])
                nc.vector.tensor_sub(brow[:], b_row[:], t[:])
                if not reps:
                    return arow, brow
                with tc.tile_pool(name="bnp", bufs=2, space="PSUM") as bp:
                    pa = bp.tile([128, feat], F32, tag="pa")
                    nc.tensor.matmul(pa[:], c_one1[:], arow[:],
                                     start=True, stop=True)
                    Ar = cp.tile([128, feat], BF, tag=f"A{feat}")
                    nc.scalar.copy(Ar[:], pa[:])
                    pb2 = bp.tile([128, feat], F32, tag="pb")
                    nc.tensor.matmul(pb2[:], c_one1[:], brow[:],
                                     start=True, stop=True)
                    Br = cp.tile([128, feat], BF, tag=f"B{feat}")
                    nc.scalar.copy(Br[:], pb2[:])
                return Ar, Br

            # ---------------- edge pass helper -------------------------
            def edge_pass(XLt, tblc, feat, nh, npos, ra, biast,
                          hp_tile, xr_tile, st_in, srcw_par, batches, scs,
                          xloc, egbufs=3, look=1):
                ssum = cp.tile([1, 2 * feat], F32, tag=f"ssum{feat}")
                nc.vector.memset(ssum[:], 0.0)
                ssum3 = ssum[:].rearrange("p (a f) -> p a f", a=2)
                nb = len(batches)
                aoff = {}
                boff = {}
                bcols = []
                for bi, ws in enumerate(batches):
                    ga = sum(G0[w] for w in ws)
                    o = 0
                    for w in ws:
                        aoff[w] = o
                        o += G0[w]
                    o = ga
                    for w in ws:
                        boff[w] = o
                        o += G1[w]
                    bcols.append(o)
                gmax = max(bcols)

                with tc.tile_pool(name="eg", bufs=egbufs) as eg, \
                     tc.tile_pool(name="esrc", bufs=1) as esrc, \
                     tc.tile_pool(name="esw", bufs=2) as esw, \
                     tc.tile_pool(name="est", bufs=3) as est, \
                     tc.tile_pool(name="exl", bufs=4) as exl, \
                     tc.tile_pool(name="es", bufs=2) as es, \
                     tc.tile_pool(name="zps", bufs=4, space="PSUM") as zps, \
                     tc.tile_pool(name="eps", bufs=2, space="PSUM") as eps, \
                     tc.tile_pool(name="sps", bufs=1, space="PSUM") as sps:
                    c_src = esrc.tile([128, NG * 8], I16, tag="src")
                    nc.sync.dma_start(out=c_src[:], in_=srcw_par[:])
                    qrr = [0]

                    def gathers(dst_tile, col0, tbl_ap, idx_col0, n_groups):
                        done = 0
                        while done < n_groups:
                            cnt = min(GCAP, n_groups - done)
                            nc.gpsimd.dma_gather(
                                dst_tile[:, col0 + done:col0 + done + cnt,
                                         :],
                                tbl_ap,
                                c_src[:, idx_col0 + done * 8:
                                      idx_col0 + (done + cnt) * 8],
                                num_idxs=cnt * 128, num_idxs_reg=cnt * 128,
                                elem_size=tblc, elem_step=tblc,
                                queue_num=qrr[0])
                            qrr[0] = (qrr[0] + 1) % 4
                            done += cnt

                    def emit_gather(bi):
                        ws = batches[bi]
                        gl = eg.tile([128, gmax, tblc], BF, tag="gl")
                        ga = sum(G0[w] for w in ws)
                        gb = sum(G1[w] for w in ws)
                        sc = scs[bi]
                        gathers(gl, 0, XLt[:, :], sc, ga)
                        gathers(gl, ga, XLt[BLKROW:, :], sc + ga * 8, gb)
                        return gl

                    def emit_streams(w):
                        gw, gc = GW[w], gcs[w]
                        sw = esw.tile([128, gw, 128], BF, tag="sw")
                        nc.sync.dma_start(
                            out=sw[:],
                            in_=swin[:, gc * 128:(gc + gw) * 128])
                        sal = est.tile([128, gw, 128], BF, tag="sal")
                        nc.sync.dma_start(
                            out=sal[:],
                            in_=salh[:, gc * 128:(gc + gw) * 128])
                        xls = exl.tile([128, feat], BF, tag="xls")
                        nc.sync.dma_start(out=xls[:], in_=xloc(w))
                        return sw, sal, xls

                    def emit_chain(w, gl, sw, xls):
                        gw = GW[w]
                        xw = xr_tile[:, w, :]
                        z = es.tile([128, gw, feat], BF, tag="z")
                        ck = 2 if feat > 96 else 4
                        segs = []
                        for j0 in range(0, G0[w], ck):
                            segs.append((j0, aoff[w] + j0,
                                         min(ck, G0[w] - j0)))
                        for j0 in range(0, G1[w], ck):
                            segs.append((G0[w] + j0, boff[w] + j0,
                                         min(ck, G1[w] - j0)))
                        for (zo, go, cnt) in segs:
                            ps = zps.tile([128, ck, feat], F32, tag="zp")
                            for j in range(cnt):
                                nc.tensor.matmul(
                                    ps[:, j, 0:feat], sw[:, zo + j, :],
                                    xw, start=True, stop=False)
                                nc.tensor.matmul(
                                    ps[:, j, 0:feat], c_ident[:],
                                    gl[:, go + j, 0:feat],
                                    start=False, stop=True)
                            nc.scalar.activation(
                                z[:, zo:zo + cnt, :], ps[:, 0:cnt, :],
                                AF.Prelu, alpha=NEG_SLOPE)
                        lp = es.tile([128, gw, nh], BF, tag="lp")
                        ln = es.tile([128, gw, nh], BF, tag="ln")
                        with nc.allow_low_precision(
                                reason="bf16 logit partial sums"):
                            for h in range(nh):
                                k = npos[h]
                                if k == 0:
                                    nc.vector.memset(lp[:, :, h], 0.0)
                                else:
                                    nc.vector.tensor_reduce(
                                        lp[:, :, h],
                                        z[:, :, h * C:h * C + k],
                                        axis=mybir.AxisListType.X,
                                        op=OP.add)
                                if k == C:
                                    nc.vector.memset(ln[:, :, h], 0.0)
                                else:
                                    nc.vector.tensor_reduce(
                                        ln[:, :, h],
                                        z[:, :, h * C + k:(h + 1) * C],
                                        axis=mybir.AxisListType.X,
                                        op=OP.add)
                        rhs = es.tile([128, gw, feat + 4], BF, tag="rhs")
                        lg = es.tile([128, gw, nh], BF, tag="lg")
                        nc.vector.tensor_sub(lg[:], lp[:], ln[:])
                        nc.scalar.activation(
                            rhs[:, :, feat:feat + nh], lg[:], AF.Exp)
                        pe = es.tile([128, gw, nh * C], BF, tag="pe")
                        nc.scalar.activation(
                            pe[:].rearrange("p g (h c) -> p g h c", c=C),
                            lg[:].unsqueeze(3).broadcast_to(
                                [128, gw, nh, C]),
                            AF.Exp)
                        nc.vector.tensor_mul(
                            rhs[:, 0:G0[w], 0:feat],
                            gl[:, aoff[w]:aoff[w] + G0[w], 0:feat],
                            pe[:, 0:G0[w], :])
                        nc.vector.tensor_mul(
                            rhs[:, G0[w]:gw, 0:feat],
                            gl[:, boff[w]:boff[w] + G1[w], 0:feat],
                            pe[:, G0[w]:gw, :])
                        # dense self-loop path (PyG add_self_loops)
                        zs = es.tile([128, feat], BF, tag="zs")
                        nc.vector.tensor_add(zs[:], xls[:], xw)
                        nc.scalar.activation(zs[:], zs[:], AF.Prelu,
                                             alpha=NEG_SLOPE)
                        lgs = es.tile([128, 2 * nh], BF, tag="lgs")
                        with nc.allow_low_precision(
                                reason="bf16 logit partial sums"):
                            for h in range(nh):
                                k = npos[h]
                                if k == 0:
                                    nc.vector.memset(lgs[:, h:h + 1], 0.0)
                                else:
                                    nc.vector.tensor_reduce(
                                        lgs[:, h:h + 1],
                                        zs[:, h * C:h * C + k],
                                        axis=mybir.AxisListType.X,
                                        op=OP.add)
                                if k == C:
                                    nc.vector.memset(
                                        lgs[:, nh + h:nh + h + 1], 0.0)
                                else:
                                    nc.vector.tensor_reduce(
                                        lgs[:, nh + h:nh + h + 1],
                                        zs[:, h * C + k:(h + 1) * C],
                                        axis=mybir.AxisListType.X,
                                        op=OP.add)
                        lgd = es.tile([128, nh], BF, tag="lgd")
                        nc.vector.tensor_sub(lgd[:], lgs[:, 0:nh],
                                             lgs[:, nh:2 * nh])
                        pself = exl.tile([128, nh], F32, tag="pself")
                        nc.scalar.activation(pself[:], lgd[:], AF.Exp)
                        return rhs, pself

                    def emit_scatter(w, sal, rhs):
                        gw = GW[w]
                        ps = eps.tile([128, feat + nh], F32, tag="acc")
                        for g in range(gw):
                            nc.tensor.matmul(
                                ps[:], sal[:, g, :], rhs[:, g, 0:feat + nh],
                                start=(g == 0), stop=(g == gw - 1))
                        return ps

                    def emit_finalize(w, ps, xls, pself):
                        dn = es.tile([128, nh], F32, tag="dn")
                        nc.vector.tensor_add(dn[:], ps[:, feat:feat + nh],
                                             pself[:])
                        nc.vector.tensor_scalar_max(dn[:], dn[:], 1e-30)
                        rc = es.tile([128, nh], F32, tag="rc")
                        nc.vector.reciprocal(rc[:], dn[:])
                        tmp = es.tile([128, feat], F32, tag="tmp")
                        for h in range(nh):
                            nc.vector.scalar_tensor_tensor(
                                tmp[:, h * C:(h + 1) * C],
                                xls[:, h * C:(h + 1) * C],
                                pself[:, h:h + 1],
                                ps[:, h * C:(h + 1) * C],
                                op0=OP.mult, op1=OP.add)
                            nc.vector.scalar_tensor_tensor(
                                tmp[:, h * C:(h + 1) * C],
                                tmp[:, h * C:(h + 1) * C],
                                rc[:, h:h + 1],
                                ra[:, h * C:(h + 1) * C],
                                op0=OP.mult, op1=OP.mult)
                        nc.vector.tensor_add(tmp[:], tmp[:], biast[:])
                        nc.vector.tensor_scalar_max(hp_tile[:, w, :],
                                                    tmp[:], 0.0)
                        sq = es.tile([128, feat], BF, tag="sq")
                        nc.vector.tensor_mul(sq[:], hp_tile[:, w, :],
                                             hp_tile[:, w, :])
                        mc = 0 if w < W - 1 else 1
                        sp = sps.tile([1, 2, feat], F32, tag="sp")
                        nc.tensor.matmul(sp[:, 0, :], c_masks[:, mc:mc + 1],
                                         hp_tile[:, w, :],
                                         start=True, stop=True)
                        nc.tensor.matmul(sp[:, 1, :], c_masks[:, mc:mc + 1],
                                         sq[:], start=True, stop=True)
                        nc.vector.tensor_add(ssum3, ssum3, sp[:])

                    WB = batches[1][0] - batches[0][0] if nb > 1 else W
                    pend_g = {}
                    pend_s = {}
                    pend_x = {}
                    pend_r = {}
                    pend_e = {}
                    pend_p = {}
                    emitted = 0

                    def need_batches(upto_w):
                        nonlocal emitted
                        while (emitted < nb
                               and batches[emitted][0] <= upto_w):
                            pend_g[emitted] = emit_gather(emitted)
                            emitted += 1

                    for i in range(W + 3):
                        if i < W:
                            need_batches(i + look * WB)
                            sw, sal, xls = emit_streams(i)
                            pend_s[i] = (sw, sal)
                            pend_x[i] = xls
                        if 1 <= i <= W:
                            w = i - 1
                            gl = pend_g[w // WB]
                            sw, sal = pend_s[w]
                            pend_r[w], pend_e[w] = emit_chain(
                                w, gl, sw, pend_x[w])
                        if 2 <= i <= W + 1:
                            w = i - 2
                            _, sal = pend_s.pop(w)
                            pend_p[w] = emit_scatter(w, sal,
                                                     pend_r.pop(w))
                            if (w + 1) % WB == 0 or w == W - 1:
                                pend_g.pop(w // WB, None)
                        if 3 <= i:
                            w = i - 3
                            emit_finalize(w, pend_p.pop(w),
                                          pend_x.pop(w), pend_e.pop(w))
                nc.sync.dma_start(out=st_in[:, :], in_=ssum[:])

            # ======== persistent activation tiles (nested scopes) ======
            with tc.tile_pool(name="actA", bufs=1) as pA:
                XR0s = pA.tile([128, W, FEAT0], BF, tag="XR0s")
                HP0s = pA.tile([128, W, FEAT0], BF, tag="HP0s")

                # ---------------- stage A -----------------------------
                # Full XL0 table computed locally (no AllGather).
                with nc.named_scope("stageA"):
                    with tc.tile_pool(name="sta", bufs=4) as sa, \
                         tc.tile_pool(name="sta_ps", bufs=4,
                                      space="PSUM") as sap:
                        XB = 8
                        for t0 in range(0, WF, XB):
                            xt_t = sa.tile([IN, XB * 128], BF, tag="xt")
                            nc.sync.dma_start(
                                out=xt_t[:],
                                in_=xTf[:, t0 * 128:(t0 + XB) * 128])
                            sb4 = sa.tile([128, XB, FEAT0], BF, tag="sb")
                            for j0 in range(0, XB, 2):
                                ps = sap.tile([128, 2, FEAT0], F32,
                                              tag="ps")
                                for j in range(2):
                                    nc.tensor.matmul(
                                        ps[:, j, :],
                                        xt_t[:, (j0 + j) * 128:
                                             (j0 + j + 1) * 128],
                                        c_wl0[:], start=True, stop=True)
                                if j0 % 4 == 0:
                                    nc.scalar.copy(
                                        sb4[:, j0:j0 + 2, :], ps[:])
                                else:
                                    nc.vector.tensor_copy(
                                        sb4[:, j0:j0 + 2, :], ps[:])
                            dview = XL0[t0 * 128:(t0 + XB) * 128,
                                        0:FEAT0].rearrange(
                                "(i p) c -> p i c", p=128)
                            nc.sync.dma_start(out=dview, in_=sb4[:])
                        # local xr0 slice (SBUF resident) + local xl0
                        # rows (DRAM, for the dense self-loop path)
                        xt_l = sa.tile([IN, SP], BF, tag="xtl")
                        nc.sync.dma_start(out=xt_l[:], in_=xT[:, :])
                        for t in range(W):
                            ps = sap.tile([128, FEAT0], F32, tag="ps")
                            nc.tensor.matmul(
                                ps[:], xt_l[:, t * 128:(t + 1) * 128],
                                c_wr0[:], start=True, stop=True)
                            nc.vector.tensor_add(XR0s[:, t, :], ps[:],
                                                 c_bs0[:])
                            psl = sap.tile([128, FEAT0], F32, tag="ps")
                            nc.tensor.matmul(
                                psl[:], xt_l[:, t * 128:(t + 1) * 128],
                                c_wl0[:], start=True, stop=True)
                            sbl = sa.tile([128, FEAT0], BF, tag="sbl")
                            nc.scalar.copy(sbl[:], psl[:])
                            nc.sync.dma_start(
                                out=XL0loc[t * 128:(t + 1) * 128, :],
                                in_=sbl[:])

                # ================= layer 0 =============================
                with nc.named_scope("l0edge"):
                    edge_pass(XL0, TBL0, FEAT0, H, meta["npos0"], c_ra0,
                              c_bias0, HP0s, XR0s, ST0i, srcw0, bat0,
                              meta["scs0"],
                              lambda w: XL0loc[w * 128:(w + 1) * 128, :],
                              egbufs=4, look=2)

                with tc.tile_pool(name="actB", bufs=1) as pB:
                    XR1s = pB.tile([128, W, OUTC], BF, tag="XR1s")
                    HP1s = pB.tile([128, W, OUTC], BF, tag="HP1s")

                    with tc.tile_pool(name="pht", bufs=1) as pht:
                        HT0 = pht.tile([128, W, 128], BF, tag="HT0")
                        HT1 = pht.tile([64, W, 128], BF, tag="HT1")
                        with nc.named_scope("bn0"):
                            nc.gpsimd.collective_compute(
                                "AllReduce", OP.add, replica_groups=[cores],
                                ins=[ST0i[:, :]], outs=[ST0o[:, :]])
                            # transpose hoist: h^T tiles built while the
                            # AllReduce is in flight (no dependency)
                            with tc.tile_pool(name="p2t", bufs=2,
                                              space="PSUM") as ptp:
                                for w in range(W):
                                    pt0 = ptp.tile([128, 128], BF,
                                                   tag="pt0")
                                    nc.tensor.transpose(
                                        pt0[:], HP0s[:, w, 0:128],
                                        c_ident[:])
                                    nc.scalar.copy(HT0[:, w, :], pt0[:])
                                    pt1 = ptp.tile([64, 128], BF,
                                                   tag="pt1")
                                    nc.tensor.transpose(
                                        pt1[:], HP0s[:, w, 128:192],
                                        c_ident[:])
                                    nc.scalar.copy(HT1[:, w, :], pt1[:])
                            arow0, brow0 = bn_coeffs(ST0o, FEAT0, c_g0,
                                                     c_b0, meta["N"],
                                                     reps=False)

                        with nc.named_scope("pass2"):
                            with tc.tile_pool(name="p2", bufs=3) as p2, \
                                 tc.tile_pool(name="p2c", bufs=1) as p2c, \
                                 tc.tile_pool(name="p2cp", bufs=1,
                                              space="PSUM") as ppc, \
                                 tc.tile_pool(name="p2ps", bufs=2,
                                              space="PSUM") as pp:
                                # BN folded into transposed-side weights:
                                # xl1 = h^T-matmul with A-scaled Wl1 rows
                                # + (brow@Wl1) row; same for Wr1 (+BSUM1).
                                ab = p2c.tile([1, FEAT0], BF, tag="ab")
                                nc.vector.tensor_copy(ab[:], arow0[:])
                                bb = p2c.tile([1, FEAT0], BF, tag="bb")
                                nc.vector.tensor_copy(bb[:], brow0[:])
                                pac = ppc.tile([128, 4], BF, tag="pac")
                                nc.tensor.transpose(pac[:, 0:1],
                                                    ab[:, 0:128],
                                                    c_ident[0:1, 0:1])
                                nc.tensor.transpose(pac[:, 2:3],
                                                    bb[:, 0:128],
                                                    c_ident[0:1, 0:1])
                                pac2 = ppc.tile([64, 4], BF, tag="pac2")
                                nc.tensor.transpose(pac2[:, 0:1],
                                                    ab[:, 128:192],
                                                    c_ident[0:1, 0:1])
                                nc.tensor.transpose(pac2[:, 2:3],
                                                    bb[:, 128:192],
                                                    c_ident[0:1, 0:1])
                                ac = p2c.tile([128, 2], BF, tag="ac")
                                nc.scalar.copy(ac[:, 0:1], pac[:, 0:1])
                                nc.scalar.copy(ac[:, 1:2], pac[:, 2:3])
                                ac2 = p2c.tile([64, 2], BF, tag="ac2")
                                nc.scalar.copy(ac2[:, 0:1], pac2[:, 0:1])
                                nc.scalar.copy(ac2[:, 1:2], pac2[:, 2:3])
                                wl1as = p2c.tile([128, OUTC], BF,
                                                 tag="wl1as")
                                nc.vector.tensor_mul(
                                    wl1as[:], c_wl1a[:],
                                    ac[:, 0:1].broadcast_to([128, OUTC]))
                                wl1bs = p2c.tile([64, OUTC], BF,
                                                 tag="wl1bs")
                                nc.vector.tensor_mul(
                                    wl1bs[:], c_wl1b[:],
                                    ac2[:, 0:1].broadcast_to([64, OUTC]))
                                wr1as = p2c.tile([128, OUTC], BF,
                                                 tag="wr1as")
                                nc.vector.tensor_mul(
                                    wr1as[:], c_wr1a[:],
                                    ac[:, 0:1].broadcast_to([128, OUTC]))
                                wr1bs = p2c.tile([64, OUTC], BF,
                                                 tag="wr1bs")
                                nc.vector.tensor_mul(
                                    wr1bs[:], c_wr1b[:],
                                    ac2[:, 0:1].broadcast_to([64, OUTC]))
                                # bias rows: brow @ Wl1 / Wr1
                                pbc = ppc.tile([1, 2 * OUTC], F32,
                                               tag="pbc")
                                nc.tensor.matmul(pbc[:, 0:OUTC],
                                                 ac[:, 1:2], c_wl1a[:],
                                                 start=True, stop=False)
                                nc.tensor.matmul(pbc[:, 0:OUTC],
                                                 ac2[:, 1:2], c_wl1b[:],
                                                 start=False, stop=True)
                                nc.tensor.matmul(pbc[:, OUTC:2 * OUTC],
                                                 ac[:, 1:2], c_wr1a[:],
                                                 start=True, stop=False)
                                nc.tensor.matmul(pbc[:, OUTC:2 * OUTC],
                                                 ac2[:, 1:2], c_wr1b[:],
                                                 start=False, stop=True)
                                bcrow = p2c.tile([1, 2 * OUTC], F32,
                                                 tag="bcrow")
                                nc.vector.tensor_copy(bcrow[:], pbc[:])
                                pbr = ppc.tile([128, 2 * OUTC], F32,
                                               tag="pbr")
                                nc.tensor.matmul(pbr[:], c_one1[:],
                                                 bcrow[:],
                                                 start=True, stop=True)
                                BCL = p2c.tile([128, OUTC], BF, tag="BCL")
                                nc.scalar.copy(BCL[:], pbr[:, 0:OUTC])
                                BCR = p2c.tile([128, OUTC], F32,
                                               tag="BCR")
                                nc.vector.tensor_add(
                                    BCR[:], pbr[:, OUTC:2 * OUTC],
                                    c_bs1[:])
                                for w in range(W):
                                    px = pp.tile([128, OUTC], F32,
                                                 tag="px")
                                    nc.tensor.matmul(px[:], HT0[:, w, :],
                                                     wl1as[:],
                                                     start=True,
                                                     stop=False)
                                    nc.tensor.matmul(px[:], HT1[:, w, :],
                                                     wl1bs[:],
                                                     start=False,
                                                     stop=True)
                                    xs = p2.tile([128, OUTC], BF,
                                                 tag="xs")
                                    nc.vector.tensor_add(xs[:], px[:],
                                                         BCL[:])
                                    nc.sync.dma_start(
                                        out=XL1s[w * 128:(w + 1) * 128,
                                                 0:OUTC],
                                        in_=xs[:])
                                    px2 = pp.tile([128, OUTC], F32,
                                                  tag="px")
                                    nc.tensor.matmul(px2[:],
                                                     HT0[:, w, :],
                                                     wr1as[:],
                                                     start=True,
                                                     stop=False)
                                    nc.tensor.matmul(px2[:],
                                                     HT1[:, w, :],
                                                     wr1bs[:],
                                                     start=False,
                                                     stop=True)
                                    nc.vector.tensor_add(XR1s[:, w, :],
                                                         px2[:], BCR[:])

                    with nc.named_scope("ag1"):
                        nc.gpsimd.collective_compute(
                            "AllGather", OP.bypass, replica_groups=[cores],
                            ins=[XL1s[:, :]], outs=[XL1[:, :]])

                    # ================= layer 1 =========================
                    with nc.named_scope("l1edge"):
                        edge_pass(XL1, TBL1, OUTC, 1, meta["npos1"],
                                  c_ra1, c_bias1, HP1s, XR1s, ST1i,
                                  srcw1, bat1, meta["scs1"],
                                  lambda w: XL1s[w * 128:(w + 1) * 128,
                                                 0:OUTC])
                    with nc.named_scope("fin"):
                        nc.gpsimd.collective_compute(
                            "AllReduce", OP.add, replica_groups=[cores],
                            ins=[ST1i[:, :]], outs=[ST1o[:, :]])
                        A1, B1t = bn_coeffs(ST1o, OUTC, c_g1, c_b1,
                                            meta["N"])
                        with tc.tile_pool(name="p3", bufs=3) as p3:
                            for w in range(W):
                                rows = min(128, SLICE - w * 128)
                                ob = p3.tile([128, OUTC], F32, tag="ob")
                                nc.vector.tensor_mul(ob[:], HP1s[:, w, :],
                                                     A1[:])
                                nc.vector.tensor_add(ob[:], ob[:], B1t[:])
                                nc.sync.dma_start(
                                    out=out[w * 128:w * 128 + rows, :],
                                    in_=ob[0:rows, :])

    nc.compile()
    return nc


# ---------------------------------------------------------------- entry

def kernel(**inputs):
    x = np.asarray(inputs["x"])
    edge_index = np.asarray(inputs["edge_index"])
    params = {k: np.asarray(v) for k, v in inputs.items()
              if k not in ("x", "edge_index")}
    n_cores = 8
    in_maps, meta, perm1 = preprocess(x, edge_index, params, n_cores)
    nc = build_program(meta)
    import os
    trace = bool(int(os.environ.get("K_TRACE", "0")))
    res = run_bass_kernel_spmd(nc, in_maps, list(range(n_cores)),
                               trace=trace)
    global LAST_RES
    LAST_RES = res
    if trace:
        print(f"HW exec time: {res.exec_time_ns} ns", flush=True)
    outs = [res.results[k]["out"] for k in range(n_cores)]
    full = np.concatenate(outs, 0)
    inv = np.argsort(perm1)
    return np.ascontiguousarray(full[:, inv]).astype(np.float32)


# revision 33
# speedup vs baseline: 1.1715x; 1.0020x over previous
"""GATv2 (2-layer, GAT_56727928046275) on 8 TRN2 NeuronCores — v3.

Strategy:
  - Nodes sharded by destination across 8 cores (6250 dst nodes each).
  - Edges (incl. self-loops) partitioned by dst, grouped into 128-dst
    "windows"; per-node softmax + scatter-add stay core-local via
    per-window PSUM accumulation with indicator matmuls.
  - Layer-0 xl table computed LOCALLY IN FULL on every core from a
    replicated x (no AllGather on the critical path); layer-1 table
    still AllGathered (activations are distributed).
  - xl tables carry NO bias: both linear biases are folded into the
    xr side (logits) and the finalize bias (scatter output).
  - Per-edge xl rows fetched with gpsimd dma_gather, full-row payloads
    (512B / 256B).  SWDGE desc-gen (~4.5ns/desc, Pool engine) is the
    floor; ops are 1024-desc, 4-queue round-robin, window-batched on
    layer 1.
  - z = xl[src]+xr[dst] built on the PE (indicator matmul + identity
    accumulate into PSUM); leaky-relu on Scalar (PSUM -> packed bf16);
    DVE does reduces + softmax muls; scatter via indicator matmuls.
  - xr/h tables SBUF-resident between phases; BN stats via masked
    ones-matmul column sums + AllReduce.
"""

import numpy as np
import ml_dtypes

import concourse.bass as bass
import concourse.mybir as mybir
import concourse.tile as tile
from concourse.bacc import Bacc
from concourse.bass_utils import run_bass_kernel_spmd

BF = mybir.dt.bfloat16
F32 = mybir.dt.float32
I16 = mybir.dt.int16
OP = mybir.AluOpType
AF = mybir.ActivationFunctionType
bf16 = ml_dtypes.bfloat16

NEG_SLOPE = 0.2
BN_EPS = 1e-5

import os as _os
B0 = 1   # window batch for layer-0 gathers
B1 = 4   # window batch for layer-1 gathers
# groups per gather op; 8 = 1024 idxs = full SWDGE ring. Smaller ops let
# descriptor GENERATION of op N+1 overlap ring DRAIN of op N.
GCAP = int(_os.environ.get("K_GCAP", "8"))


# ---------------------------------------------------------------- host prep

def _wrap16(a):
    a = np.asarray(a, np.int16).reshape(-1, 16).T  # [16, n/16]
    return np.tile(a, (8, 1))


def _rep(v, dt=np.float32):
    v = np.asarray(v, dt).reshape(1, -1)
    return np.ascontiguousarray(np.broadcast_to(v, (128, v.shape[1])))


def _sign_perm(att):
    H, C = att.shape
    perm = np.zeros(H * C, np.int64)
    npos = np.zeros(H, np.int64)
    for h in range(H):
        a = att[h]
        pos = np.nonzero(a >= 0)[0]
        neg = np.nonzero(a < 0)[0]
        perm[h * C:(h + 1) * C] = h * C + np.concatenate([pos, neg])
        npos[h] = len(pos)
    return perm, npos


def _mkbatches(W, B):
    return [list(range(s, min(s + B, W))) for s in range(0, W, B)]


def preprocess(x, edge_index, params, n_cores=8):
    N, IN = x.shape
    SLICE = N // n_cores
    SP = ((SLICE + 127) // 128) * 128          # padded slice rows (6272)
    W = SP // 128                              # windows per core (49)
    NP = n_cores * SP
    BLK = (n_cores // 2) * SP                  # int16 table split row (25088)

    # PyG's added self-loops are handled by a dense per-window path (no
    # gather descriptors); pre-existing (n,n) edges stay in the lists.
    src = np.asarray(edge_index[0], np.int64)
    dst = np.asarray(edge_index[1], np.int64)
    core = dst // SLICE
    row = (src // SLICE) * SP + (src % SLICE)

    ecnt0 = np.zeros((n_cores, W), np.int64)
    ecnt1 = np.zeros((n_cores, W), np.int64)
    lists = [[None] * W for _ in range(n_cores)]
    for k in range(n_cores):
        m = core == k
        r, d = row[m], dst[m] - k * SLICE
        win = d // 128
        blk = (r >= BLK).astype(np.int64)
        order = np.lexsort((blk, win))
        r, d, win, blk = r[order], d[order], win[order], blk[order]
        for w in range(W):
            wm = win == w
            rw, dw, bw = r[wm], d[wm], blk[wm]
            b0 = bw == 0
            lists[k][w] = (rw[b0], dw[b0], rw[~b0], dw[~b0])
            ecnt0[k][w] = int(b0.sum())
            ecnt1[k][w] = int((~b0).sum())

    G0 = np.maximum(1, (ecnt0.max(0) + 127) // 128)   # [W]
    G1 = np.maximum(1, (ecnt1.max(0) + 127) // 128)
    GW = G0 + G1
    NG = int(GW.sum())

    def build_srcw(batches):
        arr = np.zeros((n_cores, 128, NG * 8), np.int16)
        scs = {}
        sc = 0
        for bi, ws in enumerate(batches):
            scs[bi] = sc
            for k in range(n_cores):
                c = sc
                for w in ws:  # A blocks
                    r0, _, _, _ = lists[k][w]
                    s0 = np.zeros(int(G0[w]) * 128, np.int64)
                    s0[:len(r0)] = r0
                    arr[k][:, c:c + int(G0[w]) * 8] = _wrap16(s0)
                    c += int(G0[w]) * 8
                for w in ws:  # B blocks
                    _, _, r1, _ = lists[k][w]
                    s1 = np.zeros(int(G1[w]) * 128, np.int64)
                    s1[:len(r1)] = r1 - BLK
                    arr[k][:, c:c + int(G1[w]) * 8] = _wrap16(s1)
                    c += int(G1[w]) * 8
            sc += sum(int(GW[w]) for w in ws) * 8
        return arr, scs

    bat0 = _mkbatches(W, B0)
    bat1 = _mkbatches(W, B1)
    srcw0, scs0 = build_srcw(bat0)
    srcw1, scs1 = build_srcw(bat1)

    gcs = [0] * (W + 1)
    for w in range(W):
        gcs[w + 1] = gcs[w] + int(GW[w])
    swin = np.zeros((n_cores, 128, NG * 128), bf16)
    salh = np.zeros((n_cores, 128, NG * 128), bf16)
    for k in range(n_cores):
        for w in range(W):
            r0, d0, r1, d1 = lists[k][w]
            n0, n1 = len(r0), len(r1)
            gw = int(GW[w])
            rl = np.full(gw * 128, -1.0, np.float32)
            rl[:n0] = (d0 - w * 128).astype(np.float32)
            rl[int(G0[w]) * 128:int(G0[w]) * 128 + n1] = (
                d1 - w * 128).astype(np.float32)
            gc = gcs[w]
            sw = (np.arange(128)[:, None] == rl[None, :]).astype(bf16)
            swin[k][:, gc * 128:(gc + gw) * 128] = sw
            se = (rl.reshape(gw, 128).T[:, :, None]
                  == np.arange(128)[None, None, :]).astype(bf16)
            salh[k][:, gc * 128:(gc + gw) * 128] = se.reshape(128, -1)

    p = params
    H, C = p["att0"].shape
    OUTC = p["att1"].shape[1]
    perm0, npos0 = _sign_perm(np.asarray(p["att0"]))
    perm1, npos1 = _sign_perm(np.asarray(p["att1"]))
    aab0 = np.abs(np.asarray(p["att0"]).reshape(-1)[perm0])
    aab1 = np.abs(np.asarray(p["att1"]).reshape(-1)[perm1])

    def scale_cols(Wm, b, perm, aab):
        Wp = np.asarray(Wm)[:, perm] * aab[None, :]
        bp = np.asarray(b)[perm] * aab
        return Wp, bp

    Wl0p, bl0p = scale_cols(p["Wl0"], p["bl0"], perm0, aab0)
    Wr0p, br0p = scale_cols(p["Wr0"], p["br0"], perm0, aab0)
    Wl1p, bl1p = scale_cols(np.asarray(p["Wl1"])[perm0, :], p["bl1"],
                            perm1, aab1)
    Wr1p, br1p = scale_cols(np.asarray(p["Wr1"])[perm0, :], p["br1"],
                            perm1, aab1)

    FEAT0 = H * C            # 192
    TBL0 = 256               # bf16 cols -> 512B row stride
    TBL1 = 128               # bf16 cols -> 256B row stride

    shared = {
        "Wl0p": Wl0p.astype(bf16), "Wr0p": Wr0p.astype(bf16),
        "BSUM0": _rep(bl0p + br0p, bf16),
        "Wl1a": Wl1p[:128].astype(bf16), "Wl1b": Wl1p[128:].astype(bf16),
        "Wr1a": Wr1p[:128].astype(bf16), "Wr1b": Wr1p[128:].astype(bf16),
        "BSUM1": _rep(bl1p + br1p),
        "RA0": _rep(1.0 / aab0), "RA1": _rep(1.0 / aab1),
        "BIAS0": _rep(np.asarray(p["bias0"])[perm0]
                      + np.asarray(p["bl0"])[perm0]),
        "BIAS1": _rep(np.asarray(p["bias1"])[perm1]
                      + np.asarray(p["bl1"])[perm1]),
        "gamma0": np.asarray(p["gamma0"])[perm0].reshape(1, -1).astype(
            np.float32),
        "beta0": np.asarray(p["beta0"])[perm0].reshape(1, -1).astype(
            np.float32),
        "gamma1": np.asarray(p["gamma1"])[perm1].reshape(1, -1).astype(
            np.float32),
        "beta1": np.asarray(p["beta1"])[perm1].reshape(1, -1).astype(
            np.float32),
        "IDENT": np.eye(128, dtype=bf16),
        "ONE1": np.ones((1, 128), np.float32),
    }
    mlast = np.zeros(128, np.float32)
    mlast[:SLICE - (W - 1) * 128] = 1.0
    shared["MASKS"] = np.stack(
        [np.ones(128, np.float32), mlast], 1).astype(bf16)

    # full padded transposed x (same on every core) + local slice
    xt = np.asarray(x).T  # [IN, N]
    xtf = np.zeros((IN, NP), np.float32)
    for k in range(n_cores):
        xtf[:, k * SP:k * SP + SLICE] = xt[:, k * SLICE:(k + 1) * SLICE]
    xtf = xtf.astype(bf16)

    in_maps = []
    for k in range(n_cores):
        m = {"xTf": xtf,
             "xT": np.ascontiguousarray(xtf[:, k * SP:(k + 1) * SP]),
             "srcw0": srcw0[k], "srcw1": srcw1[k],
             "swin": swin[k], "salh": salh[k]}
        for kk, v in shared.items():
            m[kk] = np.ascontiguousarray(v)
        in_maps.append(m)

    meta = dict(N=N, IN=IN, SLICE=SLICE, SP=SP, NP=NP, W=W, BLK=BLK,
                G0=[int(v) for v in G0], G1=[int(v) for v in G1],
                GW=[int(v) for v in GW], NG=NG,
                H=H, C=C, FEAT0=FEAT0, OUTC=OUTC, TBL0=TBL0, TBL1=TBL1,
                npos0=[int(v) for v in npos0], npos1=[int(v) for v in npos1],
                scs0=scs0, scs1=scs1,
                n_cores=n_cores)
    return in_maps, meta, perm1


# ---------------------------------------------------------------- program

def build_program(meta):
    n_cores = meta["n_cores"]
    IN, SP, NP, W = meta["IN"], meta["SP"], meta["NP"], meta["W"]
    G0, G1, GW, NG = meta["G0"], meta["G1"], meta["GW"], meta["NG"]
    H, C, FEAT0 = meta["H"], meta["C"], meta["FEAT0"]
    OUTC, TBL0, TBL1 = meta["OUTC"], meta["TBL0"], meta["TBL1"]
    SLICE = meta["SLICE"]
    BLKROW = (n_cores // 2) * SP
    WF = NP // 128                 # full-table windows (392)
    cores = list(range(n_cores))
    bat0 = _mkbatches(W, B0)
    bat1 = _mkbatches(W, B1)
    gcs = [0] * (W + 1)
    for w in range(W):
        gcs[w + 1] = gcs[w] + GW[w]

    nc = Bacc("TRN2", target_bir_lowering=False, debug=False,
              num_devices=n_cores, num_swdge_queues=4)

    def dp(name, shape, dt):
        return nc.declare_dram_parameter(name, list(shape), dt,
                                         isOutput=False)

    xTf = dp("xTf", [IN, NP], BF)
    xT = dp("xT", [IN, SP], BF)
    srcw0 = dp("srcw0", [128, NG * 8], I16)
    srcw1 = dp("srcw1", [128, NG * 8], I16)
    swin = dp("swin", [128, NG * 128], BF)
    salh = dp("salh", [128, NG * 128], BF)
    Wl0p = dp("Wl0p", [IN, FEAT0], BF)
    Wr0p = dp("Wr0p", [IN, FEAT0], BF)
    BSUM0 = dp("BSUM0", [128, FEAT0], BF)
    Wl1a = dp("Wl1a", [128, OUTC], BF)
    Wl1b = dp("Wl1b", [FEAT0 - 128, OUTC], BF)
    Wr1a = dp("Wr1a", [128, OUTC], BF)
    Wr1b = dp("Wr1b", [FEAT0 - 128, OUTC], BF)
    BSUM1 = dp("BSUM1", [128, OUTC], F32)
    RA0 = dp("RA0", [128, FEAT0], F32)
    RA1 = dp("RA1", [128, OUTC], F32)
    BIAS0 = dp("BIAS0", [128, FEAT0], F32)
    BIAS1 = dp("BIAS1", [128, OUTC], F32)
    gamma0 = dp("gamma0", [1, FEAT0], F32)
    beta0 = dp("beta0", [1, FEAT0], F32)
    gamma1 = dp("gamma1", [1, OUTC], F32)
    beta1 = dp("beta1", [1, OUTC], F32)
    IDENT = dp("IDENT", [128, 128], BF)
    ONE1 = dp("ONE1", [1, 128], F32)
    MASKS = dp("MASKS", [128, 2], BF)

    out = nc.declare_dram_parameter("out", [SLICE, OUTC], F32, isOutput=True)

    # internal DRAM
    XL0 = nc.dram_tensor("XL0", [NP, TBL0], BF)
    XL0loc = nc.dram_tensor("XL0loc", [SP, FEAT0], BF)
    XL1s = nc.dram_tensor("XL1s", [SP, TBL1], BF)
    XL1 = nc.dram_tensor("XL1", [NP, TBL1], BF, addr_space="Shared")
    ST0i = nc.dram_tensor("ST0i", [1, 2 * FEAT0], F32)
    ST0o = nc.dram_tensor("ST0o", [1, 2 * FEAT0], F32, addr_space="Shared")
    ST1i = nc.dram_tensor("ST1i", [1, 2 * OUTC], F32)
    ST1o = nc.dram_tensor("ST1o", [1, 2 * OUTC], F32, addr_space="Shared")

    with tile.TileContext(nc) as tc:
        with tc.tile_pool(name="const", bufs=1) as cp:
            def ld(par, shape, dt):
                t = cp.tile(list(shape), dt, tag=f"c_{par.name}")
                nc.sync.dma_start(out=t[:], in_=par[:])
                return t

            c_ident = ld(IDENT, [128, 128], BF)
            c_one1 = ld(ONE1, [1, 128], F32)
            c_masks = ld(MASKS, [128, 2], BF)
            c_wl0 = ld(Wl0p, [IN, FEAT0], BF)
            c_wr0 = ld(Wr0p, [IN, FEAT0], BF)
            c_bs0 = ld(BSUM0, [128, FEAT0], BF)
            c_wl1a = ld(Wl1a, [128, OUTC], BF)
            c_wl1b = ld(Wl1b, [FEAT0 - 128, OUTC], BF)
            c_wr1a = ld(Wr1a, [128, OUTC], BF)
            c_wr1b = ld(Wr1b, [FEAT0 - 128, OUTC], BF)
            c_bs1 = ld(BSUM1, [128, OUTC], F32)
            c_ra0 = ld(RA0, [128, FEAT0], F32)
            c_ra1 = ld(RA1, [128, OUTC], F32)
            c_bias0 = ld(BIAS0, [128, FEAT0], F32)
            c_bias1 = ld(BIAS1, [128, OUTC], F32)
            c_g0 = ld(gamma0, [1, FEAT0], F32)
            c_b0 = ld(beta0, [1, FEAT0], F32)
            c_g1 = ld(gamma1, [1, OUTC], F32)
            c_b1 = ld(beta1, [1, OUTC], F32)

            # ---------------- BN coeff helper --------------------------
            def bn_coeffs(st_out, feat, g_row, b_row, nodes, reps=True):
                st = cp.tile([1, 2 * feat], F32, tag=f"st{feat}")
                nc.sync.dma_start(out=st[:], in_=st_out[:, :])
                mu = cp.tile([1, feat], F32, tag=f"mu{feat}")
                var = cp.tile([1, feat], F32, tag=f"va{feat}")
                nc.vector.tensor_scalar_mul(mu[:], st[:, 0:feat],
                                            1.0 / nodes)
                nc.vector.tensor_scalar_mul(var[:], st[:, feat:2 * feat],
                                            1.0 / nodes)
                t = cp.tile([1, feat], F32, tag=f"t{feat}")
                nc.vector.tensor_mul(t[:], mu[:], mu[:])
                nc.vector.tensor_sub(var[:], var[:], t[:])
                nc.vector.tensor_scalar_add(var[:], var[:], BN_EPS)
                nc.scalar.activation(t[:], var[:], AF.Sqrt)
                nc.vector.reciprocal(t[:], t[:])
                arow = cp.tile([1, feat], F32, tag=f"ar{feat}")
                nc.vector.tensor_mul(arow[:], g_row[:], t[:])
                brow = cp.tile([1, feat], F32, tag=f"br{feat}")
                nc.vector.tensor_mul(t[:], mu[:], arow[:])
                nc.vector.tensor_sub(brow[:], b_row[:], t[:])
                if not reps:
                    return arow, brow
                with tc.tile_pool(name="bnp", bufs=2, space="PSUM") as bp:
                    pa = bp.tile([128, feat], F32, tag="pa")
                    nc.tensor.matmul(pa[:], c_one1[:], arow[:],
                                     start=True, stop=True)
                    Ar = cp.tile([128, feat], BF, tag=f"A{feat}")
                    nc.scalar.copy(Ar[:], pa[:])
                    pb2 = bp.tile([128, feat], F32, tag="pb")
                    nc.tensor.matmul(pb2[:], c_one1[:], brow[:],
                                     start=True, stop=True)
                    Br = cp.tile([128, feat], BF, tag=f"B{feat}")
                    nc.scalar.copy(Br[:], pb2[:])
                return Ar, Br

            # ---------------- edge pass helper -------------------------
            def edge_pass(XLt, tblc, feat, nh, npos, ra, biast,
                          hp_tile, xr_tile, st_in, srcw_par, batches, scs,
                          xloc, egbufs=3, look=1):
                ssum = cp.tile([1, 2 * feat], F32, tag=f"ssum{feat}")
                nc.vector.memset(ssum[:], 0.0)
                ssum3 = ssum[:].rearrange("p (a f) -> p a f", a=2)
                nb = len(batches)
                aoff = {}
                boff = {}
                bcols = []
                for bi, ws in enumerate(batches):
                    ga = sum(G0[w] for w in ws)
                    o = 0
                    for w in ws:
                        aoff[w] = o
                        o += G0[w]
                    o = ga
                    for w in ws:
                        boff[w] = o
                        o += G1[w]
                    bcols.append(o)
                gmax = max(bcols)

                with tc.tile_pool(name="eg", bufs=egbufs) as eg, \
                     tc.tile_pool(name="esrc", bufs=1) as esrc, \
                     tc.tile_pool(name="esw", bufs=2) as esw, \
                     tc.tile_pool(name="est", bufs=3) as est, \
                     tc.tile_pool(name="exl", bufs=4) as exl, \
                     tc.tile_pool(name="es", bufs=2) as es, \
                     tc.tile_pool(name="zps", bufs=4, space="PSUM") as zps, \
                     tc.tile_pool(name="eps", bufs=2, space="PSUM") as eps, \
                     tc.tile_pool(name="sps", bufs=1, space="PSUM") as sps:
                    c_src = esrc.tile([128, NG * 8], I16, tag="src")
                    nc.sync.dma_start(out=c_src[:], in_=srcw_par[:])
                    qrr = [0]

                    def gathers(dst_tile, col0, tbl_ap, idx_col0, n_groups):
                        done = 0
                        while done < n_groups:
                            cnt = min(GCAP, n_groups - done)
                            nc.gpsimd.dma_gather(
                                dst_tile[:, col0 + done:col0 + done + cnt,
                                         :],
                                tbl_ap,
                                c_src[:, idx_col0 + done * 8:
                                      idx_col0 + (done + cnt) * 8],
                                num_idxs=cnt * 128, num_idxs_reg=cnt * 128,
                                elem_size=tblc, elem_step=tblc,
                                queue_num=qrr[0])
                            qrr[0] = (qrr[0] + 1) % 4
                            done += cnt

                    def emit_gather(bi):
                        ws = batches[bi]
                        gl = eg.tile([128, gmax, tblc], BF, tag="gl")
                        ga = sum(G0[w] for w in ws)
                        gb = sum(G1[w] for w in ws)
                        sc = scs[bi]
                        gathers(gl, 0, XLt[:, :], sc, ga)
                        gathers(gl, ga, XLt[BLKROW:, :], sc + ga * 8, gb)
                        return gl

                    def emit_streams(w):
                        gw, gc = GW[w], gcs[w]
                        sw = esw.tile([128, gw, 128], BF, tag="sw")
                        nc.sync.dma_start(
                            out=sw[:],
                            in_=swin[:, gc * 128:(gc + gw) * 128])
                        sal = est.tile([128, gw, 128], BF, tag="sal")
                        nc.sync.dma_start(
                            out=sal[:],
                            in_=salh[:, gc * 128:(gc + gw) * 128])
                        xls = exl.tile([128, feat], BF, tag="xls")
                        nc.sync.dma_start(out=xls[:], in_=xloc(w))
                        return sw, sal, xls

                    def emit_chain(w, gl, sw, xls):
                        gw = GW[w]
                        xw = xr_tile[:, w, :]
                        z = es.tile([128, gw, feat], BF, tag="z")
                        ck = 2 if feat > 96 else 4
                        segs = []
                        for j0 in range(0, G0[w], ck):
                            segs.append((j0, aoff[w] + j0,
                                         min(ck, G0[w] - j0)))
                        for j0 in range(0, G1[w], ck):
                            segs.append((G0[w] + j0, boff[w] + j0,
                                         min(ck, G1[w] - j0)))
                        for (zo, go, cnt) in segs:
                            ps = zps.tile([128, ck, feat], F32, tag="zp")
                            for j in range(cnt):
                                nc.tensor.matmul(
                                    ps[:, j, 0:feat], sw[:, zo + j, :],
                                    xw, start=True, stop=False)
                                nc.tensor.matmul(
                                    ps[:, j, 0:feat], c_ident[:],
                                    gl[:, go + j, 0:feat],
                                    start=False, stop=True)
                            nc.scalar.activation(
                                z[:, zo:zo + cnt, :], ps[:, 0:cnt, :],
                                AF.Prelu, alpha=NEG_SLOPE)
                        lp = es.tile([128, gw, nh], BF, tag="lp")
                        ln = es.tile([128, gw, nh], BF, tag="ln")
                        with nc.allow_low_precision(
                                reason="bf16 logit partial sums"):
                            for h in range(nh):
                                k = npos[h]
                                if k == 0:
                                    nc.vector.memset(lp[:, :, h], 0.0)
                                else:
                                    nc.vector.tensor_reduce(
                                        lp[:, :, h],
                                        z[:, :, h * C:h * C + k],
                                        axis=mybir.AxisListType.X,
                                        op=OP.add)
                                if k == C:
                                    nc.vector.memset(ln[:, :, h], 0.0)
                                else:
                                    nc.vector.tensor_reduce(
                                        ln[:, :, h],
                                        z[:, :, h * C + k:(h + 1) * C],
                                        axis=mybir.AxisListType.X,
                                        op=OP.add)
                        rhs = es.tile([128, gw, feat + 4], BF, tag="rhs")
                        lg = es.tile([128, gw, nh], BF, tag="lg")
                        nc.vector.tensor_sub(lg[:], lp[:], ln[:])
                        nc.scalar.activation(
                            rhs[:, :, feat:feat + nh], lg[:], AF.Exp)
                        pe = es.tile([128, gw, nh * C], BF, tag="pe")
                        nc.scalar.activation(
                            pe[:].rearrange("p g (h c) -> p g h c", c=C),
                            lg[:].unsqueeze(3).broadcast_to(
                                [128, gw, nh, C]),
                            AF.Exp)
                        nc.vector.tensor_mul(
                            rhs[:, 0:G0[w], 0:feat],
                            gl[:, aoff[w]:aoff[w] + G0[w], 0:feat],
                            pe[:, 0:G0[w], :])
                        nc.vector.tensor_mul(
                            rhs[:, G0[w]:gw, 0:feat],
                            gl[:, boff[w]:boff[w] + G1[w], 0:feat],
                            pe[:, G0[w]:gw, :])
                        # dense self-loop path (PyG add_self_loops)
                        zs = es.tile([128, feat], BF, tag="zs")
                        nc.vector.tensor_add(zs[:], xls[:], xw)
                        nc.scalar.activation(zs[:], zs[:], AF.Prelu,
                                             alpha=NEG_SLOPE)
                        lgs = es.tile([128, 2 * nh], BF, tag="lgs")
                        with nc.allow_low_precision(
                                reason="bf16 logit partial sums"):
                            for h in range(nh):
                                k = npos[h]
                                if k == 0:
                                    nc.vector.memset(lgs[:, h:h + 1], 0.0)
                                else:
                                    nc.vector.tensor_reduce(
                                        lgs[:, h:h + 1],
                                        zs[:, h * C:h * C + k],
                                        axis=mybir.AxisListType.X,
                                        op=OP.add)
                                if k == C:
                                    nc.vector.memset(
                                        lgs[:, nh + h:nh + h + 1], 0.0)
                                else:
                                    nc.vector.tensor_reduce(
                                        lgs[:, nh + h:nh + h + 1],
                                        zs[:, h * C + k:(h + 1) * C],
                                        axis=mybir.AxisListType.X,
                                        op=OP.add)
                        lgd = es.tile([128, nh], BF, tag="lgd")
                        nc.vector.tensor_sub(lgd[:], lgs[:, 0:nh],
                                             lgs[:, nh:2 * nh])
                        pself = exl.tile([128, nh], F32, tag="pself")
                        nc.scalar.activation(pself[:], lgd[:], AF.Exp)
                        return rhs, pself

                    def emit_scatter(w, sal, rhs):
                        gw = GW[w]
                        ps = eps.tile([128, feat + nh], F32, tag="acc")
                        for g in range(gw):
                            nc.tensor.matmul(
                                ps[:], sal[:, g, :], rhs[:, g, 0:feat + nh],
                                start=(g == 0), stop=(g == gw - 1))
                        return ps

                    def emit_finalize(w, ps, xls, pself):
                        dn = es.tile([128, nh], F32, tag="dn")
                        nc.vector.tensor_add(dn[:], ps[:, feat:feat + nh],
                                             pself[:])
                        nc.vector.tensor_scalar_max(dn[:], dn[:], 1e-30)
                        rc = es.tile([128, nh], F32, tag="rc")
                        nc.vector.reciprocal(rc[:], dn[:])
                        tmp = es.tile([128, feat], F32, tag="tmp")
                        for h in range(nh):
                            nc.vector.scalar_tensor_tensor(
                                tmp[:, h * C:(h + 1) * C],
                                xls[:, h * C:(h + 1) * C],
                                pself[:, h:h + 1],
                                ps[:, h * C:(h + 1) * C],
                                op0=OP.mult, op1=OP.add)
                            nc.vector.scalar_tensor_tensor(
                                tmp[:, h * C:(h + 1) * C],
                                tmp[:, h * C:(h + 1) * C],
                                rc[:, h:h + 1],
                                ra[:, h * C:(h + 1) * C],
                                op0=OP.mult, op1=OP.mult)
                        nc.vector.tensor_add(tmp[:], tmp[:], biast[:])
                        nc.vector.tensor_scalar_max(hp_tile[:, w, :],
                                                    tmp[:], 0.0)
                        sq = es.tile([128, feat], BF, tag="sq")
                        nc.vector.tensor_mul(sq[:], hp_tile[:, w, :],
                                             hp_tile[:, w, :])
                        mc = 0 if w < W - 1 else 1
                        sp = sps.tile([1, 2, feat], F32, tag="sp")
                        nc.tensor.matmul(sp[:, 0, :], c_masks[:, mc:mc + 1],
                                         hp_tile[:, w, :],
                                         start=True, stop=True)
                        nc.tensor.matmul(sp[:, 1, :], c_masks[:, mc:mc + 1],
                                         sq[:], start=True, stop=True)
                        nc.vector.tensor_add(ssum3, ssum3, sp[:])

                    WB = batches[1][0] - batches[0][0] if nb > 1 else W
                    pend_g = {}
                    pend_s = {}
                    pend_x = {}
                    pend_r = {}
                    pend_e = {}
                    pend_p = {}
                    emitted = 0

                    def need_batches(upto_w):
                        nonlocal emitted
                        while (emitted < nb
                               and batches[emitted][0] <= upto_w):
                            pend_g[emitted] = emit_gather(emitted)
                            emitted += 1

                    for i in range(W + 3):
                        if i < W:
                            need_batches(i + look * WB)
                            sw, sal, xls = emit_streams(i)
                            pend_s[i] = (sw, sal)
                            pend_x[i] = xls
                        if 1 <= i <= W:
                            w = i - 1
                            gl = pend_g[w // WB]
                            sw, sal = pend_s[w]
                            pend_r[w], pend_e[w] = emit_chain(
                                w, gl, sw, pend_x[w])
                        if 2 <= i <= W + 1:
                            w = i - 2
                            _, sal = pend_s.pop(w)
                            pend_p[w] = emit_scatter(w, sal,
                                                     pend_r.pop(w))
                            if (w + 1) % WB == 0 or w == W - 1:
                                pend_g.pop(w // WB, None)
                        if 3 <= i:
                            w = i - 3
                            emit_finalize(w, pend_p.pop(w),
                                          pend_x.pop(w), pend_e.pop(w))
                nc.sync.dma_start(out=st_in[:, :], in_=ssum[:])

            # ======== persistent activation tiles (nested scopes) ======
            with tc.tile_pool(name="actA", bufs=1) as pA:
                XR0s = pA.tile([128, W, FEAT0], BF, tag="XR0s")
                HP0s = pA.tile([128, W, FEAT0], BF, tag="HP0s")

                # ---------------- stage A -----------------------------
                # Full XL0 table computed locally (no AllGather).
                with nc.named_scope("stageA"):
                    with tc.tile_pool(name="sta", bufs=4) as sa, \
                         tc.tile_pool(name="sta_ps", bufs=4,
                                      space="PSUM") as sap:
                        XB = 8
                        for t0 in range(0, WF, XB):
                            xt_t = sa.tile([IN, XB * 128], BF, tag="xt")
                            nc.sync.dma_start(
                                out=xt_t[:],
                                in_=xTf[:, t0 * 128:(t0 + XB) * 128])
                            sb4 = sa.tile([128, XB, FEAT0], BF, tag="sb")
                            for j0 in range(0, XB, 2):
                                ps = sap.tile([128, 2, FEAT0], F32,
                                              tag="ps")
                                for j in range(2):
                                    nc.tensor.matmul(
                                        ps[:, j, :],
                                        xt_t[:, (j0 + j) * 128:
                                             (j0 + j + 1) * 128],
                                        c_wl0[:], start=True, stop=True)
                                if j0 % 4 == 0:
                                    nc.scalar.copy(
                                        sb4[:, j0:j0 + 2, :], ps[:])
                                else:
                                    nc.vector.tensor_copy(
                                        sb4[:, j0:j0 + 2, :], ps[:])
                            dview = XL0[t0 * 128:(t0 + XB) * 128,
                                        0:FEAT0].rearrange(
                                "(i p) c -> p i c", p=128)
                            nc.sync.dma_start(out=dview, in_=sb4[:])
                        # local xr0 slice (SBUF resident) + local xl0
                        # rows (DRAM, for the dense self-loop path)
                        xt_l = sa.tile([IN, SP], BF, tag="xtl")
                        nc.sync.dma_start(out=xt_l[:], in_=xT[:, :])
                        for t in range(W):
                            ps = sap.tile([128, FEAT0], F32, tag="ps")
                            nc.tensor.matmul(
                                ps[:], xt_l[:, t * 128:(t + 1) * 128],
                                c_wr0[:], start=True, stop=True)
                            nc.vector.tensor_add(XR0s[:, t, :], ps[:],
                                                 c_bs0[:])
                            psl = sap.tile([128, FEAT0], F32, tag="ps")
                            nc.tensor.matmul(
                                psl[:], xt_l[:, t * 128:(t + 1) * 128],
                                c_wl0[:], start=True, stop=True)
                            sbl = sa.tile([128, FEAT0], BF, tag="sbl")
                            nc.scalar.copy(sbl[:], psl[:])
                            nc.sync.dma_start(
                                out=XL0loc[t * 128:(t + 1) * 128, :],
                                in_=sbl[:])

                # ================= layer 0 =============================
                with nc.named_scope("l0edge"):
                    edge_pass(XL0, TBL0, FEAT0, H, meta["npos0"], c_ra0,
                              c_bias0, HP0s, XR0s, ST0i, srcw0, bat0,
                              meta["scs0"],
                              lambda w: XL0loc[w * 128:(w + 1) * 128, :],
                              egbufs=4, look=2)

                with tc.tile_pool(name="actB", bufs=1) as pB:
                    XR1s = pB.tile([128, W, OUTC], BF, tag="XR1s")
                    HP1s = pB.tile([128, W, OUTC], BF, tag="HP1s")

                    with tc.tile_pool(name="pht", bufs=1) as pht:
                        HT0 = pht.tile([128, W, 128], BF, tag="HT0")
                        HT1 = pht.tile([64, W, 128], BF, tag="HT1")
                        with nc.named_scope("bn0"):
                            nc.gpsimd.collective_compute(
                                "AllReduce", OP.add, replica_groups=[cores],
                                ins=[ST0i[:, :]], outs=[ST0o[:, :]])
                            # transpose hoist: h^T tiles built while the
                            # AllReduce is in flight (no dependency)
                            with tc.tile_pool(name="p2t", bufs=2,
                                              space="PSUM") as ptp:
                                for w in range(W):
                                    pt0 = ptp.tile([128, 128], BF,
                                                   tag="pt0")
                                    nc.tensor.transpose(
                                        pt0[:], HP0s[:, w, 0:128],
                                        c_ident[:])
                                    nc.scalar.copy(HT0[:, w, :], pt0[:])
                                    pt1 = ptp.tile([64, 128], BF,
                                                   tag="pt1")
                                    nc.tensor.transpose(
                                        pt1[:], HP0s[:, w, 128:192],
                                        c_ident[:])
                                    nc.scalar.copy(HT1[:, w, :], pt1[:])
                            arow0, brow0 = bn_coeffs(ST0o, FEAT0, c_g0,
                                                     c_b0, meta["N"],
                                                     reps=False)

                        with nc.named_scope("pass2"):
                            with tc.tile_pool(name="p2", bufs=3) as p2, \
                                 tc.tile_pool(name="p2c", bufs=1) as p2c, \
                                 tc.tile_pool(name="p2cp", bufs=1,
                                              space="PSUM") as ppc, \
                                 tc.tile_pool(name="p2ps", bufs=2,
                                              space="PSUM") as pp:
                                # BN folded into transposed-side weights:
                                # xl1 = h^T-matmul with A-scaled Wl1 rows
                                # + (brow@Wl1) row; same for Wr1 (+BSUM1).
                                ab = p2c.tile([1, FEAT0], BF, tag="ab")
                                nc.vector.tensor_copy(ab[:], arow0[:])
                                bb = p2c.tile([1, FEAT0], BF, tag="bb")
                                nc.vector.tensor_copy(bb[:], brow0[:])
                                pac = ppc.tile([128, 4], BF, tag="pac")
                                nc.tensor.transpose(pac[:, 0:1],
                                                    ab[:, 0:128],
                                                    c_ident[0:1, 0:1])
                                nc.tensor.transpose(pac[:, 2:3],
                                                    bb[:, 0:128],
                                                    c_ident[0:1, 0:1])
                                pac2 = ppc.tile([64, 4], BF, tag="pac2")
                                nc.tensor.transpose(pac2[:, 0:1],
                                                    ab[:, 128:192],
                                                    c_ident[0:1, 0:1])
                                nc.tensor.transpose(pac2[:, 2:3],
                                                    bb[:, 128:192],
                                                    c_ident[0:1, 0:1])
                                ac = p2c.tile([128, 2], BF, tag="ac")
                                nc.scalar.copy(ac[:, 0:1], pac[:, 0:1])
                                nc.scalar.copy(ac[:, 1:2], pac[:, 2:3])
                                ac2 = p2c.tile([64, 2], BF, tag="ac2")
                                nc.scalar.copy(ac2[:, 0:1], pac2[:, 0:1])
                                nc.scalar.copy(ac2[:, 1:2], pac2[:, 2:3])
                                wl1as = p2c.tile([128, OUTC], BF,
                                                 tag="wl1as")
                                nc.vector.tensor_mul(
                                    wl1as[:], c_wl1a[:],
                                    ac[:, 0:1].broadcast_to([128, OUTC]))
                                wl1bs = p2c.tile([64, OUTC], BF,
                                                 tag="wl1bs")
                                nc.vector.tensor_mul(
                                    wl1bs[:], c_wl1b[:],
                                    ac2[:, 0:1].broadcast_to([64, OUTC]))
                                wr1as = p2c.tile([128, OUTC], BF,
                                                 tag="wr1as")
                                nc.vector.tensor_mul(
                                    wr1as[:], c_wr1a[:],
                                    ac[:, 0:1].broadcast_to([128, OUTC]))
                                wr1bs = p2c.tile([64, OUTC], BF,
                                                 tag="wr1bs")
                                nc.vector.tensor_mul(
                                    wr1bs[:], c_wr1b[:],
                                    ac2[:, 0:1].broadcast_to([64, OUTC]))
                                # bias rows: brow @ Wl1 / Wr1
                                pbc = ppc.tile([1, 2 * OUTC], F32,
                                               tag="pbc")
                                nc.tensor.matmul(pbc[:, 0:OUTC],
                                                 ac[:, 1:2], c_wl1a[:],
                                                 start=True, stop=False)
                                nc.tensor.matmul(pbc[:, 0:OUTC],
                                                 ac2[:, 1:2], c_wl1b[:],
                                                 start=False, stop=True)
                                nc.tensor.matmul(pbc[:, OUTC:2 * OUTC],
                                                 ac[:, 1:2], c_wr1a[:],
                                                 start=True, stop=False)
                                nc.tensor.matmul(pbc[:, OUTC:2 * OUTC],
                                                 ac2[:, 1:2], c_wr1b[:],
                                                 start=False, stop=True)
                                bcrow = p2c.tile([1, 2 * OUTC], F32,
                                                 tag="bcrow")
                                nc.vector.tensor_copy(bcrow[:], pbc[:])
                                pbr = ppc.tile([128, 2 * OUTC], F32,
                                               tag="pbr")
                                nc.tensor.matmul(pbr[:], c_one1[:],
                                                 bcrow[:],
                                                 start=True, stop=True)
                                BCL = p2c.tile([128, OUTC], BF, tag="BCL")
                                nc.scalar.copy(BCL[:], pbr[:, 0:OUTC])
                                BCR = p2c.tile([128, OUTC], F32,
                                               tag="BCR")
                                nc.vector.tensor_add(
                                    BCR[:], pbr[:, OUTC:2 * OUTC],
                                    c_bs1[:])
                                for w in range(W):
                                    px = pp.tile([128, OUTC], F32,
                                                 tag="px")
                                    nc.tensor.matmul(px[:], HT0[:, w, :],
                                                     wl1as[:],
                                                     start=True,
                                                     stop=False)
                                    nc.tensor.matmul(px[:], HT1[:, w, :],
                                                     wl1bs[:],
                                                     start=False,
                                                     stop=True)
                                    xs = p2.tile([128, OUTC], BF,
                                                 tag="xs")
                                    nc.vector.tensor_add(xs[:], px[:],
                                                         BCL[:])
                                    nc.sync.dma_start(
                                        out=XL1s[w * 128:(w + 1) * 128,
                                                 0:OUTC],
                                        in_=xs[:])
                                    px2 = pp.tile([128, OUTC], F32,
                                                  tag="px")
                                    nc.tensor.matmul(px2[:],
                                                     HT0[:, w, :],
                                                     wr1as[:],
                                                     start=True,
                                                     stop=False)
                                    nc.tensor.matmul(px2[:],
                                                     HT1[:, w, :],
                                                     wr1bs[:],
                                                     start=False,
                                                     stop=True)
                                    nc.vector.tensor_add(XR1s[:, w, :],
                                                         px2[:], BCR[:])

                    with nc.named_scope("ag1"):
                        nc.gpsimd.collective_compute(
                            "AllGather", OP.bypass, replica_groups=[cores],
                            ins=[XL1s[:, :]], outs=[XL1[:, :]])

                    # ================= layer 1 =========================
                    with nc.named_scope("l1edge"):
                        edge_pass(XL1, TBL1, OUTC, 1, meta["npos1"],
                                  c_ra1, c_bias1, HP1s, XR1s, ST1i,
                                  srcw1, bat1, meta["scs1"],
                                  lambda w: XL1s[w * 128:(w + 1) * 128,
                                                 0:OUTC],
                                  egbufs=4, look=2)
                    with nc.named_scope("fin"):
                        nc.gpsimd.collective_compute(
                            "AllReduce", OP.add, replica_groups=[cores],
                            ins=[ST1i[:, :]], outs=[ST1o[:, :]])
                        A1, B1t = bn_coeffs(ST1o, OUTC, c_g1, c_b1,
                                            meta["N"])
                        with tc.tile_pool(name="p3", bufs=3) as p3:
                            for w in range(W):
                                rows = min(128, SLICE - w * 128)
                                ob = p3.tile([128, OUTC], F32, tag="ob")
                                nc.vector.tensor_mul(ob[:], HP1s[:, w, :],
                                                     A1[:])
                                nc.vector.tensor_add(ob[:], ob[:], B1t[:])
                                nc.sync.dma_start(
                                    out=out[w * 128:w * 128 + rows, :],
                                    in_=ob[0:rows, :])

    nc.compile()
    return nc


# ---------------------------------------------------------------- entry

def kernel(**inputs):
    x = np.asarray(inputs["x"])
    edge_index = np.asarray(inputs["edge_index"])
    params = {k: np.asarray(v) for k, v in inputs.items()
              if k not in ("x", "edge_index")}
    n_cores = 8
    in_maps, meta, perm1 = preprocess(x, edge_index, params, n_cores)
    nc = build_program(meta)
    import os
    trace = bool(int(os.environ.get("K_TRACE", "0")))
    res = run_bass_kernel_spmd(nc, in_maps, list(range(n_cores)),
                               trace=trace)
    global LAST_RES
    LAST_RES = res
    if trace:
        print(f"HW exec time: {res.exec_time_ns} ns", flush=True)
    outs = [res.results[k]["out"] for k in range(n_cores)]
    full = np.concatenate(outs, 0)
    inv = np.argsort(perm1)
    return np.ascontiguousarray(full[:, inv]).astype(np.float32)


# revision 37
# speedup vs baseline: 1.2220x; 1.0431x over previous
"""GATv2 (2-layer, GAT_56727928046275) on 8 TRN2 NeuronCores — v3.

Strategy:
  - Nodes sharded by destination across 8 cores (6250 dst nodes each).
  - Edges (incl. self-loops) partitioned by dst, grouped into 128-dst
    "windows"; per-node softmax + scatter-add stay core-local via
    per-window PSUM accumulation with indicator matmuls.
  - Layer-0 xl table computed LOCALLY IN FULL on every core from a
    replicated x (no AllGather on the critical path); layer-1 table
    still AllGathered (activations are distributed).
  - xl tables carry NO bias: both linear biases are folded into the
    xr side (logits) and the finalize bias (scatter output).
  - Per-edge xl rows fetched with gpsimd dma_gather, full-row payloads
    (512B / 256B).  SWDGE desc-gen (~4.5ns/desc, Pool engine) is the
    floor; ops are 1024-desc, 4-queue round-robin, window-batched on
    layer 1.
  - z = xl[src]+xr[dst] built on the PE (indicator matmul + identity
    accumulate into PSUM); leaky-relu on Scalar (PSUM -> packed bf16);
    DVE does reduces + softmax muls; scatter via indicator matmuls.
  - xr/h tables SBUF-resident between phases; BN stats via masked
    ones-matmul column sums + AllReduce.
"""

import numpy as np
import ml_dtypes

import concourse.bass as bass
import concourse.mybir as mybir
import concourse.tile as tile
from concourse.bacc import Bacc
from concourse.bass_utils import run_bass_kernel_spmd

BF = mybir.dt.bfloat16
F32 = mybir.dt.float32
I16 = mybir.dt.int16
OP = mybir.AluOpType
AF = mybir.ActivationFunctionType
bf16 = ml_dtypes.bfloat16

NEG_SLOPE = 0.2
BN_EPS = 1e-5

import os as _os
B0 = 1   # window batch for layer-0 gathers
B1 = 4   # window batch for layer-1 gathers
# groups per gather op; 8 = 1024 idxs = full SWDGE ring. Smaller ops let
# descriptor GENERATION of op N+1 overlap ring DRAIN of op N.
GCAP = int(_os.environ.get("K_GCAP", "8"))


# ---------------------------------------------------------------- host prep

def _wrap16(a):
    a = np.asarray(a, np.int16).reshape(-1, 16).T  # [16, n/16]
    return np.tile(a, (8, 1))


def _rep(v, dt=np.float32):
    v = np.asarray(v, dt).reshape(1, -1)
    return np.ascontiguousarray(np.broadcast_to(v, (128, v.shape[1])))


def _sign_perm(att):
    H, C = att.shape
    perm = np.zeros(H * C, np.int64)
    npos = np.zeros(H, np.int64)
    for h in range(H):
        a = att[h]
        pos = np.nonzero(a >= 0)[0]
        neg = np.nonzero(a < 0)[0]
        perm[h * C:(h + 1) * C] = h * C + np.concatenate([pos, neg])
        npos[h] = len(pos)
    return perm, npos


def _mkbatches(W, B):
    return [list(range(s, min(s + B, W))) for s in range(0, W, B)]


def preprocess(x, edge_index, params, n_cores=8):
    N, IN = x.shape
    SLICE = N // n_cores
    SP = ((SLICE + 127) // 128) * 128          # padded slice rows (6272)
    W = SP // 128                              # windows per core (49)
    NP = n_cores * SP
    BLK = (n_cores // 2) * SP                  # int16 table split row (25088)

    # PyG's added self-loops are handled by a dense per-window path (no
    # gather descriptors); pre-existing (n,n) edges stay in the lists.
    src = np.asarray(edge_index[0], np.int64)
    dst = np.asarray(edge_index[1], np.int64)
    core = dst // SLICE
    row = (src // SLICE) * SP + (src % SLICE)

    ecnt0 = np.zeros((n_cores, W), np.int64)
    ecnt1 = np.zeros((n_cores, W), np.int64)
    lists = [[None] * W for _ in range(n_cores)]
    for k in range(n_cores):
        m = core == k
        r, d = row[m], dst[m] - k * SLICE
        win = d // 128
        blk = (r >= BLK).astype(np.int64)
        order = np.lexsort((blk, win))
        r, d, win, blk = r[order], d[order], win[order], blk[order]
        for w in range(W):
            wm = win == w
            rw, dw, bw = r[wm], d[wm], blk[wm]
            b0 = bw == 0
            lists[k][w] = (rw[b0], dw[b0], rw[~b0], dw[~b0])
            ecnt0[k][w] = int(b0.sum())
            ecnt1[k][w] = int((~b0).sum())

    G0 = np.maximum(1, (ecnt0.max(0) + 127) // 128)   # [W]
    G1 = np.maximum(1, (ecnt1.max(0) + 127) // 128)
    GW = G0 + G1
    NG = int(GW.sum())

    def build_srcw(batches):
        arr = np.zeros((n_cores, 128, NG * 8), np.int16)
        scs = {}
        sc = 0
        for bi, ws in enumerate(batches):
            scs[bi] = sc
            for k in range(n_cores):
                c = sc
                for w in ws:  # A blocks
                    r0, _, _, _ = lists[k][w]
                    s0 = np.zeros(int(G0[w]) * 128, np.int64)
                    s0[:len(r0)] = r0
                    arr[k][:, c:c + int(G0[w]) * 8] = _wrap16(s0)
                    c += int(G0[w]) * 8
                for w in ws:  # B blocks
                    _, _, r1, _ = lists[k][w]
                    s1 = np.zeros(int(G1[w]) * 128, np.int64)
                    s1[:len(r1)] = r1 - BLK
                    arr[k][:, c:c + int(G1[w]) * 8] = _wrap16(s1)
                    c += int(G1[w]) * 8
            sc += sum(int(GW[w]) for w in ws) * 8
        return arr, scs

    bat0 = _mkbatches(W, B0)
    bat1 = _mkbatches(W, B1)
    srcw0, scs0 = build_srcw(bat0)
    srcw1, scs1 = build_srcw(bat1)

    gcs = [0] * (W + 1)
    for w in range(W):
        gcs[w + 1] = gcs[w] + int(GW[w])
    swin = np.zeros((n_cores, 128, NG * 128), bf16)
    salh = np.zeros((n_cores, 128, NG * 128), bf16)
    for k in range(n_cores):
        for w in range(W):
            r0, d0, r1, d1 = lists[k][w]
            n0, n1 = len(r0), len(r1)
            gw = int(GW[w])
            rl = np.full(gw * 128, -1.0, np.float32)
            rl[:n0] = (d0 - w * 128).astype(np.float32)
            rl[int(G0[w]) * 128:int(G0[w]) * 128 + n1] = (
                d1 - w * 128).astype(np.float32)
            gc = gcs[w]
            sw = (np.arange(128)[:, None] == rl[None, :]).astype(bf16)
            swin[k][:, gc * 128:(gc + gw) * 128] = sw
            se = (rl.reshape(gw, 128).T[:, :, None]
                  == np.arange(128)[None, None, :]).astype(bf16)
            salh[k][:, gc * 128:(gc + gw) * 128] = se.reshape(128, -1)

    p = params
    H, C = p["att0"].shape
    OUTC = p["att1"].shape[1]
    perm0, npos0 = _sign_perm(np.asarray(p["att0"]))
    perm1, npos1 = _sign_perm(np.asarray(p["att1"]))
    aab0 = np.abs(np.asarray(p["att0"]).reshape(-1)[perm0])
    aab1 = np.abs(np.asarray(p["att1"]).reshape(-1)[perm1])

    def scale_cols(Wm, b, perm, aab):
        Wp = np.asarray(Wm)[:, perm] * aab[None, :]
        bp = np.asarray(b)[perm] * aab
        return Wp, bp

    Wl0p, bl0p = scale_cols(p["Wl0"], p["bl0"], perm0, aab0)
    Wr0p, br0p = scale_cols(p["Wr0"], p["br0"], perm0, aab0)
    Wl1p, bl1p = scale_cols(np.asarray(p["Wl1"])[perm0, :], p["bl1"],
                            perm1, aab1)
    Wr1p, br1p = scale_cols(np.asarray(p["Wr1"])[perm0, :], p["br1"],
                            perm1, aab1)

    FEAT0 = H * C            # 192
    TBL0 = 256               # bf16 cols -> 512B row stride
    TBL1 = 128               # bf16 cols -> 256B row stride

    shared = {
        "Wl0p": Wl0p.astype(bf16), "Wr0p": Wr0p.astype(bf16),
        "BSUM0": _rep(bl0p + br0p, bf16),
        "Wl1a": Wl1p[:128].astype(bf16), "Wl1b": Wl1p[128:].astype(bf16),
        "Wr1a": Wr1p[:128].astype(bf16), "Wr1b": Wr1p[128:].astype(bf16),
        "BSUM1": _rep(bl1p + br1p),
        "RA0": _rep(1.0 / aab0), "RA1": _rep(1.0 / aab1),
        "BIAS0": _rep(np.asarray(p["bias0"])[perm0]
                      + np.asarray(p["bl0"])[perm0]),
        "BIAS1": _rep(np.asarray(p["bias1"])[perm1]
                      + np.asarray(p["bl1"])[perm1]),
        "gamma0": np.asarray(p["gamma0"])[perm0].reshape(1, -1).astype(
            np.float32),
        "beta0": np.asarray(p["beta0"])[perm0].reshape(1, -1).astype(
            np.float32),
        "gamma1": np.asarray(p["gamma1"])[perm1].reshape(1, -1).astype(
            np.float32),
        "beta1": np.asarray(p["beta1"])[perm1].reshape(1, -1).astype(
            np.float32),
        "IDENT": np.eye(128, dtype=bf16),
        "ONE1": np.ones((1, 128), np.float32),
    }
    mlast = np.zeros(128, np.float32)
    mlast[:SLICE - (W - 1) * 128] = 1.0
    shared["MASKS"] = np.stack(
        [np.ones(128, np.float32), mlast], 1).astype(bf16)

    # full padded transposed x (same on every core) + local slice
    xt = np.asarray(x).T  # [IN, N]
    xtf = np.zeros((IN, NP), np.float32)
    for k in range(n_cores):
        xtf[:, k * SP:k * SP + SLICE] = xt[:, k * SLICE:(k + 1) * SLICE]
    xtf = xtf.astype(bf16)

    in_maps = []
    for k in range(n_cores):
        m = {"xTf": xtf,
             "xT": np.ascontiguousarray(xtf[:, k * SP:(k + 1) * SP]),
             "srcw0": srcw0[k], "srcw1": srcw1[k],
             "swin": swin[k], "salh": salh[k]}
        for kk, v in shared.items():
            m[kk] = np.ascontiguousarray(v)
        in_maps.append(m)

    meta = dict(N=N, IN=IN, SLICE=SLICE, SP=SP, NP=NP, W=W, BLK=BLK,
                G0=[int(v) for v in G0], G1=[int(v) for v in G1],
                GW=[int(v) for v in GW], NG=NG,
                H=H, C=C, FEAT0=FEAT0, OUTC=OUTC, TBL0=TBL0, TBL1=TBL1,
                npos0=[int(v) for v in npos0], npos1=[int(v) for v in npos1],
                scs0=scs0, scs1=scs1,
                n_cores=n_cores)
    return in_maps, meta, perm1


# ---------------------------------------------------------------- program

def build_program(meta):
    n_cores = meta["n_cores"]
    IN, SP, NP, W = meta["IN"], meta["SP"], meta["NP"], meta["W"]
    G0, G1, GW, NG = meta["G0"], meta["G1"], meta["GW"], meta["NG"]
    H, C, FEAT0 = meta["H"], meta["C"], meta["FEAT0"]
    OUTC, TBL0, TBL1 = meta["OUTC"], meta["TBL0"], meta["TBL1"]
    SLICE = meta["SLICE"]
    BLKROW = (n_cores // 2) * SP
    WF = NP // 128                 # full-table windows (392)
    cores = list(range(n_cores))
    bat0 = _mkbatches(W, B0)
    bat1 = _mkbatches(W, B1)
    gcs = [0] * (W + 1)
    for w in range(W):
        gcs[w + 1] = gcs[w] + GW[w]

    nc = Bacc("TRN2", target_bir_lowering=False, debug=False,
              num_devices=n_cores, num_swdge_queues=4)

    def dp(name, shape, dt):
        return nc.declare_dram_parameter(name, list(shape), dt,
                                         isOutput=False)

    xTf = dp("xTf", [IN, NP], BF)
    xT = dp("xT", [IN, SP], BF)
    srcw0 = dp("srcw0", [128, NG * 8], I16)
    srcw1 = dp("srcw1", [128, NG * 8], I16)
    swin = dp("swin", [128, NG * 128], BF)
    salh = dp("salh", [128, NG * 128], BF)
    Wl0p = dp("Wl0p", [IN, FEAT0], BF)
    Wr0p = dp("Wr0p", [IN, FEAT0], BF)
    BSUM0 = dp("BSUM0", [128, FEAT0], BF)
    Wl1a = dp("Wl1a", [128, OUTC], BF)
    Wl1b = dp("Wl1b", [FEAT0 - 128, OUTC], BF)
    Wr1a = dp("Wr1a", [128, OUTC], BF)
    Wr1b = dp("Wr1b", [FEAT0 - 128, OUTC], BF)
    BSUM1 = dp("BSUM1", [128, OUTC], F32)
    RA0 = dp("RA0", [128, FEAT0], F32)
    RA1 = dp("RA1", [128, OUTC], F32)
    BIAS0 = dp("BIAS0", [128, FEAT0], F32)
    BIAS1 = dp("BIAS1", [128, OUTC], F32)
    gamma0 = dp("gamma0", [1, FEAT0], F32)
    beta0 = dp("beta0", [1, FEAT0], F32)
    gamma1 = dp("gamma1", [1, OUTC], F32)
    beta1 = dp("beta1", [1, OUTC], F32)
    IDENT = dp("IDENT", [128, 128], BF)
    ONE1 = dp("ONE1", [1, 128], F32)
    MASKS = dp("MASKS", [128, 2], BF)

    out = nc.declare_dram_parameter("out", [SLICE, OUTC], F32, isOutput=True)

    # internal DRAM
    XL0 = nc.dram_tensor("XL0", [NP, TBL0], BF)
    XL0loc = nc.dram_tensor("XL0loc", [SP, FEAT0], BF)
    XL1s = nc.dram_tensor("XL1s", [SP, TBL1], BF)
    XL1 = nc.dram_tensor("XL1", [NP, TBL1], BF, addr_space="Shared")
    ST0i = nc.dram_tensor("ST0i", [1, 2 * FEAT0], F32)
    ST0o = nc.dram_tensor("ST0o", [1, 2 * FEAT0], F32, addr_space="Shared")
    ST1i = nc.dram_tensor("ST1i", [1, 2 * OUTC], F32)
    ST1o = nc.dram_tensor("ST1o", [1, 2 * OUTC], F32, addr_space="Shared")

    with tile.TileContext(nc) as tc:
        with tc.tile_pool(name="const", bufs=1) as cp:
            def ld(par, shape, dt):
                t = cp.tile(list(shape), dt, tag=f"c_{par.name}")
                nc.sync.dma_start(out=t[:], in_=par[:])
                return t

            c_ident = ld(IDENT, [128, 128], BF)
            c_one1 = ld(ONE1, [1, 128], F32)
            c_masks = ld(MASKS, [128, 2], BF)
            c_wl0 = ld(Wl0p, [IN, FEAT0], BF)
            c_wr0 = ld(Wr0p, [IN, FEAT0], BF)
            c_bs0 = ld(BSUM0, [128, FEAT0], BF)
            c_wl1a = ld(Wl1a, [128, OUTC], BF)
            c_wl1b = ld(Wl1b, [FEAT0 - 128, OUTC], BF)
            c_wr1a = ld(Wr1a, [128, OUTC], BF)
            c_wr1b = ld(Wr1b, [FEAT0 - 128, OUTC], BF)
            c_bs1 = ld(BSUM1, [128, OUTC], F32)
            c_ra0 = ld(RA0, [128, FEAT0], F32)
            c_ra1 = ld(RA1, [128, OUTC], F32)
            c_bias0 = ld(BIAS0, [128, FEAT0], F32)
            c_bias1 = ld(BIAS1, [128, OUTC], F32)
            c_g0 = ld(gamma0, [1, FEAT0], F32)
            c_b0 = ld(beta0, [1, FEAT0], F32)
            c_g1 = ld(gamma1, [1, OUTC], F32)
            c_b1 = ld(beta1, [1, OUTC], F32)

            # ---------------- BN coeff helper --------------------------
            def bn_coeffs(st_out, feat, g_row, b_row, nodes, reps=True):
                st = cp.tile([1, 2 * feat], F32, tag=f"st{feat}")
                nc.sync.dma_start(out=st[:], in_=st_out[:, :])
                mu = cp.tile([1, feat], F32, tag=f"mu{feat}")
                var = cp.tile([1, feat], F32, tag=f"va{feat}")
                nc.vector.tensor_scalar_mul(mu[:], st[:, 0:feat],
                                            1.0 / nodes)
                nc.vector.tensor_scalar_mul(var[:], st[:, feat:2 * feat],
                                            1.0 / nodes)
                t = cp.tile([1, feat], F32, tag=f"t{feat}")
                nc.vector.tensor_mul(t[:], mu[:], mu[:])
                nc.vector.tensor_sub(var[:], var[:], t[:])
                nc.vector.tensor_scalar_add(var[:], var[:], BN_EPS)
                nc.scalar.activation(t[:], var[:], AF.Sqrt)
                nc.vector.reciprocal(t[:], t[:])
                arow = cp.tile([1, feat], F32, tag=f"ar{feat}")
                nc.vector.tensor_mul(arow[:], g_row[:], t[:])
                brow = cp.tile([1, feat], F32, tag=f"br{feat}")
                nc.vector.tensor_mul(t[:], mu[:], arow[:])
                nc.vector.tensor_sub(brow[:], b_row[:], t[:])
                if not reps:
                    return arow, brow
                with tc.tile_pool(name="bnp", bufs=2, space="PSUM") as bp:
                    pa = bp.tile([128, feat], F32, tag="pa")
                    nc.tensor.matmul(pa[:], c_one1[:], arow[:],
                                     start=True, stop=True)
                    Ar = cp.tile([128, feat], BF, tag=f"A{feat}")
                    nc.scalar.copy(Ar[:], pa[:])
                    pb2 = bp.tile([128, feat], F32, tag="pb")
                    nc.tensor.matmul(pb2[:], c_one1[:], brow[:],
                                     start=True, stop=True)
                    Br = cp.tile([128, feat], BF, tag=f"B{feat}")
                    nc.scalar.copy(Br[:], pb2[:])
                return Ar, Br

            # ---------------- edge pass helper -------------------------
            def edge_pass(XLt, tblc, feat, nh, npos, ra, biast,
                          hp_tile, xr_tile, st_in, srcw_par, batches, scs,
                          xloc, egbufs=3, look=1):
                ssum = cp.tile([1, 2 * feat], F32, tag=f"ssum{feat}")
                nc.vector.memset(ssum[:], 0.0)
                ssum3 = ssum[:].rearrange("p (a f) -> p a f", a=2)
                nb = len(batches)
                aoff = {}
                boff = {}
                bcols = []
                for bi, ws in enumerate(batches):
                    ga = sum(G0[w] for w in ws)
                    o = 0
                    for w in ws:
                        aoff[w] = o
                        o += G0[w]
                    o = ga
                    for w in ws:
                        boff[w] = o
                        o += G1[w]
                    bcols.append(o)
                gmax = max(bcols)

                with tc.tile_pool(name="eg", bufs=egbufs) as eg, \
                     tc.tile_pool(name="esrc", bufs=1) as esrc, \
                     tc.tile_pool(name="esw", bufs=2) as esw, \
                     tc.tile_pool(name="est", bufs=3) as est, \
                     tc.tile_pool(name="exl", bufs=4) as exl, \
                     tc.tile_pool(name="es", bufs=2) as es, \
                     tc.tile_pool(name="es1", bufs=1) as es1, \
                     tc.tile_pool(name="zps", bufs=4, space="PSUM") as zps, \
                     tc.tile_pool(name="eps", bufs=2, space="PSUM") as eps, \
                     tc.tile_pool(name="sps", bufs=1, space="PSUM") as sps:
                    c_src = esrc.tile([128, NG * 8], I16, tag="src")
                    nc.sync.dma_start(out=c_src[:], in_=srcw_par[:])
                    qrr = [0]

                    def gathers(dst_tile, col0, tbl_ap, idx_col0, n_groups):
                        done = 0
                        while done < n_groups:
                            cnt = min(GCAP, n_groups - done)
                            nc.gpsimd.dma_gather(
                                dst_tile[:, col0 + done:col0 + done + cnt,
                                         :],
                                tbl_ap,
                                c_src[:, idx_col0 + done * 8:
                                      idx_col0 + (done + cnt) * 8],
                                num_idxs=cnt * 128, num_idxs_reg=cnt * 128,
                                elem_size=tblc, elem_step=tblc,
                                queue_num=qrr[0])
                            qrr[0] = (qrr[0] + 1) % 4
                            done += cnt

                    def emit_gather(bi):
                        ws = batches[bi]
                        gl = eg.tile([128, gmax, tblc], BF, tag="gl")
                        ga = sum(G0[w] for w in ws)
                        gb = sum(G1[w] for w in ws)
                        sc = scs[bi]
                        gathers(gl, 0, XLt[:, :], sc, ga)
                        gathers(gl, ga, XLt[BLKROW:, :], sc + ga * 8, gb)
                        return gl

                    def emit_streams(w):
                        gw, gc = GW[w], gcs[w]
                        sw = esw.tile([128, gw, 128], BF, tag="sw")
                        nc.sync.dma_start(
                            out=sw[:],
                            in_=swin[:, gc * 128:(gc + gw) * 128])
                        sal = est.tile([128, gw, 128], BF, tag="sal")
                        nc.sync.dma_start(
                            out=sal[:],
                            in_=salh[:, gc * 128:(gc + gw) * 128])
                        xls = exl.tile([128, feat], BF, tag="xls")
                        nc.sync.dma_start(out=xls[:], in_=xloc(w))
                        return sw, sal, xls

                    def emit_chain(w, gl, sw, xls):
                        gw = GW[w]
                        xw = xr_tile[:, w, :]
                        z = es.tile([128, gw, feat], BF, tag="z")
                        ck = 2 if feat > 96 else 4
                        segs = []
                        for j0 in range(0, G0[w], ck):
                            segs.append((j0, aoff[w] + j0,
                                         min(ck, G0[w] - j0)))
                        for j0 in range(0, G1[w], ck):
                            segs.append((G0[w] + j0, boff[w] + j0,
                                         min(ck, G1[w] - j0)))
                        for (zo, go, cnt) in segs:
                            ps = zps.tile([128, ck, feat], F32, tag="zp")
                            for j in range(cnt):
                                nc.tensor.matmul(
                                    ps[:, j, 0:feat], sw[:, zo + j, :],
                                    xw, start=True, stop=False)
                                nc.tensor.matmul(
                                    ps[:, j, 0:feat], c_ident[:],
                                    gl[:, go + j, 0:feat],
                                    start=False, stop=True)
                            nc.scalar.activation(
                                z[:, zo:zo + cnt, :], ps[:, 0:cnt, :],
                                AF.Prelu, alpha=NEG_SLOPE)
                        lp = es.tile([128, gw, nh], BF, tag="lp")
                        ln = es.tile([128, gw, nh], BF, tag="ln")
                        with nc.allow_low_precision(
                                reason="bf16 logit partial sums"):
                            for h in range(nh):
                                k = npos[h]
                                if k == 0:
                                    nc.vector.memset(lp[:, :, h], 0.0)
                                else:
                                    nc.vector.tensor_reduce(
                                        lp[:, :, h],
                                        z[:, :, h * C:h * C + k],
                                        axis=mybir.AxisListType.X,
                                        op=OP.add)
                                if k == C:
                                    nc.vector.memset(ln[:, :, h], 0.0)
                                else:
                                    nc.vector.tensor_reduce(
                                        ln[:, :, h],
                                        z[:, :, h * C + k:(h + 1) * C],
                                        axis=mybir.AxisListType.X,
                                        op=OP.add)
                        rhs = es.tile([128, gw, feat + 4], BF, tag="rhs")
                        lg = es.tile([128, gw, nh], BF, tag="lg")
                        nc.vector.tensor_sub(lg[:], lp[:], ln[:])
                        nc.scalar.activation(
                            rhs[:, :, feat:feat + nh], lg[:], AF.Exp)
                        pe = es1.tile([128, gw, nh * C], BF, tag="pe")
                        nc.scalar.activation(
                            pe[:].rearrange("p g (h c) -> p g h c", c=C),
                            lg[:].unsqueeze(3).broadcast_to(
                                [128, gw, nh, C]),
                            AF.Exp)
                        nc.vector.tensor_mul(
                            rhs[:, 0:G0[w], 0:feat],
                            gl[:, aoff[w]:aoff[w] + G0[w], 0:feat],
                            pe[:, 0:G0[w], :])
                        nc.vector.tensor_mul(
                            rhs[:, G0[w]:gw, 0:feat],
                            gl[:, boff[w]:boff[w] + G1[w], 0:feat],
                            pe[:, G0[w]:gw, :])
                        # dense self-loop path (PyG add_self_loops)
                        zs = es.tile([128, feat], BF, tag="zs")
                        nc.vector.tensor_add(zs[:], xls[:], xw)
                        nc.scalar.activation(zs[:], zs[:], AF.Prelu,
                                             alpha=NEG_SLOPE)
                        lgs = es.tile([128, 2 * nh], BF, tag="lgs")
                        with nc.allow_low_precision(
                                reason="bf16 logit partial sums"):
                            for h in range(nh):
                                k = npos[h]
                                if k == 0:
                                    nc.vector.memset(lgs[:, h:h + 1], 0.0)
                                else:
                                    nc.vector.tensor_reduce(
                                        lgs[:, h:h + 1],
                                        zs[:, h * C:h * C + k],
                                        axis=mybir.AxisListType.X,
                                        op=OP.add)
                                if k == C:
                                    nc.vector.memset(
                                        lgs[:, nh + h:nh + h + 1], 0.0)
                                else:
                                    nc.vector.tensor_reduce(
                                        lgs[:, nh + h:nh + h + 1],
                                        zs[:, h * C + k:(h + 1) * C],
                                        axis=mybir.AxisListType.X,
                                        op=OP.add)
                        lgd = es.tile([128, nh], BF, tag="lgd")
                        nc.vector.tensor_sub(lgd[:], lgs[:, 0:nh],
                                             lgs[:, nh:2 * nh])
                        pself = exl.tile([128, nh], F32, tag="pself")
                        nc.scalar.activation(pself[:], lgd[:], AF.Exp)
                        return rhs, pself

                    def emit_scatter(w, sal, rhs):
                        gw = GW[w]
                        ps = eps.tile([128, feat + nh], F32, tag="acc")
                        for g in range(gw):
                            nc.tensor.matmul(
                                ps[:], sal[:, g, :], rhs[:, g, 0:feat + nh],
                                start=(g == 0), stop=(g == gw - 1))
                        return ps

                    def emit_finalize(w, ps, xls, pself):
                        dn = es.tile([128, nh], F32, tag="dn")
                        nc.vector.tensor_add(dn[:], ps[:, feat:feat + nh],
                                             pself[:])
                        nc.vector.tensor_scalar_max(dn[:], dn[:], 1e-30)
                        rc = es.tile([128, nh], F32, tag="rc")
                        nc.vector.reciprocal(rc[:], dn[:])
                        tmp = es.tile([128, feat], F32, tag="tmp")
                        for h in range(nh):
                            nc.vector.scalar_tensor_tensor(
                                tmp[:, h * C:(h + 1) * C],
                                xls[:, h * C:(h + 1) * C],
                                pself[:, h:h + 1],
                                ps[:, h * C:(h + 1) * C],
                                op0=OP.mult, op1=OP.add)
                            nc.vector.scalar_tensor_tensor(
                                tmp[:, h * C:(h + 1) * C],
                                tmp[:, h * C:(h + 1) * C],
                                rc[:, h:h + 1],
                                ra[:, h * C:(h + 1) * C],
                                op0=OP.mult, op1=OP.mult)
                        nc.vector.tensor_add(tmp[:], tmp[:], biast[:])
                        nc.vector.tensor_scalar_max(hp_tile[:, w, :],
                                                    tmp[:], 0.0)
                        sq = es.tile([128, feat], BF, tag="sq")
                        nc.scalar.activation(sq[:], hp_tile[:, w, :],
                                             AF.Square)
                        mc = 0 if w < W - 1 else 1
                        sp = sps.tile([1, 2, feat], F32, tag="sp")
                        nc.tensor.matmul(sp[:, 0, :], c_masks[:, mc:mc + 1],
                                         hp_tile[:, w, :],
                                         start=True, stop=True)
                        nc.tensor.matmul(sp[:, 1, :], c_masks[:, mc:mc + 1],
                                         sq[:], start=True, stop=True)
                        nc.vector.tensor_add(ssum3, ssum3, sp[:])

                    WB = batches[1][0] - batches[0][0] if nb > 1 else W
                    pend_g = {}
                    pend_s = {}
                    pend_x = {}
                    pend_r = {}
                    pend_e = {}
                    pend_p = {}
                    emitted = 0

                    def need_batches(upto_w):
                        nonlocal emitted
                        while (emitted < nb
                               and batches[emitted][0] <= upto_w):
                            pend_g[emitted] = emit_gather(emitted)
                            emitted += 1

                    for i in range(W + 3):
                        if i < W:
                            need_batches(i + look * WB)
                            sw, sal, xls = emit_streams(i)
                            pend_s[i] = (sw, sal)
                            pend_x[i] = xls
                        if 1 <= i <= W:
                            w = i - 1
                            gl = pend_g[w // WB]
                            sw, sal = pend_s[w]
                            pend_r[w], pend_e[w] = emit_chain(
                                w, gl, sw, pend_x[w])
                        if 2 <= i <= W + 1:
                            w = i - 2
                            _, sal = pend_s.pop(w)
                            pend_p[w] = emit_scatter(w, sal,
                                                     pend_r.pop(w))
                            if (w + 1) % WB == 0 or w == W - 1:
                                pend_g.pop(w // WB, None)
                        if 3 <= i:
                            w = i - 3
                            emit_finalize(w, pend_p.pop(w),
                                          pend_x.pop(w), pend_e.pop(w))
                nc.sync.dma_start(out=st_in[:, :], in_=ssum[:])

            # ======== persistent activation tiles (nested scopes) ======
            with tc.tile_pool(name="actA", bufs=1) as pA:
                XR0s = pA.tile([128, W, FEAT0], BF, tag="XR0s")
                HP0s = pA.tile([128, W, FEAT0], BF, tag="HP0s")

                # ---------------- stage A -----------------------------
                # Full XL0 table computed locally (no AllGather).
                with nc.named_scope("stageA"):
                    with tc.tile_pool(name="sta", bufs=4) as sa, \
                         tc.tile_pool(name="sta_ps", bufs=4,
                                      space="PSUM") as sap:
                        XB = 8
                        for t0 in range(0, WF, XB):
                            xt_t = sa.tile([IN, XB * 128], BF, tag="xt")
                            nc.sync.dma_start(
                                out=xt_t[:],
                                in_=xTf[:, t0 * 128:(t0 + XB) * 128])
                            sb4 = sa.tile([128, XB, FEAT0], BF, tag="sb")
                            for j0 in range(0, XB, 2):
                                ps = sap.tile([128, 2, FEAT0], F32,
                                              tag="ps")
                                for j in range(2):
                                    nc.tensor.matmul(
                                        ps[:, j, :],
                                        xt_t[:, (j0 + j) * 128:
                                             (j0 + j + 1) * 128],
                                        c_wl0[:], start=True, stop=True)
                                if j0 % 4 == 0:
                                    nc.scalar.copy(
                                        sb4[:, j0:j0 + 2, :], ps[:])
                                else:
                                    nc.vector.tensor_copy(
                                        sb4[:, j0:j0 + 2, :], ps[:])
                            dview = XL0[t0 * 128:(t0 + XB) * 128,
                                        0:FEAT0].rearrange(
                                "(i p) c -> p i c", p=128)
                            nc.sync.dma_start(out=dview, in_=sb4[:])
                        # local xr0 slice (SBUF resident) + local xl0
                        # rows (DRAM, for the dense self-loop path)
                        xt_l = sa.tile([IN, SP], BF, tag="xtl")
                        nc.sync.dma_start(out=xt_l[:], in_=xT[:, :])
                        for t in range(W):
                            ps = sap.tile([128, FEAT0], F32, tag="ps")
                            nc.tensor.matmul(
                                ps[:], xt_l[:, t * 128:(t + 1) * 128],
                                c_wr0[:], start=True, stop=True)
                            nc.vector.tensor_add(XR0s[:, t, :], ps[:],
                                                 c_bs0[:])
                            psl = sap.tile([128, FEAT0], F32, tag="ps")
                            nc.tensor.matmul(
                                psl[:], xt_l[:, t * 128:(t + 1) * 128],
                                c_wl0[:], start=True, stop=True)
                            sbl = sa.tile([128, FEAT0], BF, tag="sbl")
                            nc.scalar.copy(sbl[:], psl[:])
                            nc.sync.dma_start(
                                out=XL0loc[t * 128:(t + 1) * 128, :],
                                in_=sbl[:])

                # ================= layer 0 =============================
                with nc.named_scope("l0edge"):
                    edge_pass(XL0, TBL0, FEAT0, H, meta["npos0"], c_ra0,
                              c_bias0, HP0s, XR0s, ST0i, srcw0, bat0,
                              meta["scs0"],
                              lambda w: XL0loc[w * 128:(w + 1) * 128, :],
                              egbufs=5, look=3)

                with tc.tile_pool(name="actB", bufs=1) as pB:
                    XR1s = pB.tile([128, W, OUTC], BF, tag="XR1s")
                    HP1s = pB.tile([128, W, OUTC], BF, tag="HP1s")

                    with tc.tile_pool(name="pht", bufs=1) as pht:
                        HT0 = pht.tile([128, W, 128], BF, tag="HT0")
                        HT1 = pht.tile([64, W, 128], BF, tag="HT1")
                        with nc.named_scope("bn0"):
                            nc.gpsimd.collective_compute(
                                "AllReduce", OP.add, replica_groups=[cores],
                                ins=[ST0i[:, :]], outs=[ST0o[:, :]])
                            # transpose hoist: h^T tiles built while the
                            # AllReduce is in flight (no dependency)
                            with tc.tile_pool(name="p2t", bufs=2,
                                              space="PSUM") as ptp:
                                for w in range(W):
                                    pt0 = ptp.tile([128, 128], BF,
                                                   tag="pt0")
                                    nc.tensor.transpose(
                                        pt0[:], HP0s[:, w, 0:128],
                                        c_ident[:])
                                    nc.scalar.copy(HT0[:, w, :], pt0[:])
                                    pt1 = ptp.tile([64, 128], BF,
                                                   tag="pt1")
                                    nc.tensor.transpose(
                                        pt1[:], HP0s[:, w, 128:192],
                                        c_ident[:])
                                    nc.scalar.copy(HT1[:, w, :], pt1[:])
                            arow0, brow0 = bn_coeffs(ST0o, FEAT0, c_g0,
                                                     c_b0, meta["N"],
                                                     reps=False)

                        with nc.named_scope("pass2"):
                            with tc.tile_pool(name="p2", bufs=3) as p2, \
                                 tc.tile_pool(name="p2c", bufs=1) as p2c, \
                                 tc.tile_pool(name="p2cp", bufs=1,
                                              space="PSUM") as ppc, \
                                 tc.tile_pool(name="p2ps", bufs=2,
                                              space="PSUM") as pp:
                                # BN folded into transposed-side weights:
                                # xl1 = h^T-matmul with A-scaled Wl1 rows
                                # + (brow@Wl1) row; same for Wr1 (+BSUM1).
                                ab = p2c.tile([1, FEAT0], BF, tag="ab")
                                nc.vector.tensor_copy(ab[:], arow0[:])
                                bb = p2c.tile([1, FEAT0], BF, tag="bb")
                                nc.vector.tensor_copy(bb[:], brow0[:])
                                pac = ppc.tile([128, 4], BF, tag="pac")
                                nc.tensor.transpose(pac[:, 0:1],
                                                    ab[:, 0:128],
                                                    c_ident[0:1, 0:1])
                                nc.tensor.transpose(pac[:, 2:3],
                                                    bb[:, 0:128],
                                                    c_ident[0:1, 0:1])
                                pac2 = ppc.tile([64, 4], BF, tag="pac2")
                                nc.tensor.transpose(pac2[:, 0:1],
                                                    ab[:, 128:192],
                                                    c_ident[0:1, 0:1])
                                nc.tensor.transpose(pac2[:, 2:3],
                                                    bb[:, 128:192],
                                                    c_ident[0:1, 0:1])
                                ac = p2c.tile([128, 2], BF, tag="ac")
                                nc.scalar.copy(ac[:, 0:1], pac[:, 0:1])
                                nc.scalar.copy(ac[:, 1:2], pac[:, 2:3])
                                ac2 = p2c.tile([64, 2], BF, tag="ac2")
                                nc.scalar.copy(ac2[:, 0:1], pac2[:, 0:1])
                                nc.scalar.copy(ac2[:, 1:2], pac2[:, 2:3])
                                wl1as = p2c.tile([128, OUTC], BF,
                                                 tag="wl1as")
                                nc.vector.tensor_mul(
                                    wl1as[:], c_wl1a[:],
                                    ac[:, 0:1].broadcast_to([128, OUTC]))
                                wl1bs = p2c.tile([64, OUTC], BF,
                                                 tag="wl1bs")
                                nc.vector.tensor_mul(
                                    wl1bs[:], c_wl1b[:],
                                    ac2[:, 0:1].broadcast_to([64, OUTC]))
                                wr1as = p2c.tile([128, OUTC], BF,
                                                 tag="wr1as")
                                nc.vector.tensor_mul(
                                    wr1as[:], c_wr1a[:],
                                    ac[:, 0:1].broadcast_to([128, OUTC]))
                                wr1bs = p2c.tile([64, OUTC], BF,
                                                 tag="wr1bs")
                                nc.vector.tensor_mul(
                                    wr1bs[:], c_wr1b[:],
                                    ac2[:, 0:1].broadcast_to([64, OUTC]))
                                # bias rows: brow @ Wl1 / Wr1
                                pbc = ppc.tile([1, 2 * OUTC], F32,
                                               tag="pbc")
                                nc.tensor.matmul(pbc[:, 0:OUTC],
                                                 ac[:, 1:2], c_wl1a[:],
                                                 start=True, stop=False)
                                nc.tensor.matmul(pbc[:, 0:OUTC],
                                                 ac2[:, 1:2], c_wl1b[:],
                                                 start=False, stop=True)
                                nc.tensor.matmul(pbc[:, OUTC:2 * OUTC],
                                                 ac[:, 1:2], c_wr1a[:],
                                                 start=True, stop=False)
                                nc.tensor.matmul(pbc[:, OUTC:2 * OUTC],
                                                 ac2[:, 1:2], c_wr1b[:],
                                                 start=False, stop=True)
                                bcrow = p2c.tile([1, 2 * OUTC], F32,
                                                 tag="bcrow")
                                nc.vector.tensor_copy(bcrow[:], pbc[:])
                                pbr = ppc.tile([128, 2 * OUTC], F32,
                                               tag="pbr")
                                nc.tensor.matmul(pbr[:], c_one1[:],
                                                 bcrow[:],
                                                 start=True, stop=True)
                                BCL = p2c.tile([128, OUTC], BF, tag="BCL")
                                nc.scalar.copy(BCL[:], pbr[:, 0:OUTC])
                                BCR = p2c.tile([128, OUTC], F32,
                                               tag="BCR")
                                nc.vector.tensor_add(
                                    BCR[:], pbr[:, OUTC:2 * OUTC],
                                    c_bs1[:])
                                for w in range(W):
                                    px = pp.tile([128, OUTC], F32,
                                                 tag="px")
                                    nc.tensor.matmul(px[:], HT0[:, w, :],
                                                     wl1as[:],
                                                     start=True,
                                                     stop=False)
                                    nc.tensor.matmul(px[:], HT1[:, w, :],
                                                     wl1bs[:],
                                                     start=False,
                                                     stop=True)
                                    xs = p2.tile([128, OUTC], BF,
                                                 tag="xs")
                                    nc.vector.tensor_add(xs[:], px[:],
                                                         BCL[:])
                                    nc.sync.dma_start(
                                        out=XL1s[w * 128:(w + 1) * 128,
                                                 0:OUTC],
                                        in_=xs[:])
                                    px2 = pp.tile([128, OUTC], F32,
                                                  tag="px")
                                    nc.tensor.matmul(px2[:],
                                                     HT0[:, w, :],
                                                     wr1as[:],
                                                     start=True,
                                                     stop=False)
                                    nc.tensor.matmul(px2[:],
                                                     HT1[:, w, :],
                                                     wr1bs[:],
                                                     start=False,
                                                     stop=True)
                                    nc.vector.tensor_add(XR1s[:, w, :],
                                                         px2[:], BCR[:])

                    with nc.named_scope("ag1"):
                        nc.gpsimd.collective_compute(
                            "AllGather", OP.bypass, replica_groups=[cores],
                            ins=[XL1s[:, :]], outs=[XL1[:, :]])

                    # ================= layer 1 =========================
                    with nc.named_scope("l1edge"):
                        edge_pass(XL1, TBL1, OUTC, 1, meta["npos1"],
                                  c_ra1, c_bias1, HP1s, XR1s, ST1i,
                                  srcw1, bat1, meta["scs1"],
                                  lambda w: XL1s[w * 128:(w + 1) * 128,
                                                 0:OUTC],
                                  egbufs=4, look=2)
                    with nc.named_scope("fin"):
                        nc.gpsimd.collective_compute(
                            "AllReduce", OP.add, replica_groups=[cores],
                            ins=[ST1i[:, :]], outs=[ST1o[:, :]])
                        A1, B1t = bn_coeffs(ST1o, OUTC, c_g1, c_b1,
                                            meta["N"])
                        with tc.tile_pool(name="p3", bufs=3) as p3:
                            for w in range(W):
                                rows = min(128, SLICE - w * 128)
                                ob = p3.tile([128, OUTC], F32, tag="ob")
                                nc.vector.tensor_mul(ob[:], HP1s[:, w, :],
                                                     A1[:])
                                nc.vector.tensor_add(ob[:], ob[:], B1t[:])
                                nc.sync.dma_start(
                                    out=out[w * 128:w * 128 + rows, :],
                                    in_=ob[0:rows, :])

    nc.compile()
    return nc


# ---------------------------------------------------------------- entry

def kernel(**inputs):
    x = np.asarray(inputs["x"])
    edge_index = np.asarray(inputs["edge_index"])
    params = {k: np.asarray(v) for k, v in inputs.items()
              if k not in ("x", "edge_index")}
    n_cores = 8
    in_maps, meta, perm1 = preprocess(x, edge_index, params, n_cores)
    nc = build_program(meta)
    import os
    trace = bool(int(os.environ.get("K_TRACE", "0")))
    res = run_bass_kernel_spmd(nc, in_maps, list(range(n_cores)),
                               trace=trace)
    global LAST_RES
    LAST_RES = res
    if trace:
        print(f"HW exec time: {res.exec_time_ns} ns", flush=True)
    outs = [res.results[k]["out"] for k in range(n_cores)]
    full = np.concatenate(outs, 0)
    inv = np.argsort(perm1)
    return np.ascontiguousarray(full[:, inv]).astype(np.float32)
